# revision 1
# baseline (speedup 1.0000x reference)
"""Trainium2 Bass kernel for complex-valued spatial-reduction attention.

x: [B=4, N=2304, C=512] complex64 (re/im f32 planes), H=W=48, 8 heads,
head_dim 64, sr_ratio 2 -> Nk=576.

Sharding: 8 cores = 4 batches x 2 head-groups (4 heads each). Each core:
sr-conv over full C, complex LayerNorm, q/k/v for its heads,
softmax(|q.k^T|) attention, attn @ v, partial output projection.
Host sums the two partials per batch and adds bproj.

Precision: softmax logits reach |a| ~ 37, so the conv -> LN-stats ->
q/kv -> scores chain runs in f32r matmuls (f32-equivalent precision at
bf16-class speed for free dims >= 256, measured on HW). E/v/attn@v/proj
use bf16.
"""

import os
import contextlib

import numpy as np
import ml_dtypes

import concourse.bass as bass
import concourse.mybir as mybir
import concourse.tile as tile
from concourse import bacc
from concourse.masks import make_identity

BF16 = mybir.dt.bfloat16
F16 = mybir.dt.float16
F32 = mybir.dt.float32
F32R = mybir.dt.float32r
AF = mybir.ActivationFunctionType
ALU = mybir.AluOpType

B, N, C, HEADS, HD, SR = 4, 2304, 512, 8, 64, 2
NK = 576
HR = 24
EPS = 1e-5
SCALE = HD ** -0.5  # folded into Wk host-side

M_GROUPS = [
    [(0, 5), (5, 5), (10, 5)],
    [(15, 5), (20, 4)],
]
K_CHUNKS = [(0, 128), (128, 128), (256, 128), (384, 128), (512, 64)]
Q_CHUNKS = [(0, 512), (512, 512), (1024, 512), (1536, 512), (2048, 256)]

DEBUG = bool(int(os.environ.get("KBUILD_DEBUG", "0")))
PHASES = int(os.environ.get("KBUILD_PHASES", "4"))
LNLEVEL = int(os.environ.get("KBUILD_LN", "3"))


def _r(ap):
    return ap.bitcast(F32R)


def build_nc():
    nc = bacc.Bacc("TRN2", target_bir_lowering=False, debug=False, num_devices=8)

    xT_d = nc.dram_tensor("xT", [2, C, N], F32R, kind="ExternalInput")
    xP_d = nc.dram_tensor("xP", [2, 4 * C, NK], F32R, kind="ExternalInput")
    wc_d = nc.dram_tensor("wc", [3, 4 * C, C], F32R, kind="ExternalInput")
    srb_d = nc.dram_tensor("srb", [2, C], F32R, kind="ExternalInput")
    ones_d = nc.dram_tensor("ones", [1, 512], F32R, kind="ExternalInput")
    wq_d = nc.dram_tensor("wq", [3, C, 256], F32R, kind="ExternalInput")
    wk_d = nc.dram_tensor("wk", [3, C, 256], F32R, kind="ExternalInput")
    wv_d = nc.dram_tensor("wv", [3, C, 256], F32R, kind="ExternalInput")
    wp_d = nc.dram_tensor("wp", [3, 256, C], BF16, kind="ExternalInput")
    bkv_d = nc.dram_tensor("bkv", [2, 2, 256], F32R, kind="ExternalInput")
    outT_d = nc.dram_tensor("outT", [2, C, N], F16, kind="ExternalOutput")
    qT_s = nc.dram_tensor("qT_scratch", [2, 2, 128, N], F32R)   # plane, half
    oT_s = nc.dram_tensor("oT_scratch", [2, 2, 128, N], BF16)  # plane, hp
    dbg = {}
    if DEBUG:
        dbg["xnT"] = nc.dram_tensor("dbg_xnT", [2, C, NK], F32, kind="ExternalOutput")
        dbg["qT"] = nc.dram_tensor("dbg_qT", [2, 256, N], F32, kind="ExternalOutput")
        dbg["kT"] = nc.dram_tensor("dbg_kT", [3, 256, NK], F32, kind="ExternalOutput")
        dbg["v"] = nc.dram_tensor("dbg_v", [128, 5 * 4 * 128], BF16, kind="ExternalOutput")
        dbg["dn"] = nc.dram_tensor("dbg_dn", [2, 2, N], F32, kind="ExternalOutput")
        dbg["conv"] = nc.dram_tensor("dbg_conv", [2, NK, C], F32, kind="ExternalOutput")

    with tile.TileContext(nc) as tc:
        _body(nc, tc, xT_d, xP_d, wc_d, srb_d, ones_d, wq_d, wk_d, wv_d, wp_d,
              bkv_d, outT_d, qT_s, oT_s, dbg)

    nc.compile()
    return nc


def _ln_chunk(nc, work, stats, psum_pool, cre, cim, sz):
    """Complex LayerNorm for one [sz, C] chunk in PSUM -> (xnr, xni, re, im)."""
    inv_c = 1.0 / C
    re_sb = work.tile([128, C], F32, tag="ln_re")
    im_sb = work.tile([128, C], F32, tag="ln_im")
    sum_r = stats.tile([128, 1], F32, tag="sum_r")
    sum_i = stats.tile([128, 1], F32, tag="sum_i")
    nc.vector.tensor_copy(re_sb[:sz], cre[:sz, :])
    nc.vector.tensor_copy(im_sb[:sz], cim[:sz, :])
    nc.vector.tensor_reduce(sum_r[:sz], re_sb[:sz], mybir.AxisListType.X, ALU.add)
    nc.vector.tensor_reduce(sum_i[:sz], im_sb[:sz], mybir.AxisListType.X, ALU.add)
    junk = work.tile([128, C], F32, tag="ln_junk", bufs=1)
    sxx = stats.tile([128, 1], F32, tag="sxx")
    sii = stats.tile([128, 1], F32, tag="sii")
    sxi = stats.tile([128, 1], F32, tag="sxi")
    if LNLEVEL < 1:
        return re_sb, im_sb
    nc.vector.tensor_mul(junk[:sz], re_sb[:sz], re_sb[:sz])
    nc.vector.tensor_reduce(sxx[:sz], junk[:sz], mybir.AxisListType.X, ALU.add)
    nc.vector.tensor_mul(junk[:sz], im_sb[:sz], im_sb[:sz])
    nc.vector.tensor_reduce(sii[:sz], junk[:sz], mybir.AxisListType.X, ALU.add)
    nc.vector.tensor_mul(junk[:sz], re_sb[:sz], im_sb[:sz])
    nc.vector.tensor_reduce(sxi[:sz], junk[:sz], mybir.AxisListType.X, ALU.add)
    mr = stats.tile([128, 1], F32, tag="mr")
    mi = stats.tile([128, 1], F32, tag="mi")
    nc.vector.tensor_scalar_mul(mr[:sz], sum_r[:sz], inv_c)
    nc.vector.tensor_scalar_mul(mi[:sz], sum_i[:sz], inv_c)
    vre = stats.tile([128, 1], F32, tag="vre")
    vim = stats.tile([128, 1], F32, tag="vim")
    tA = stats.tile([128, 1], F32, tag="tA")
    tB = stats.tile([128, 1], F32, tag="tB")
    nc.vector.tensor_sub(tA[:sz], sxx[:sz], sii[:sz])
    nc.vector.tensor_scalar_mul(tA[:sz], tA[:sz], inv_c)
    nc.vector.tensor_mul(vre[:sz], mr[:sz], mr[:sz])
    nc.vector.tensor_mul(tB[:sz], mi[:sz], mi[:sz])
    nc.vector.tensor_sub(vre[:sz], vre[:sz], tB[:sz])
    nc.vector.tensor_sub(vre[:sz], tA[:sz], vre[:sz])
    nc.vector.tensor_scalar_add(vre[:sz], vre[:sz], EPS)
    nc.vector.tensor_mul(tB[:sz], mr[:sz], mi[:sz])
    nc.vector.tensor_scalar_mul(tB[:sz], tB[:sz], 2.0)
    nc.vector.tensor_scalar_mul(vim[:sz], sxi[:sz], 2.0 * inv_c)
    nc.vector.tensor_sub(vim[:sz], vim[:sz], tB[:sz])
    if LNLEVEL < 2:
        return re_sb, im_sb
    r2 = stats.tile([128, 1], F32, tag="r2")
    nc.vector.tensor_mul(r2[:sz], vre[:sz], vre[:sz])
    nc.vector.tensor_mul(tB[:sz], vim[:sz], vim[:sz])
    nc.vector.tensor_add(r2[:sz], r2[:sz], tB[:sz])
    def _sqrt_newton(out, x, sc):
        # y0 = LUT sqrt(sc*x); y1 = 0.5*(y0 + sc*x/y0)  (one Newton step)
        y0 = stats.tile([128, 1], F32, tag="nw_y0")
        nc.scalar.activation(y0[:sz], x[:sz], AF.Sqrt, scale=sc)
        yr = stats.tile([128, 1], F32, tag="nw_yr")
        nc.vector.tensor_scalar_add(y0[:sz], y0[:sz], 1e-30)
        nc.vector.reciprocal(yr[:sz], y0[:sz])
        nc.vector.tensor_mul(yr[:sz], yr[:sz], x[:sz])
        if sc != 1.0:
            nc.vector.tensor_scalar_mul(yr[:sz], yr[:sz], sc)
        nc.vector.tensor_add(out[:sz], y0[:sz], yr[:sz])
        nc.vector.tensor_scalar_mul(out[:sz], out[:sz], 0.5)

    rr = stats.tile([128, 1], F32, tag="rr")
    _sqrt_newton(rr, r2, 1.0)
    srt = stats.tile([128, 1], F32, tag="srt")
    sia = stats.tile([128, 1], F32, tag="sia")
    nc.vector.tensor_add(tA[:sz], rr[:sz], vre[:sz])
    _sqrt_newton(srt, tA, 0.5)
    nc.vector.tensor_sub(tA[:sz], rr[:sz], vre[:sz])
    _sqrt_newton(sia, tA, 0.5)
    sgn = stats.tile([128, 1], F32, tag="sgn")
    nc.scalar.activation(sgn[:sz], vim[:sz], AF.Sign)
    nc.vector.tensor_mul(sia[:sz], sia[:sz], sgn[:sz])
    rin = stats.tile([128, 1], F32, tag="rin")
    nc.vector.reciprocal(rin[:sz], rr[:sz])
    wr = stats.tile([128, 1], F32, tag="wr")
    wn = stats.tile([128, 1], F32, tag="wn")  # = -w_im
    nc.vector.tensor_mul(wr[:sz], srt[:sz], rin[:sz])
    nc.vector.tensor_mul(wn[:sz], sia[:sz], rin[:sz])
    if LNLEVEL < 3:
        return re_sb, im_sb
    aT = work.tile([128, C], F32, tag="ln_a")
    bT = work.tile([128, C], F32, tag="ln_b")
    xnr = work.tile([128, C], F32, tag="ln_xnr")
    xni = work.tile([128, C], F32, tag="ln_xni")
    nc.vector.tensor_scalar(aT[:sz], re_sb[:sz], mr[:sz], wr[:sz],
                            ALU.subtract, ALU.mult)
    nc.vector.tensor_scalar(bT[:sz], im_sb[:sz], mi[:sz], wn[:sz],
                            ALU.subtract, ALU.mult)
    nc.vector.tensor_add(xnr[:sz], aT[:sz], bT[:sz])
    nc.vector.tensor_scalar(aT[:sz], re_sb[:sz], mr[:sz], wn[:sz],
                            ALU.subtract, ALU.mult)
    nc.vector.tensor_scalar(bT[:sz], im_sb[:sz], mi[:sz], wr[:sz],
                            ALU.subtract, ALU.mult)
    nc.vector.tensor_sub(xni[:sz], bT[:sz], aT[:sz])
    return xnr, xni


def _body(nc, tc, xT_d, xP_d, wc_d, srb_d, ones_d, wq_d, wk_d, wv_d, wp_d,
          bkv_d, outT_d, qT_s, oT_s, dbg):
    ctx = contextlib.ExitStack()
    consts = ctx.enter_context(tc.tile_pool(name="consts", bufs=1))
    big = ctx.enter_context(tc.tile_pool(name="big", bufs=1))
    xs = ctx.enter_context(tc.tile_pool(name="xs", bufs=4))
    qs = ctx.enter_context(tc.tile_pool(name="qs", bufs=2))
    ws = ctx.enter_context(tc.tile_pool(name="ws", bufs=2))
    wcp = ctx.enter_context(tc.tile_pool(name="wcp", bufs=2))
    work = ctx.enter_context(tc.tile_pool(name="work", bufs=2))
    sm = ctx.enter_context(tc.tile_pool(name="sm", bufs=2))
    stats = ctx.enter_context(tc.tile_pool(name="stats", bufs=2))
    psum = ctx.enter_context(tc.tile_pool(name="psum", bufs=7, space="PSUM"))

    # ---- constants ----
    ident = consts.tile([128, 128], F32, tag="ident")
    make_identity(nc, ident)
    ones_col = consts.tile([128, 1], BF16, tag="ones_col")
    nc.vector.memset(ones_col, 1.0)
    ones_row = consts.tile([1, 512], F32R, tag="ones_row")
    nc.sync.dma_start(ones_row[:], ones_d[:])
    nbias = consts.tile([128, 1], F32, tag="nbias")
    nc.vector.memset(nbias, -50.0)

    srb_re = consts.tile([1, C], F32R, tag="srb_re")
    srb_im = consts.tile([1, C], F32R, tag="srb_im")
    nc.sync.dma_start(srb_re[:], srb_d[0:1, :])
    nc.sync.dma_start(srb_im[:], srb_d[1:2, :])
    bk_re = consts.tile([1, 256], F32R, tag="bk_re")
    bk_im = consts.tile([1, 256], F32R, tag="bk_im")
    bv_re = consts.tile([1, 256], F32R, tag="bv_re")
    bv_im = consts.tile([1, 256], F32R, tag="bv_im")
    nc.sync.dma_start(bk_re[:], bkv_d[0:1, 0, :])
    nc.sync.dma_start(bv_re[:], bkv_d[0:1, 1, :])
    nc.sync.dma_start(bk_im[:], bkv_d[1:2, 0, :])
    nc.sync.dma_start(bv_im[:], bkv_d[1:2, 1, :])

    # ---- persistent SBUF ----
    xnTr = big.tile([128, 4, NK], F32R, tag="xnTr")
    xnTi = big.tile([128, 4, NK], F32R, tag="xnTi")
    kTr = big.tile([128, 2, NK], F32R, tag="kTr")
    kTi = big.tile([128, 2, NK], F32R, tag="kTi")
    kTin = big.tile([128, 2, NK], F32R, tag="kTin")
    vpk = big.tile([128, 5, 4, 128], BF16, tag="vpk")
    if DEBUG:
        nc.vector.memset(vpk, 0.0)

    xT_v = [xT_d[pl].rearrange("(j p) t -> p j t", p=128) for pl in (0, 1)]

    # =====================================================================
    # Phase 1: conv (f32r) + LayerNorm + transposes + q-projection
    # =====================================================================
    for mg in M_GROUPS:
        tg0 = mg[0][0] * HR
        tgs = sum(nh for _, nh in mg) * HR
        cps = []
        for (hr0, nh) in mg:
            cre = psum.tile([128, C], F32, tag="bank")
            cim = psum.tile([128, C], F32, tag="bank")
            cps.append((cre, cim, hr0 * HR - tg0, hr0 * HR, nh * HR))

        first = [[True, True] for _ in mg]
        for kk in range(16):
            xp_r = xs.tile([128, 3 * 120], F32R, tag="xp_r")
            xp_i = xs.tile([128, 3 * 120], F32R, tag="xp_i")
            nc.gpsimd.dma_start(xp_r[:, :tgs], xP_d[0, 128 * kk:128 * (kk + 1),
                                                    tg0:tg0 + tgs])
            nc.sync.dma_start(xp_i[:, :tgs], xP_d[1, 128 * kk:128 * (kk + 1),
                                                  tg0:tg0 + tgs])
            w_re = wcp.tile([128, C], F32R, tag="wc_re")
            w_im = wcp.tile([128, C], F32R, tag="wc_im")
            w_in = wcp.tile([128, C], F32R, tag="wc_in")
            nc.gpsimd.dma_start(w_re[:], wc_d[0, 128 * kk:128 * (kk + 1), :])
            nc.sync.dma_start(w_im[:], wc_d[1, 128 * kk:128 * (kk + 1), :])
            nc.gpsimd.dma_start(w_in[:], wc_d[2, 128 * kk:128 * (kk + 1), :])
            for mi_, (cre, cim, off, t0, sz) in enumerate(cps):
                pat_r = xp_r[:, off:off + sz]
                pat_i = xp_i[:, off:off + sz]
                nc.tensor.matmul(cre[:sz, :], _r(pat_r), _r(w_re[:]),
                                 start=first[mi_][0], stop=False)
                nc.tensor.matmul(cim[:sz, :], _r(pat_r), _r(w_im[:]),
                                 start=first[mi_][1], stop=False)
                first[mi_] = [False, False]
                nc.tensor.matmul(cre[:sz, :], _r(pat_i), _r(w_in[:]),
                                 start=False, stop=False)
                nc.tensor.matmul(cim[:sz, :], _r(pat_i), _r(w_re[:]),
                                 start=False, stop=False)
        for (cre, cim, off, t0, sz) in cps:
            nc.tensor.matmul(cre[:sz, :], _r(ones_row[:, :sz]), _r(srb_re[:]),
                             start=False, stop=True)
            nc.tensor.matmul(cim[:sz, :], _r(ones_row[:, :sz]), _r(srb_im[:]),
                             start=False, stop=True)
        if PHASES < 1:
            for (cre, cim, off, t0, sz) in cps:
                dmp = work.tile([128, C], F32, tag="ln_a")
                nc.vector.tensor_copy(dmp[:sz], cre[:sz, :])
                dmp2 = work.tile([128, C], F32, tag="ln_b")
                nc.vector.tensor_copy(dmp2[:sz], cim[:sz, :])

        # ---- LayerNorm + transpose into xnT ----
        for (cre, cim, off, t0, sz) in (cps if PHASES >= 1 else []):
            if DEBUG:
                for tt, src_ in ((0, cre), (1, cim)):
                    cdbg = work.tile([128, C], F32, tag="ln_a")
                    nc.vector.tensor_copy(cdbg[:sz], src_[:sz, :])
                    nc.sync.dma_start(dbg["conv"][tt, t0:t0 + sz, :], cdbg[:sz])
            xnr, xni = _ln_chunk(nc, work, stats, psum, cre, cim, sz)
            for cj in range(4):
                for src, dst in ((xnr, xnTr), (xni, xnTi)):
                    pt = psum.tile([128, 128], F32, tag="bank")
                    nc.tensor.transpose(pt[:, :sz],
                                        src[:sz, 128 * cj:128 * (cj + 1)],
                                        ident[:sz, :sz])
                    nc.vector.tensor_copy(dst[:, cj, t0:t0 + sz], pt[:, :sz])

    # =====================================================================
    # Phase 1b: q-projection (f32r), x^T re-streamed per chunk
    # =====================================================================
    for (q0, nq) in (Q_CHUNKS if PHASES >= 2 else []):
        prs = []
        for half in range(2):
            prs.append((psum.tile([128, 512], F32, tag="bank", name=f"qpr{half}"),
                        psum.tile([128, 512], F32, tag="bank", name=f"qpi{half}")))
        for cj in range(4):
            xq_r = qs.tile([128, 512], F32R, tag="xq_r", bufs=1)
            xq_i = qs.tile([128, 512], F32R, tag="xq_i", bufs=1)
            nc.gpsimd.dma_start(xq_r[:, :nq], xT_v[0][:, cj, q0:q0 + nq])
            nc.sync.dma_start(xq_i[:, :nq], xT_v[1][:, cj, q0:q0 + nq])
            wq_r = ws.tile([128, 256], F32R, tag="w_r")
            wq_i = ws.tile([128, 256], F32R, tag="w_i")
            wq_n = ws.tile([128, 256], F32R, tag="w_n")
            nc.sync.dma_start(wq_r[:], wq_d[0, 128 * cj:128 * (cj + 1), :])
            nc.sync.dma_start(wq_i[:], wq_d[1, 128 * cj:128 * (cj + 1), :])
            nc.sync.dma_start(wq_n[:], wq_d[2, 128 * cj:128 * (cj + 1), :])
            st = cj == 0
            sp = cj == 3
            for half in range(2):
                hs = slice(128 * half, 128 * (half + 1))
                pr, pi = prs[half]
                nc.tensor.matmul(pr[:, :nq], _r(wq_r[:, hs]), _r(xq_r[:, :nq]),
                                 start=st, stop=False)
                nc.tensor.matmul(pr[:, :nq], _r(wq_n[:, hs]), _r(xq_i[:, :nq]),
                                 start=False, stop=sp)
                nc.tensor.matmul(pi[:, :nq], _r(wq_i[:, hs]), _r(xq_r[:, :nq]),
                                 start=st, stop=False)
                nc.tensor.matmul(pi[:, :nq], _r(wq_r[:, hs]), _r(xq_i[:, :nq]),
                                 start=False, stop=sp)
        for half in range(2):
            pr, pi = prs[half]
            o1 = work.tile([128, 512], F32R, tag="cp_r")
            o2 = work.tile([128, 512], F32R, tag="cp_i")
            nc.scalar.copy(o1[:, :nq], pr[:, :nq])
            nc.scalar.copy(o2[:, :nq], pi[:, :nq])
            nc.sync.dma_start(qT_s[0, half, :, q0:q0 + nq], o1[:, :nq])
            nc.sync.dma_start(qT_s[1, half, :, q0:q0 + nq], o2[:, :nq])
            if DEBUG:
                hs = slice(128 * half, 128 * (half + 1))
                nc.sync.dma_start(dbg["qT"][0, hs, q0:q0 + nq], o1[:, :nq].bitcast(F32))
                nc.sync.dma_start(dbg["qT"][1, hs, q0:q0 + nq], o2[:, :nq].bitcast(F32))

    if DEBUG:
        for cj in range(4):
            nc.sync.dma_start(dbg["xnT"][0, 128 * cj:128 * (cj + 1), :], xnTr[:, cj, :].bitcast(F32))
            nc.sync.dma_start(dbg["xnT"][1, 128 * cj:128 * (cj + 1), :], xnTi[:, cj, :].bitcast(F32))

    # =====================================================================
    # Phase 2: k^T and v projections (f32r)
    # =====================================================================
    for half in (range(2) if PHASES >= 2 else []):
        hs = slice(128 * half, 128 * (half + 1))
        p512r = psum.tile([128, 512], F32, tag="bank")
        p512i = psum.tile([128, 512], F32, tag="bank")
        p64r = psum.tile([128, 512], F32, tag="bank")
        p64i = psum.tile([128, 512], F32, tag="bank")
        for cj in range(4):
            wk_r = ws.tile([128, 256], F32R, tag="w_r")
            wk_i = ws.tile([128, 256], F32R, tag="w_i")
            wk_n = ws.tile([128, 256], F32R, tag="w_n")
            nc.sync.dma_start(wk_r[:], wk_d[0, 128 * cj:128 * (cj + 1), :])
            nc.sync.dma_start(wk_i[:], wk_d[1, 128 * cj:128 * (cj + 1), :])
            nc.sync.dma_start(wk_n[:], wk_d[2, 128 * cj:128 * (cj + 1), :])
            st = cj == 0
            for (pr, pi, n0, nn) in ((p512r, p512i, 0, 512), (p64r, p64i, 512, 64)):
                nc.tensor.matmul(pr[:, :nn], _r(wk_r[:, hs]),
                                 _r(xnTr[:, cj, n0:n0 + nn]), start=st, stop=False)
                nc.tensor.matmul(pr[:, :nn], _r(wk_n[:, hs]),
                                 _r(xnTi[:, cj, n0:n0 + nn]), start=False, stop=False)
                nc.tensor.matmul(pi[:, :nn], _r(wk_i[:, hs]),
                                 _r(xnTr[:, cj, n0:n0 + nn]), start=st, stop=False)
                nc.tensor.matmul(pi[:, :nn], _r(wk_r[:, hs]),
                                 _r(xnTi[:, cj, n0:n0 + nn]), start=False, stop=False)
        for (pr, pi, n0, nn) in ((p512r, p512i, 0, 512), (p64r, p64i, 512, 64)):
            nc.tensor.matmul(pr[:, :nn], _r(bk_re[:, hs]), _r(ones_row[:, :nn]),
                             start=False, stop=True)
            nc.tensor.matmul(pi[:, :nn], _r(bk_im[:, hs]), _r(ones_row[:, :nn]),
                             start=False, stop=True)
            nc.vector.tensor_copy(kTr[:, half, n0:n0 + nn], pr[:, :nn])
            nc.vector.tensor_copy(kTi[:, half, n0:n0 + nn], pi[:, :nn])
            nc.vector.tensor_scalar_mul(kTin[:, half, n0:n0 + nn], pi[:, :nn], -1.0)

    for kcg in (((0, 1, 2), (3, 4)) if PHASES >= 2 else ()):
        pps = {}
        for kc in kcg:
            pps[kc] = (psum.tile([128, 512], F32, tag="bank", name=f"vpr{kc}"),
                       psum.tile([128, 512], F32, tag="bank", name=f"vpi{kc}"))
        for cj in range(4):
            wv_r = ws.tile([128, 256], F32R, tag="w_r")
            wv_i = ws.tile([128, 256], F32R, tag="w_i")
            wv_n = ws.tile([128, 256], F32R, tag="w_n")
            nc.sync.dma_start(wv_r[:], wv_d[0, 128 * cj:128 * (cj + 1), :])
            nc.sync.dma_start(wv_i[:], wv_d[1, 128 * cj:128 * (cj + 1), :])
            nc.sync.dma_start(wv_n[:], wv_d[2, 128 * cj:128 * (cj + 1), :])
            st = cj == 0
            for kc in kcg:
                k0, szk = K_CHUNKS[kc]
                pr, pi = pps[kc]
                nc.tensor.matmul(pr[:szk, :256], _r(xnTr[:, cj, k0:k0 + szk]),
                                 _r(wv_r[:]), start=st, stop=False)
                nc.tensor.matmul(pr[:szk, :256], _r(xnTi[:, cj, k0:k0 + szk]),
                                 _r(wv_n[:]), start=False, stop=False)
                nc.tensor.matmul(pi[:szk, :256], _r(xnTr[:, cj, k0:k0 + szk]),
                                 _r(wv_i[:]), start=st, stop=False)
                nc.tensor.matmul(pi[:szk, :256], _r(xnTi[:, cj, k0:k0 + szk]),
                                 _r(wv_r[:]), start=False, stop=False)
        for kc in kcg:
            k0, szk = K_CHUNKS[kc]
            pr, pi = pps[kc]
            nc.tensor.matmul(pr[:szk, :256], _r(ones_row[:, :szk]), _r(bv_re[:]),
                             start=False, stop=True)
            nc.tensor.matmul(pi[:szk, :256], _r(ones_row[:, :szk]), _r(bv_im[:]),
                             start=False, stop=True)
            vr_v = pr[:szk, :256].rearrange("p (h d) -> p h d", h=4)
            vi_v = pi[:szk, :256].rearrange("p (h d) -> p h d", h=4)
            nc.vector.tensor_copy(vpk[:szk, kc, :, 0:64], vr_v)
            nc.vector.tensor_copy(vpk[:szk, kc, :, 64:128], vi_v)

    if DEBUG:
        for half in range(2):
            hs = slice(128 * half, 128 * (half + 1))
            nc.sync.dma_start(dbg["kT"][0, hs, :], kTr[:, half, :].bitcast(F32))
            nc.sync.dma_start(dbg["kT"][1, hs, :], kTi[:, half, :].bitcast(F32))
            nc.sync.dma_start(dbg["kT"][2, hs, :], kTin[:, half, :].bitcast(F32))
        nc.sync.dma_start(dbg["v"][:, :], vpk.rearrange("p a b c -> p (a b c)"))

    # =====================================================================
    # Phase 3: attention (S^T layout, f32r scores), softmax over |.|
    # =====================================================================
    def emit_front(q0, nq, hp, qTr_s, qTi_s):
        ebufs = {}
        for kcg in ((0, 1, 2), (3, 4)):
            sbufs = {}
            for kc in kcg:
                k0, szk = K_CHUNKS[kc]
                sbuf = sm.tile([128, 2, 512], F16, tag="sbuf", name=f"sb{kc}",
                               bufs=3)
                for i in range(2):
                    rs = slice(64 * i, 64 * (i + 1))
                    sre = psum.tile([128, 512], F32, tag="bank")
                    sim = psum.tile([128, 512], F32, tag="bank")
                    nc.tensor.matmul(sre[:szk, :nq], _r(kTr[rs, hp, k0:k0 + szk]),
                                     _r(qTr_s[rs, hp, :nq]), start=True, stop=False)
                    nc.tensor.matmul(sim[:szk, :nq], _r(kTi[rs, hp, k0:k0 + szk]),
                                     _r(qTr_s[rs, hp, :nq]), start=True, stop=False)
                    nc.tensor.matmul(sre[:szk, :nq], _r(kTin[rs, hp, k0:k0 + szk]),
                                     _r(qTi_s[rs, hp, :nq]), start=False, stop=True)
                    nc.tensor.matmul(sim[:szk, :nq], _r(kTr[rs, hp, k0:k0 + szk]),
                                     _r(qTi_s[rs, hp, :nq]), start=False, stop=True)
                    s1 = sm.tile([128, 512], F16, tag="s1")
                    nc.scalar.activation(s1[:szk, :nq], sre[:szk, :nq], AF.Square)
                    c2 = sm.tile([128, 512], F16, tag="c2")
                    nc.vector.tensor_copy(c2[:szk, :nq], sim[:szk, :nq])
                    s2 = sm.tile([128, 512], F16, tag="s2")
                    nc.vector.tensor_mul(s2[:szk, :nq], c2[:szk, :nq],
                                         c2[:szk, :nq])
                    nc.vector.tensor_add(sbuf[:szk, i, :nq], s1[:szk, :nq],
                                         s2[:szk, :nq])
                sbufs[kc] = sbuf
            ubs = {}
            for kc in kcg:
                k0, szk = K_CHUNKS[kc]
                ub = sm.tile([128, 2, 512], F32, tag="ubuf", name=f"ub{kc}",
                             bufs=3)
                nc.scalar.activation(ub[:szk, :, :nq], sbufs[kc][:szk, :, :nq],
                                     AF.Ln)
                ubs[kc] = ub
            for kc in kcg:
                k0, szk = K_CHUNKS[kc]
                ub = ubs[kc]
                nc.scalar.activation(ub[:szk, :, :nq], ub[:szk, :, :nq], AF.Exp,
                                     scale=0.5)
                ebuf = sm.tile([128, 2, 512], BF16, tag="ebuf", name=f"eb{kc}",
                               bufs=10)
                # constant shift keeps exp sums in f32 range; softmax is
                # shift-invariant so the result is exact
                nc.scalar.activation(ebuf[:szk, :, :nq], ub[:szk, :, :nq],
                                     AF.Exp, bias=nbias[:szk])
                ebufs[kc] = ebuf
        return ebufs

    def emit_back(q0, nq, hp, ebufs):
        op0 = psum.tile([128, 512], F32, tag="bank", name="op0")
        op1 = psum.tile([128, 512], F32, tag="bank", name="op1")
        dn = psum.tile([128, 512], F32, tag="bank", name="dn")
        for kc in range(5):
            k0, szk = K_CHUNKS[kc]
            ebuf = ebufs[kc]
            for i in range(2):
                hh = 2 * hp + i
                opt = op0 if i == 0 else op1
                nc.tensor.matmul(opt[:, :nq], vpk[:szk, kc, hh, :],
                                 ebuf[:szk, i, :nq], start=kc == 0, stop=kc == 4)
                nc.tensor.matmul(dn[32 * i:32 * i + 1, :nq], ones_col[:szk, :],
                                 ebuf[:szk, i, :nq], start=kc == 0, stop=kc == 4,
                                 tile_position=(0, 32 * i))
        otr = sm.tile([128, 512], BF16, tag="otr")
        oti = sm.tile([128, 512], BF16, tag="oti")
        # drain attn-out + denominators out of PSUM right away so the next
        # iteration's score matmuls get banks immediately
        ops_sb = []
        for i, opt in enumerate((op0, op1)):
            osb = sm.tile([128, 512], F32, tag="opsb", name=f"opsb{i}")
            nc.vector.tensor_copy(osb[:, :nq], opt[:, :nq])
            ops_sb.append(osb)
        rhs_t = []
        for i in range(2):
            rh = stats.tile([1, 512], F32R, tag="lnd", name=f"rh{i}")
            nc.scalar.activation(rh[:, :nq], dn[32 * i:32 * i + 1, :nq], AF.Ln)
            rhs_t.append(rh)
        for i in range(2):
            rh = rhs_t[i]
            nc.scalar.activation(rh[:, :nq], rh[:, :nq], AF.Exp, scale=-1.0)
            rbp = psum.tile([128, 512], F32, tag="bank")
            nc.tensor.matmul(rbp[:, :nq], _r(ones_row[:1, :128]), _r(rh[:, :nq]),
                             start=True, stop=True)
            rb = sm.tile([128, 512], F32, tag="rb", bufs=1)
            nc.vector.tensor_copy(rb[:, :nq], rbp[:, :nq])
            osb = ops_sb[i]
            rs = slice(64 * i, 64 * (i + 1))
            nc.vector.tensor_mul(otr[rs, :nq], osb[0:64, :nq], rb[0:64, :nq])
            nc.vector.tensor_mul(oti[rs, :nq], osb[64:128, :nq], rb[64:128, :nq])
        nc.sync.dma_start(oT_s[0, hp, :, q0:q0 + nq], otr[:, :nq])
        nc.gpsimd.dma_start(oT_s[1, hp, :, q0:q0 + nq], oti[:, :nq])

    # software pipeline: iteration i+1's score matmuls are emitted before
    # iteration i's attn@v/normalize tail, so the PE never waits on the
    # softmax ACT chain
    prev = None
    for (q0, nq) in (Q_CHUNKS if PHASES >= 3 else []):
        qTr_s = qs.tile([128, 2, 512], F32R, tag="qTr_s")
        qTi_s = qs.tile([128, 2, 512], F32R, tag="qTi_s")
        for half in range(2):
            for c4 in range(2):
                s0 = 256 * c4
                w = min(256, nq - s0)
                if w > 0:
                    nc.gpsimd.dma_start(qTr_s[:, half, s0:s0 + w],
                                        qT_s[0, half, :, q0 + s0:q0 + s0 + w])
                    nc.sync.dma_start(qTi_s[:, half, s0:s0 + w],
                                      qT_s[1, half, :, q0 + s0:q0 + s0 + w])
        for hp in range(2):
            ebufs = emit_front(q0, nq, hp, qTr_s, qTi_s)
            if prev is not None:
                emit_back(*prev)
            prev = (q0, nq, hp, ebufs)
    if prev is not None:
        emit_back(*prev)

    # =====================================================================
    # Phase 4: partial output projection (bf16)
    # =====================================================================
    wp_sb = big.tile([128, 3, 2, C], BF16, tag="wp")
    nc.sync.dma_start(wp_sb[:], wp_d.rearrange("s (j p) n -> p s j n", p=128))
    for (q0, nq) in (Q_CHUNKS if PHASES >= 4 else []):
        oTr_l = qs.tile([128, 2, 512], BF16, tag="oTr_l")
        oTi_l = qs.tile([128, 2, 512], BF16, tag="oTi_l")
        for hp in range(2):
            nc.sync.dma_start(oTr_l[:, hp, :nq], oT_s[0, hp, :, q0:q0 + nq])
            nc.sync.dma_start(oTi_l[:, hp, :nq], oT_s[1, hp, :, q0:q0 + nq])
        for cc in range(4):
            cs = slice(128 * cc, 128 * (cc + 1))
            pr = psum.tile([128, 512], F32, tag="bank")
            pi = psum.tile([128, 512], F32, tag="bank")
            for half in range(2):
                st = half == 0
                sp = half == 1
                nc.tensor.matmul(pr[:, :nq], wp_sb[:, 0, half, cs],
                                 oTr_l[:, half, :nq], start=st, stop=False)
                nc.tensor.matmul(pr[:, :nq], wp_sb[:, 2, half, cs],
                                 oTi_l[:, half, :nq], start=False, stop=sp)
                nc.tensor.matmul(pi[:, :nq], wp_sb[:, 1, half, cs],
                                 oTr_l[:, half, :nq], start=st, stop=False)
                nc.tensor.matmul(pi[:, :nq], wp_sb[:, 0, half, cs],
                                 oTi_l[:, half, :nq], start=False, stop=sp)
            o1 = work.tile([128, 512], F16, tag="cp_r16")
            o2 = work.tile([128, 512], F16, tag="cp_i16")
            nc.vector.tensor_copy(o1[:, :nq], pr[:, :nq])
            nc.vector.tensor_copy(o2[:, :nq], pi[:, :nq])
            nc.gpsimd.dma_start(outT_d[0, cs, q0:q0 + nq], o1[:, :nq])
            nc.sync.dma_start(outT_d[1, cs, q0:q0 + nq], o2[:, :nq])

    ctx.close()


# =========================================================================
# Host side
# =========================================================================

def _f32(x):
    return np.ascontiguousarray(x, dtype=np.float32)


def _bf(x):
    return np.asarray(x, dtype=ml_dtypes.bfloat16)


def host_prep(x_re, x_im, Wq, Wkv, Wproj, bproj, sr_w, sr_b, gain, bias):
    x_re = np.asarray(x_re)
    x_im = np.asarray(x_im)
    Wq = np.asarray(Wq)
    Wkv = np.asarray(Wkv)
    Wproj = np.asarray(Wproj)
    sr_w = np.asarray(sr_w)
    sr_b = np.asarray(sr_b)
    gain = np.asarray(gain)
    bias = np.asarray(bias)

    Wkv_eff = gain[:, None] * Wkv
    bkv_full = bias @ Wkv
    Wc = sr_w.transpose(2, 3, 1, 0).reshape(4 * C, C)

    def planes3f(w):
        return np.stack([_f32(w.real), _f32(w.imag), _f32(-w.imag)])

    def planes3b(w):
        return np.stack([_bf(w.real), _bf(w.imag), _bf(-w.imag)])

    in_maps = []
    for core in range(8):
        b, g = core // 2, core % 2
        cols = slice(256 * g, 256 * (g + 1))
        wk_c = Wkv_eff[:, :C][:, cols] * SCALE
        wv_c = Wkv_eff[:, C:][:, cols]
        bk_c = bkv_full[:C][cols] * SCALE
        bv_c = bkv_full[C:][cols]
        xs_c = np.stack([x_re[b].T, x_im[b].T])  # [2, C, N]
        xsp = xs_c.reshape(2, C, HR, 2, HR, 2)
        xP = np.stack([xsp[:, :, :, p, :, q].reshape(2, C, NK)
                       for p in range(2) for q in range(2)], axis=1)
        m = {
            "xT": _f32(xs_c),
            "xP": _f32(xP.reshape(2, 4 * C, NK)),
            "wc": planes3f(Wc),
            "srb": np.stack([_f32(sr_b.real), _f32(sr_b.imag)]),
            "ones": np.ones((1, 512), np.float32),
            "wq": planes3f(Wq[:, cols]),
            "wk": planes3f(wk_c),
            "wv": planes3f(wv_c),
            "wp": planes3b(Wproj[256 * g:256 * (g + 1), :]),
            "bkv": np.stack([
                np.stack([_f32(bk_c.real), _f32(bv_c.real)]),
                np.stack([_f32(bk_c.imag), _f32(bv_c.imag)]),
            ]),
        }
        in_maps.append(m)
    return in_maps


_NC_CACHE = None


def _get_nc():
    global _NC_CACHE
    if _NC_CACHE is None:
        _NC_CACHE = build_nc()
    return _NC_CACHE


def kernel(x_re, x_im, Wq, Wkv, Wproj, bproj, sr_w, sr_b, gain, bias, H, W):
    from concourse.bass_utils import run_bass_kernel_spmd

    nc = _get_nc()
    in_maps = host_prep(x_re, x_im, Wq, Wkv, Wproj, bproj, sr_w, sr_b, gain, bias)
    res = run_bass_kernel_spmd(nc, in_maps, list(range(8)))
    bproj = np.asarray(bproj)
    out = np.zeros((B, N, C), dtype=np.complex64)
    for b in range(B):
        p0 = res.results[2 * b]["outT"].astype(np.float32)
        p1 = res.results[2 * b + 1]["outT"].astype(np.float32)
        acc = (p0[0] + p1[0]).T + 1j * (p0[1] + p1[1]).T
        out[b] = acc + bproj[None, :]
    return out



# revision 20
# speedup vs baseline: 1.0351x; 1.0351x over previous
"""Trainium2 Bass kernel for complex-valued spatial-reduction attention.

x: [B=4, N=2304, C=512] complex64 (re/im f32 planes), H=W=48, 8 heads,
head_dim 64, sr_ratio 2 -> Nk=576.

Sharding: 8 cores = 4 batches x 2 head-groups (4 heads each). Each core:
sr-conv over full C, complex LayerNorm, q/k/v for its heads,
softmax(|q.k^T|) attention, attn @ v, partial output projection.
Host sums the two partials per batch and adds bproj.

Precision: softmax logits reach |a| ~ 37, so the conv -> LN-stats ->
q/kv -> scores chain runs in f32r matmuls (f32-equivalent precision at
bf16-class speed for free dims >= 256, measured on HW). E/v/attn@v/proj
use bf16.
"""

import os
import contextlib

import numpy as np
import ml_dtypes

import concourse.bass as bass
import concourse.mybir as mybir
import concourse.tile as tile
from concourse import bacc
from concourse.masks import make_identity

BF16 = mybir.dt.bfloat16
F16 = mybir.dt.float16
F32 = mybir.dt.float32
F32R = mybir.dt.float32r
AF = mybir.ActivationFunctionType
ALU = mybir.AluOpType

B, N, C, HEADS, HD, SR = 4, 2304, 512, 8, 64, 2
NK = 576
HR = 24
EPS = 1e-5
SCALE = HD ** -0.5  # folded into Wk host-side

M_GROUPS = [
    [(0, 5), (5, 5), (10, 5)],
    [(15, 5), (20, 4)],
]
K_CHUNKS = [(0, 128), (128, 128), (256, 128), (384, 128), (512, 64)]
Q_CHUNKS = [(0, 512), (512, 512), (1024, 512), (1536, 512), (2048, 256)]

DEBUG = bool(int(os.environ.get("KBUILD_DEBUG", "0")))
PHASES = int(os.environ.get("KBUILD_PHASES", "4"))
LNLEVEL = int(os.environ.get("KBUILD_LN", "3"))


def _r(ap):
    return ap.bitcast(F32R)


def build_nc():
    nc = bacc.Bacc("TRN2", target_bir_lowering=False, debug=False, num_devices=8)

    xT_d = nc.dram_tensor("xT", [2, C, N], F32R, kind="ExternalInput")
    xP_d = nc.dram_tensor("xP", [2, 4 * C, NK], F32R, kind="ExternalInput")
    wc_d = nc.dram_tensor("wc", [3, 4 * C, C], F32R, kind="ExternalInput")
    srb_d = nc.dram_tensor("srb", [2, C], F32R, kind="ExternalInput")
    ones_d = nc.dram_tensor("ones", [1, 512], F32R, kind="ExternalInput")
    onesc_d = nc.dram_tensor("onesc", [128, 1], F32R, kind="ExternalInput")
    wq_d = nc.dram_tensor("wq", [3, C, 256], F32R, kind="ExternalInput")
    wk_d = nc.dram_tensor("wk", [3, C, 256], F32R, kind="ExternalInput")
    wv_d = nc.dram_tensor("wv", [3, C, 256], F32R, kind="ExternalInput")
    wp_d = nc.dram_tensor("wp", [3, 256, C], BF16, kind="ExternalInput")
    bkv_d = nc.dram_tensor("bkv", [2, 2, 256], F32R, kind="ExternalInput")
    outT_d = nc.dram_tensor("outT", [2, C, N], F16, kind="ExternalOutput")
    qT_s = nc.dram_tensor("qT_scratch", [2, 2, 128, N], F32R)   # plane, half
    oT_s = nc.dram_tensor("oT_scratch", [2, 2, 128, N], BF16)  # plane, hp
    dbg = {}
    if DEBUG:
        dbg["xnT"] = nc.dram_tensor("dbg_xnT", [2, C, NK], F32, kind="ExternalOutput")
        dbg["qT"] = nc.dram_tensor("dbg_qT", [2, 256, N], F32, kind="ExternalOutput")
        dbg["kT"] = nc.dram_tensor("dbg_kT", [3, 256, NK], F32, kind="ExternalOutput")
        dbg["v"] = nc.dram_tensor("dbg_v", [128, 5 * 4 * 128], F32, kind="ExternalOutput")
        dbg["dn"] = nc.dram_tensor("dbg_dn", [2, 2, N], F32, kind="ExternalOutput")
        dbg["conv"] = nc.dram_tensor("dbg_conv", [2, NK, C], F32, kind="ExternalOutput")

    with tile.TileContext(nc) as tc:
        _body(nc, tc, xT_d, xP_d, wc_d, srb_d, ones_d, onesc_d, wq_d, wk_d,
              wv_d, wp_d, bkv_d, outT_d, qT_s, oT_s, dbg)

    nc.compile()
    return nc


def _ln_chunk(nc, work, stats, psum_pool, cre, cim, sz):
    """Complex LayerNorm for one [sz, C] chunk in PSUM -> (xnr, xni, re, im)."""
    inv_c = 1.0 / C
    re_sb = work.tile([128, C], F32, tag="ln_re", bufs=1)
    im_sb = work.tile([128, C], F32, tag="ln_im", bufs=1)
    sum_r = stats.tile([128, 1], F32, tag="sum_r")
    sum_i = stats.tile([128, 1], F32, tag="sum_i")
    nc.vector.tensor_copy(re_sb[:sz], cre[:sz, :])
    nc.vector.tensor_copy(im_sb[:sz], cim[:sz, :])
    nc.vector.tensor_reduce(sum_r[:sz], re_sb[:sz], mybir.AxisListType.X, ALU.add)
    nc.vector.tensor_reduce(sum_i[:sz], im_sb[:sz], mybir.AxisListType.X, ALU.add)
    junk = work.tile([128, C], F32, tag="ln_junk", bufs=1)
    sxx = stats.tile([128, 1], F32, tag="sxx")
    sii = stats.tile([128, 1], F32, tag="sii")
    sxi = stats.tile([128, 1], F32, tag="sxi")
    if LNLEVEL < 1:
        return re_sb, im_sb
    nc.vector.tensor_mul(junk[:sz], re_sb[:sz], re_sb[:sz])
    nc.vector.tensor_reduce(sxx[:sz], junk[:sz], mybir.AxisListType.X, ALU.add)
    nc.vector.tensor_mul(junk[:sz], im_sb[:sz], im_sb[:sz])
    nc.vector.tensor_reduce(sii[:sz], junk[:sz], mybir.AxisListType.X, ALU.add)
    nc.vector.tensor_mul(junk[:sz], re_sb[:sz], im_sb[:sz])
    nc.vector.tensor_reduce(sxi[:sz], junk[:sz], mybir.AxisListType.X, ALU.add)
    mr = stats.tile([128, 1], F32, tag="mr")
    mi = stats.tile([128, 1], F32, tag="mi")
    nc.vector.tensor_scalar_mul(mr[:sz], sum_r[:sz], inv_c)
    nc.vector.tensor_scalar_mul(mi[:sz], sum_i[:sz], inv_c)
    vre = stats.tile([128, 1], F32, tag="vre")
    vim = stats.tile([128, 1], F32, tag="vim")
    tA = stats.tile([128, 1], F32, tag="tA")
    tB = stats.tile([128, 1], F32, tag="tB")
    nc.vector.tensor_sub(tA[:sz], sxx[:sz], sii[:sz])
    nc.vector.tensor_scalar_mul(tA[:sz], tA[:sz], inv_c)
    nc.vector.tensor_mul(vre[:sz], mr[:sz], mr[:sz])
    nc.vector.tensor_mul(tB[:sz], mi[:sz], mi[:sz])
    nc.vector.tensor_sub(vre[:sz], vre[:sz], tB[:sz])
    nc.vector.tensor_sub(vre[:sz], tA[:sz], vre[:sz])
    nc.vector.tensor_scalar_add(vre[:sz], vre[:sz], EPS)
    nc.vector.tensor_mul(tB[:sz], mr[:sz], mi[:sz])
    nc.vector.tensor_scalar_mul(tB[:sz], tB[:sz], 2.0)
    nc.vector.tensor_scalar_mul(vim[:sz], sxi[:sz], 2.0 * inv_c)
    nc.vector.tensor_sub(vim[:sz], vim[:sz], tB[:sz])
    if LNLEVEL < 2:
        return re_sb, im_sb
    r2 = stats.tile([128, 1], F32, tag="r2")
    nc.vector.tensor_mul(r2[:sz], vre[:sz], vre[:sz])
    nc.vector.tensor_mul(tB[:sz], vim[:sz], vim[:sz])
    nc.vector.tensor_add(r2[:sz], r2[:sz], tB[:sz])
    def _sqrt_newton(out, x, sc):
        # y0 = LUT sqrt(sc*x); y1 = 0.5*(y0 + sc*x/y0)  (one Newton step)
        y0 = stats.tile([128, 1], F32, tag="nw_y0")
        nc.scalar.activation(y0[:sz], x[:sz], AF.Sqrt, scale=sc)
        yr = stats.tile([128, 1], F32, tag="nw_yr")
        nc.vector.tensor_scalar_add(y0[:sz], y0[:sz], 1e-30)
        nc.vector.reciprocal(yr[:sz], y0[:sz])
        nc.vector.tensor_mul(yr[:sz], yr[:sz], x[:sz])
        if sc != 1.0:
            nc.vector.tensor_scalar_mul(yr[:sz], yr[:sz], sc)
        nc.vector.tensor_add(out[:sz], y0[:sz], yr[:sz])
        nc.vector.tensor_scalar_mul(out[:sz], out[:sz], 0.5)

    rr = stats.tile([128, 1], F32, tag="rr")
    _sqrt_newton(rr, r2, 1.0)
    srt = stats.tile([128, 1], F32, tag="srt")
    sia = stats.tile([128, 1], F32, tag="sia")
    nc.vector.tensor_add(tA[:sz], rr[:sz], vre[:sz])
    _sqrt_newton(srt, tA, 0.5)
    nc.vector.tensor_sub(tA[:sz], rr[:sz], vre[:sz])
    _sqrt_newton(sia, tA, 0.5)
    sgn = stats.tile([128, 1], F32, tag="sgn")
    nc.scalar.activation(sgn[:sz], vim[:sz], AF.Sign)
    nc.vector.tensor_mul(sia[:sz], sia[:sz], sgn[:sz])
    rin = stats.tile([128, 1], F32, tag="rin")
    nc.vector.reciprocal(rin[:sz], rr[:sz])
    wr = stats.tile([128, 1], F32, tag="wr")
    wn = stats.tile([128, 1], F32, tag="wn")  # = -w_im
    nc.vector.tensor_mul(wr[:sz], srt[:sz], rin[:sz])
    nc.vector.tensor_mul(wn[:sz], sia[:sz], rin[:sz])
    if LNLEVEL < 3:
        return re_sb, im_sb
    aT = work.tile([128, C], F32, tag="ln_a", bufs=1)
    bT = work.tile([128, C], F32, tag="ln_b", bufs=1)
    xnr = work.tile([128, C], F32, tag="ln_xnr", bufs=1)
    xni = work.tile([128, C], F32, tag="ln_xni", bufs=1)
    nc.vector.tensor_scalar(aT[:sz], re_sb[:sz], mr[:sz], wr[:sz],
                            ALU.subtract, ALU.mult)
    nc.vector.tensor_scalar(bT[:sz], im_sb[:sz], mi[:sz], wn[:sz],
                            ALU.subtract, ALU.mult)
    nc.vector.tensor_add(xnr[:sz], aT[:sz], bT[:sz])
    nc.vector.tensor_scalar(aT[:sz], re_sb[:sz], mr[:sz], wn[:sz],
                            ALU.subtract, ALU.mult)
    nc.vector.tensor_scalar(bT[:sz], im_sb[:sz], mi[:sz], wr[:sz],
                            ALU.subtract, ALU.mult)
    nc.vector.tensor_sub(xni[:sz], bT[:sz], aT[:sz])
    return xnr, xni


def _body(nc, tc, xT_d, xP_d, wc_d, srb_d, ones_d, onesc_d, wq_d, wk_d,
          wv_d, wp_d, bkv_d, outT_d, qT_s, oT_s, dbg):
    ctx = contextlib.ExitStack()
    consts = ctx.enter_context(tc.tile_pool(name="consts", bufs=1))
    big = ctx.enter_context(tc.tile_pool(name="big", bufs=1))
    xs = ctx.enter_context(tc.tile_pool(name="xs", bufs=4))
    qs = ctx.enter_context(tc.tile_pool(name="qs", bufs=2))
    ws = ctx.enter_context(tc.tile_pool(name="ws", bufs=2))
    wcp = ctx.enter_context(tc.tile_pool(name="wcp", bufs=2))
    work = ctx.enter_context(tc.tile_pool(name="work", bufs=2))
    sm = ctx.enter_context(tc.tile_pool(name="sm", bufs=2))
    stats = ctx.enter_context(tc.tile_pool(name="stats", bufs=2))
    psum = ctx.enter_context(tc.tile_pool(name="psum", bufs=7, space="PSUM"))

    # ---- constants ----
    ident = consts.tile([128, 128], F32, tag="ident")
    make_identity(nc, ident)
    ones_col = consts.tile([128, 1], F32R, tag="ones_col")
    nc.sync.dma_start(ones_col[:], onesc_d[:, :])
    ones_row = consts.tile([1, 512], F32R, tag="ones_row")
    nc.sync.dma_start(ones_row[:], ones_d[:])
    nbias = consts.tile([128, 1], F32, tag="nbias")
    nc.vector.memset(nbias, -50.0)

    srb_re = consts.tile([1, C], F32R, tag="srb_re")
    srb_im = consts.tile([1, C], F32R, tag="srb_im")
    nc.sync.dma_start(srb_re[:], srb_d[0:1, :])
    nc.sync.dma_start(srb_im[:], srb_d[1:2, :])
    bk_re = consts.tile([1, 256], F32R, tag="bk_re")
    bk_im = consts.tile([1, 256], F32R, tag="bk_im")
    bv_re = consts.tile([1, 256], F32R, tag="bv_re")
    bv_im = consts.tile([1, 256], F32R, tag="bv_im")
    nc.sync.dma_start(bk_re[:], bkv_d[0:1, 0, :])
    nc.sync.dma_start(bv_re[:], bkv_d[0:1, 1, :])
    nc.sync.dma_start(bk_im[:], bkv_d[1:2, 0, :])
    nc.sync.dma_start(bv_im[:], bkv_d[1:2, 1, :])

    # ---- persistent SBUF ----
    xnTr = big.tile([128, 4, NK], F32R, tag="xnTr")
    xnTi = big.tile([128, 4, NK], F32R, tag="xnTi")
    kTr = big.tile([128, 2, NK], F32R, tag="kTr")
    kTi = big.tile([128, 2, NK], F32R, tag="kTi")
    kTin = big.tile([128, 2, NK], F32R, tag="kTin")
    vpk = big.tile([128, 5, 4, 128], F32R, tag="vpk")
    if DEBUG:
        nc.vector.memset(vpk, 0.0)

    xT_v = [xT_d[pl].rearrange("(j p) t -> p j t", p=128) for pl in (0, 1)]

    # =====================================================================
    # Phase 1: conv (f32r) + LayerNorm + transposes + q-projection
    # =====================================================================
    for mg in M_GROUPS:
        tg0 = mg[0][0] * HR
        tgs = sum(nh for _, nh in mg) * HR
        cps = []
        for (hr0, nh) in mg:
            cre = psum.tile([128, C], F32, tag="bank")
            cim = psum.tile([128, C], F32, tag="bank")
            cps.append((cre, cim, hr0 * HR - tg0, hr0 * HR, nh * HR))

        first = [[True, True] for _ in mg]
        for kk in range(16):
            xp_r = xs.tile([128, 3 * 120], F32R, tag="xp_r")
            xp_i = xs.tile([128, 3 * 120], F32R, tag="xp_i")
            nc.gpsimd.dma_start(xp_r[:, :tgs], xP_d[0, 128 * kk:128 * (kk + 1),
                                                    tg0:tg0 + tgs])
            nc.sync.dma_start(xp_i[:, :tgs], xP_d[1, 128 * kk:128 * (kk + 1),
                                                  tg0:tg0 + tgs])
            w_re = wcp.tile([128, C], F32R, tag="wc_re")
            w_im = wcp.tile([128, C], F32R, tag="wc_im")
            w_in = wcp.tile([128, C], F32R, tag="wc_in")
            nc.gpsimd.dma_start(w_re[:], wc_d[0, 128 * kk:128 * (kk + 1), :])
            nc.sync.dma_start(w_im[:], wc_d[1, 128 * kk:128 * (kk + 1), :])
            nc.gpsimd.dma_start(w_in[:], wc_d[2, 128 * kk:128 * (kk + 1), :])
            for mi_, (cre, cim, off, t0, sz) in enumerate(cps):
                pat_r = xp_r[:, off:off + sz]
                pat_i = xp_i[:, off:off + sz]
                nc.tensor.matmul(cre[:sz, :], _r(pat_r), _r(w_re[:]),
                                 start=first[mi_][0], stop=False)
                nc.tensor.matmul(cim[:sz, :], _r(pat_r), _r(w_im[:]),
                                 start=first[mi_][1], stop=False)
                first[mi_] = [False, False]
                nc.tensor.matmul(cre[:sz, :], _r(pat_i), _r(w_in[:]),
                                 start=False, stop=False)
                nc.tensor.matmul(cim[:sz, :], _r(pat_i), _r(w_re[:]),
                                 start=False, stop=False)
        for (cre, cim, off, t0, sz) in cps:
            nc.tensor.matmul(cre[:sz, :], _r(ones_row[:, :sz]), _r(srb_re[:]),
                             start=False, stop=True)
            nc.tensor.matmul(cim[:sz, :], _r(ones_row[:, :sz]), _r(srb_im[:]),
                             start=False, stop=True)
        if PHASES < 1:
            for (cre, cim, off, t0, sz) in cps:
                dmp = work.tile([128, C], F32, tag="ln_a")
                nc.vector.tensor_copy(dmp[:sz], cre[:sz, :])
                dmp2 = work.tile([128, C], F32, tag="ln_b")
                nc.vector.tensor_copy(dmp2[:sz], cim[:sz, :])

        # ---- LayerNorm + transpose into xnT ----
        for (cre, cim, off, t0, sz) in (cps if PHASES >= 1 else []):
            if DEBUG:
                for tt, src_ in ((0, cre), (1, cim)):
                    cdbg = work.tile([128, C], F32, tag="ln_a")
                    nc.vector.tensor_copy(cdbg[:sz], src_[:sz, :])
                    nc.sync.dma_start(dbg["conv"][tt, t0:t0 + sz, :], cdbg[:sz])
            xnr, xni = _ln_chunk(nc, work, stats, psum, cre, cim, sz)
            for cj in range(4):
                for src, dst in ((xnr, xnTr), (xni, xnTi)):
                    pt = psum.tile([128, 128], F32, tag="bank")
                    nc.tensor.transpose(pt[:, :sz],
                                        src[:sz, 128 * cj:128 * (cj + 1)],
                                        ident[:sz, :sz])
                    nc.vector.tensor_copy(dst[:, cj, t0:t0 + sz], pt[:, :sz])

    # =====================================================================
    # Phase 1b: q-projection (f32r), x^T re-streamed per chunk
    # =====================================================================
    for (q0, nq) in (Q_CHUNKS if PHASES >= 2 else []):
        prs = []
        for half in range(2):
            prs.append((psum.tile([128, 512], F32, tag="bank", name=f"qpr{half}"),
                        psum.tile([128, 512], F32, tag="bank", name=f"qpi{half}")))
        for cj in range(4):
            xq_r = qs.tile([128, 512], F32R, tag="xq_r", bufs=1)
            xq_i = qs.tile([128, 512], F32R, tag="xq_i", bufs=1)
            nc.gpsimd.dma_start(xq_r[:, :nq], xT_v[0][:, cj, q0:q0 + nq])
            nc.sync.dma_start(xq_i[:, :nq], xT_v[1][:, cj, q0:q0 + nq])
            wq_r = ws.tile([128, 256], F32R, tag="w_r")
            wq_i = ws.tile([128, 256], F32R, tag="w_i")
            wq_n = ws.tile([128, 256], F32R, tag="w_n")
            nc.sync.dma_start(wq_r[:], wq_d[0, 128 * cj:128 * (cj + 1), :])
            nc.sync.dma_start(wq_i[:], wq_d[1, 128 * cj:128 * (cj + 1), :])
            nc.sync.dma_start(wq_n[:], wq_d[2, 128 * cj:128 * (cj + 1), :])
            st = cj == 0
            sp = cj == 3
            for half in range(2):
                hs = slice(128 * half, 128 * (half + 1))
                pr, pi = prs[half]
                # wq_r's two matmuls adjacent: one LDWEIGHTS serves both
                nc.tensor.matmul(pr[:, :nq], _r(wq_r[:, hs]), _r(xq_r[:, :nq]),
                                 start=st, stop=False)
                nc.tensor.matmul(pi[:, :nq], _r(wq_r[:, hs]), _r(xq_i[:, :nq]),
                                 start=st, stop=False)
                nc.tensor.matmul(pr[:, :nq], _r(wq_n[:, hs]), _r(xq_i[:, :nq]),
                                 start=False, stop=sp)
                nc.tensor.matmul(pi[:, :nq], _r(wq_i[:, hs]), _r(xq_r[:, :nq]),
                                 start=False, stop=sp)
        for half in range(2):
            pr, pi = prs[half]
            o1 = work.tile([128, 512], F32R, tag="cp_r", bufs=1)
            o2 = work.tile([128, 512], F32R, tag="cp_i", bufs=1)
            nc.scalar.copy(o1[:, :nq], pr[:, :nq])
            nc.scalar.copy(o2[:, :nq], pi[:, :nq])
            nc.sync.dma_start(qT_s[0, half, :, q0:q0 + nq], o1[:, :nq])
            nc.sync.dma_start(qT_s[1, half, :, q0:q0 + nq], o2[:, :nq])
            if DEBUG:
                hs = slice(128 * half, 128 * (half + 1))
                nc.sync.dma_start(dbg["qT"][0, hs, q0:q0 + nq], o1[:, :nq].bitcast(F32))
                nc.sync.dma_start(dbg["qT"][1, hs, q0:q0 + nq], o2[:, :nq].bitcast(F32))

    if DEBUG:
        for cj in range(4):
            nc.sync.dma_start(dbg["xnT"][0, 128 * cj:128 * (cj + 1), :], xnTr[:, cj, :].bitcast(F32))
            nc.sync.dma_start(dbg["xnT"][1, 128 * cj:128 * (cj + 1), :], xnTi[:, cj, :].bitcast(F32))

    # =====================================================================
    # Phase 2: k^T and v projections (f32r)
    # =====================================================================
    for half in (range(2) if PHASES >= 2 else []):
        hs = slice(128 * half, 128 * (half + 1))
        p512r = psum.tile([128, 512], F32, tag="bank")
        p512i = psum.tile([128, 512], F32, tag="bank")
        p64r = psum.tile([128, 512], F32, tag="bank")
        p64i = psum.tile([128, 512], F32, tag="bank")
        for cj in range(4):
            wk_r = ws.tile([128, 256], F32R, tag="w_r")
            wk_i = ws.tile([128, 256], F32R, tag="w_i")
            wk_n = ws.tile([128, 256], F32R, tag="w_n")
            nc.sync.dma_start(wk_r[:], wk_d[0, 128 * cj:128 * (cj + 1), :])
            nc.sync.dma_start(wk_i[:], wk_d[1, 128 * cj:128 * (cj + 1), :])
            nc.sync.dma_start(wk_n[:], wk_d[2, 128 * cj:128 * (cj + 1), :])
            st = cj == 0
            kchunks = ((p512r, p512i, 0, 512), (p64r, p64i, 512, 64))
            # group by stationary: wk_r serves 4 matmuls on one LDWEIGHTS
            for (pr, pi, n0, nn) in kchunks:
                nc.tensor.matmul(pr[:, :nn], _r(wk_r[:, hs]),
                                 _r(xnTr[:, cj, n0:n0 + nn]), start=st, stop=False)
                nc.tensor.matmul(pi[:, :nn], _r(wk_r[:, hs]),
                                 _r(xnTi[:, cj, n0:n0 + nn]), start=st, stop=False)
            for (pr, pi, n0, nn) in kchunks:
                nc.tensor.matmul(pr[:, :nn], _r(wk_n[:, hs]),
                                 _r(xnTi[:, cj, n0:n0 + nn]), start=False, stop=False)
            for (pr, pi, n0, nn) in kchunks:
                nc.tensor.matmul(pi[:, :nn], _r(wk_i[:, hs]),
                                 _r(xnTr[:, cj, n0:n0 + nn]), start=False, stop=False)
        for (pr, pi, n0, nn) in ((p512r, p512i, 0, 512), (p64r, p64i, 512, 64)):
            nc.tensor.matmul(pr[:, :nn], _r(bk_re[:, hs]), _r(ones_row[:, :nn]),
                             start=False, stop=True)
            nc.tensor.matmul(pi[:, :nn], _r(bk_im[:, hs]), _r(ones_row[:, :nn]),
                             start=False, stop=True)
            nc.vector.tensor_copy(kTr[:, half, n0:n0 + nn], pr[:, :nn])
            nc.vector.tensor_copy(kTi[:, half, n0:n0 + nn], pi[:, :nn])
            nc.vector.tensor_scalar_mul(kTin[:, half, n0:n0 + nn], pi[:, :nn], -1.0)

    for kcg in (((0, 1, 2), (3, 4)) if PHASES >= 2 else ()):
        pps = {}
        for kc in kcg:
            pps[kc] = (psum.tile([128, 512], F32, tag="bank", name=f"vpr{kc}"),
                       psum.tile([128, 512], F32, tag="bank", name=f"vpi{kc}"))
        for cj in range(4):
            wv_r = ws.tile([128, 256], F32R, tag="w_r")
            wv_i = ws.tile([128, 256], F32R, tag="w_i")
            wv_n = ws.tile([128, 256], F32R, tag="w_n")
            nc.sync.dma_start(wv_r[:], wv_d[0, 128 * cj:128 * (cj + 1), :])
            nc.sync.dma_start(wv_i[:], wv_d[1, 128 * cj:128 * (cj + 1), :])
            nc.sync.dma_start(wv_n[:], wv_d[2, 128 * cj:128 * (cj + 1), :])
            st = cj == 0
            for kc in kcg:
                k0, szk = K_CHUNKS[kc]
                pr, pi = pps[kc]
                # group by stationary xnT slice: 2 LDWEIGHTS per 4 matmuls
                nc.tensor.matmul(pr[:szk, :256], _r(xnTr[:, cj, k0:k0 + szk]),
                                 _r(wv_r[:]), start=st, stop=False)
                nc.tensor.matmul(pi[:szk, :256], _r(xnTr[:, cj, k0:k0 + szk]),
                                 _r(wv_i[:]), start=st, stop=False)
                nc.tensor.matmul(pr[:szk, :256], _r(xnTi[:, cj, k0:k0 + szk]),
                                 _r(wv_n[:]), start=False, stop=False)
                nc.tensor.matmul(pi[:szk, :256], _r(xnTi[:, cj, k0:k0 + szk]),
                                 _r(wv_r[:]), start=False, stop=False)
        for kc in kcg:
            k0, szk = K_CHUNKS[kc]
            pr, pi = pps[kc]
            nc.tensor.matmul(pr[:szk, :256], _r(ones_row[:, :szk]), _r(bv_re[:]),
                             start=False, stop=True)
            nc.tensor.matmul(pi[:szk, :256], _r(ones_row[:, :szk]), _r(bv_im[:]),
                             start=False, stop=True)
            vr_v = pr[:szk, :256].rearrange("p (h d) -> p h d", h=4)
            vi_v = pi[:szk, :256].rearrange("p (h d) -> p h d", h=4)
            nc.vector.tensor_copy(vpk[:szk, kc, :, 0:64], vr_v)
            nc.vector.tensor_copy(vpk[:szk, kc, :, 64:128], vi_v)

    if DEBUG:
        for half in range(2):
            hs = slice(128 * half, 128 * (half + 1))
            nc.sync.dma_start(dbg["kT"][0, hs, :], kTr[:, half, :].bitcast(F32))
            nc.sync.dma_start(dbg["kT"][1, hs, :], kTi[:, half, :].bitcast(F32))
            nc.sync.dma_start(dbg["kT"][2, hs, :], kTin[:, half, :].bitcast(F32))
        nc.sync.dma_start(dbg["v"][:, :], vpk.rearrange("p a b c -> p (a b c)"))

    # =====================================================================
    # Phase 3: attention (S^T layout, f32r scores), softmax over |.|
    # =====================================================================
    def emit_front(q0, nq, hp, qTr_s, qTi_s):
        ebufs = {}
        sbufs = {}
        for kc in range(5):
            k0, szk = K_CHUNKS[kc]
            sbuf = sm.tile([128, 2, 512], F32, tag="sbuf", name=f"sb{kc}",
                           bufs=3)
            for i in range(2):
                rs = slice(64 * i, 64 * (i + 1))
                sre = psum.tile([128, 512], F32, tag="bank")
                sim = psum.tile([128, 512], F32, tag="bank")
                # kTr's two matmuls adjacent: one LDWEIGHTS serves both
                nc.tensor.matmul(sre[:szk, :nq], _r(kTr[rs, hp, k0:k0 + szk]),
                                 _r(qTr_s[rs, hp, :nq]), start=True, stop=False)
                nc.tensor.matmul(sim[:szk, :nq], _r(kTr[rs, hp, k0:k0 + szk]),
                                 _r(qTi_s[rs, hp, :nq]), start=True, stop=False)
                nc.tensor.matmul(sre[:szk, :nq], _r(kTin[rs, hp, k0:k0 + szk]),
                                 _r(qTi_s[rs, hp, :nq]), start=False, stop=True)
                nc.tensor.matmul(sim[:szk, :nq], _r(kTi[rs, hp, k0:k0 + szk]),
                                 _r(qTr_s[rs, hp, :nq]), start=False, stop=True)
                # DVE may read only one operand from PSUM: square re on ACT
                # (Square is in every table set: no table switch), copy im
                # out via DVE, square+accumulate in SBUF.
                s1 = sm.tile([128, 512], F32, tag="s1")
                nc.scalar.activation(s1[:szk, :nq], sre[:szk, :nq], AF.Square)
                c2 = sm.tile([128, 512], F32, tag="c2")
                nc.vector.tensor_copy(c2[:szk, :nq], sim[:szk, :nq])
                nc.vector.tensor_mul(sbuf[:szk, i, :nq], c2[:szk, :nq],
                                     c2[:szk, :nq])
                nc.vector.tensor_add(sbuf[:szk, i, :nq], sbuf[:szk, i, :nq],
                                     s1[:szk, :nq])
            sbufs[kc] = sbuf
        for kc in range(5):  # batched: one sqrt table load per iteration
            k0, szk = K_CHUNKS[kc]
            nc.scalar.activation(sbufs[kc][:szk, :, :nq],
                                 sbufs[kc][:szk, :, :nq], AF.Sqrt)
        for kc in range(5):  # batched: one exp table load per iteration
            k0, szk = K_CHUNKS[kc]
            ebuf = sm.tile([128, 2, 512], F32R, tag="ebuf", name=f"eb{kc}",
                           bufs=8)
            # constant shift keeps exp sums in f32 range; softmax is
            # shift-invariant so the result is exact
            nc.scalar.activation(ebuf[:szk, :, :nq], sbufs[kc][:szk, :, :nq],
                                 AF.Exp, bias=nbias[:szk])
            ebufs[kc] = ebuf
        return ebufs

    def emit_back(q0, nq, hp, ebufs):
        op0 = psum.tile([128, 512], F32, tag="bank", name="op0")
        op1 = psum.tile([128, 512], F32, tag="bank", name="op1")
        # f32r matmul rejects tile_position: one dn bank per head
        dn0 = psum.tile([128, 512], F32, tag="bank", name="dn0")
        dn1 = psum.tile([128, 512], F32, tag="bank", name="dn1")
        dns = (dn0, dn1)
        for kc in range(5):
            k0, szk = K_CHUNKS[kc]
            ebuf = ebufs[kc]
            for i in range(2):
                hh = 2 * hp + i
                opt = op0 if i == 0 else op1
                nc.tensor.matmul(opt[:, :nq], _r(vpk[:szk, kc, hh, :]),
                                 _r(ebuf[:szk, i, :nq]), start=kc == 0,
                                 stop=kc == 4)
                nc.tensor.matmul(dns[i][:1, :nq],
                                 _r(ones_col[:szk, :]), _r(ebuf[:szk, i, :nq]),
                                 start=kc == 0, stop=kc == 4)
        otr = sm.tile([128, 512], BF16, tag="otr")
        oti = sm.tile([128, 512], BF16, tag="oti")
        # drain attn-out + denominators out of PSUM right away so the next
        # iteration's score matmuls get banks immediately
        ops_sb = []
        for i, opt in enumerate((op0, op1)):
            osb = sm.tile([128, 512], F32, tag="opsb", name=f"opsb{i}")
            nc.vector.tensor_copy(osb[:, :nq], opt[:, :nq])
            ops_sb.append(osb)
        rhs_t = []
        for i in range(2):
            rh = stats.tile([1, 512], F32R, tag="lnd", name=f"rh{i}")
            # f32r is f32 bits; only the matmul-rounding contract differs
            with nc.allow_low_precision(reason="f32r out is full f32"):
                nc.vector.reciprocal(rh[:, :nq], dns[i][:1, :nq])
            rhs_t.append(rh)
        for i in range(2):
            rh = rhs_t[i]
            rbp = psum.tile([128, 512], F32, tag="bank")
            nc.tensor.matmul(rbp[:, :nq], _r(ones_row[:1, :128]), _r(rh[:, :nq]),
                             start=True, stop=True)
            rb = sm.tile([128, 512], F32, tag="rb", bufs=1)
            nc.vector.tensor_copy(rb[:, :nq], rbp[:, :nq])
            osb = ops_sb[i]
            rs = slice(64 * i, 64 * (i + 1))
            nc.vector.tensor_mul(otr[rs, :nq], osb[0:64, :nq], rb[0:64, :nq])
            nc.vector.tensor_mul(oti[rs, :nq], osb[64:128, :nq], rb[64:128, :nq])
        nc.sync.dma_start(oT_s[0, hp, :, q0:q0 + nq], otr[:, :nq])
        nc.gpsimd.dma_start(oT_s[1, hp, :, q0:q0 + nq], oti[:, :nq])

    # software pipeline: iteration i+1's score matmuls are emitted before
    # iteration i's attn@v/normalize tail, so the PE never waits on the
    # softmax ACT chain
    prev = None
    for (q0, nq) in (Q_CHUNKS if PHASES >= 3 else []):
        qTr_s = qs.tile([128, 2, 512], F32R, tag="qTr_s")
        qTi_s = qs.tile([128, 2, 512], F32R, tag="qTi_s")
        for half in range(2):
            for c4 in range(2):
                s0 = 256 * c4
                w = min(256, nq - s0)
                if w > 0:
                    nc.gpsimd.dma_start(qTr_s[:, half, s0:s0 + w],
                                        qT_s[0, half, :, q0 + s0:q0 + s0 + w])
                    nc.sync.dma_start(qTi_s[:, half, s0:s0 + w],
                                      qT_s[1, half, :, q0 + s0:q0 + s0 + w])
        for hp in range(2):
            ebufs = emit_front(q0, nq, hp, qTr_s, qTi_s)
            if prev is not None:
                emit_back(*prev)
            prev = (q0, nq, hp, ebufs)
    if prev is not None:
        emit_back(*prev)

    # =====================================================================
    # Phase 4: partial output projection (bf16)
    # =====================================================================
    wp_sb = big.tile([128, 3, 2, C], BF16, tag="wp")
    nc.sync.dma_start(wp_sb[:], wp_d.rearrange("s (j p) n -> p s j n", p=128))
    for (q0, nq) in (Q_CHUNKS if PHASES >= 4 else []):
        oTr_l = qs.tile([128, 2, 512], BF16, tag="oTr_l")
        oTi_l = qs.tile([128, 2, 512], BF16, tag="oTi_l")
        for hp in range(2):
            nc.sync.dma_start(oTr_l[:, hp, :nq], oT_s[0, hp, :, q0:q0 + nq])
            nc.sync.dma_start(oTi_l[:, hp, :nq], oT_s[1, hp, :, q0:q0 + nq])
        for cc in range(4):
            cs = slice(128 * cc, 128 * (cc + 1))
            pr = psum.tile([128, 512], F32, tag="bank")
            pi = psum.tile([128, 512], F32, tag="bank")
            for half in range(2):
                st = half == 0
                sp = half == 1
                nc.tensor.matmul(pr[:, :nq], wp_sb[:, 0, half, cs],
                                 oTr_l[:, half, :nq], start=st, stop=False)
                nc.tensor.matmul(pr[:, :nq], wp_sb[:, 2, half, cs],
                                 oTi_l[:, half, :nq], start=False, stop=sp)
                nc.tensor.matmul(pi[:, :nq], wp_sb[:, 1, half, cs],
                                 oTr_l[:, half, :nq], start=st, stop=False)
                nc.tensor.matmul(pi[:, :nq], wp_sb[:, 0, half, cs],
                                 oTi_l[:, half, :nq], start=False, stop=sp)
            o1 = work.tile([128, 512], F16, tag="cp_r16")
            o2 = work.tile([128, 512], F16, tag="cp_i16")
            nc.vector.tensor_copy(o1[:, :nq], pr[:, :nq])
            nc.vector.tensor_copy(o2[:, :nq], pi[:, :nq])
            nc.gpsimd.dma_start(outT_d[0, cs, q0:q0 + nq], o1[:, :nq])
            nc.sync.dma_start(outT_d[1, cs, q0:q0 + nq], o2[:, :nq])

    ctx.close()


# =========================================================================
# Host side
# =========================================================================

def _f32(x):
    return np.ascontiguousarray(x, dtype=np.float32)


def _bf(x):
    return np.asarray(x, dtype=ml_dtypes.bfloat16)


def host_prep(x_re, x_im, Wq, Wkv, Wproj, bproj, sr_w, sr_b, gain, bias):
    x_re = np.asarray(x_re)
    x_im = np.asarray(x_im)
    Wq = np.asarray(Wq)
    Wkv = np.asarray(Wkv)
    Wproj = np.asarray(Wproj)
    sr_w = np.asarray(sr_w)
    sr_b = np.asarray(sr_b)
    gain = np.asarray(gain)
    bias = np.asarray(bias)

    Wkv_eff = gain[:, None] * Wkv
    bkv_full = bias @ Wkv
    Wc = sr_w.transpose(2, 3, 1, 0).reshape(4 * C, C)

    def planes3f(w):
        return np.stack([_f32(w.real), _f32(w.imag), _f32(-w.imag)])

    def planes3b(w):
        return np.stack([_bf(w.real), _bf(w.imag), _bf(-w.imag)])

    in_maps = []
    for core in range(8):
        b, g = core // 2, core % 2
        cols = slice(256 * g, 256 * (g + 1))
        wk_c = Wkv_eff[:, :C][:, cols] * SCALE
        wv_c = Wkv_eff[:, C:][:, cols]
        bk_c = bkv_full[:C][cols] * SCALE
        bv_c = bkv_full[C:][cols]
        xs_c = np.stack([x_re[b].T, x_im[b].T])  # [2, C, N]
        xsp = xs_c.reshape(2, C, HR, 2, HR, 2)
        xP = np.stack([xsp[:, :, :, p, :, q].reshape(2, C, NK)
                       for p in range(2) for q in range(2)], axis=1)
        m = {
            "xT": _f32(xs_c),
            "xP": _f32(xP.reshape(2, 4 * C, NK)),
            "wc": planes3f(Wc),
            "srb": np.stack([_f32(sr_b.real), _f32(sr_b.imag)]),
            "ones": np.ones((1, 512), np.float32),
            "onesc": np.ones((128, 1), np.float32),
            "wq": planes3f(Wq[:, cols]),
            "wk": planes3f(wk_c),
            "wv": planes3f(wv_c),
            "wp": planes3b(Wproj[256 * g:256 * (g + 1), :]),
            "bkv": np.stack([
                np.stack([_f32(bk_c.real), _f32(bv_c.real)]),
                np.stack([_f32(bk_c.imag), _f32(bv_c.imag)]),
            ]),
        }
        in_maps.append(m)
    return in_maps


_NC_CACHE = None


def _get_nc():
    global _NC_CACHE
    if _NC_CACHE is None:
        _NC_CACHE = build_nc()
    return _NC_CACHE


def kernel(x_re, x_im, Wq, Wkv, Wproj, bproj, sr_w, sr_b, gain, bias, H, W):
    from concourse.bass_utils import run_bass_kernel_spmd

    nc = _get_nc()
    in_maps = host_prep(x_re, x_im, Wq, Wkv, Wproj, bproj, sr_w, sr_b, gain, bias)
    res = run_bass_kernel_spmd(nc, in_maps, list(range(8)))
    bproj = np.asarray(bproj)
    out = np.zeros((B, N, C), dtype=np.complex64)
    for b in range(B):
        p0 = res.results[2 * b]["outT"].astype(np.float32)
        p1 = res.results[2 * b + 1]["outT"].astype(np.float32)
        acc = (p0[0] + p1[0]).T + 1j * (p0[1] + p1[1]).T
        out[b] = acc + bproj[None, :]
    return out



# revision 34
# speedup vs baseline: 1.4212x; 1.3730x over previous
"""Trainium2 Bass kernel for complex-valued spatial-reduction attention.

x: [B=4, N=2304, C=512] complex64 (re/im f32 planes), H=W=48, 8 heads,
head_dim 64, sr_ratio 2 -> Nk=576.

Sharding: 8 cores = 4 batches x 2 head-groups (4 heads each). Each core:
sr-conv over full C, complex LayerNorm, q/k/v for its heads,
softmax(|q.k^T|) attention, attn @ v, partial output projection.
Host sums the two partials per batch and adds bproj.

Everything stays on-chip: xP (patch-permuted x) is resident in SBUF and
feeds both the conv and the q-projection (q columns come out in
(patch-pos, nk) order; the host unpermutes). q / attention-out / weights
are SBUF-resident, so HBM traffic is inputs + weights + output only.

Precision: the f32r matmul path (~tf32-class rounding) dominates the
error budget; the softmax s=re^2+im^2 chain runs in f16 which measures
as noise against that. ebuf/v/proj are f32r.
"""

import os
import contextlib

import numpy as np
import ml_dtypes

import concourse.bass as bass
import concourse.mybir as mybir
import concourse.tile as tile
from concourse import bacc
from concourse.masks import make_identity

BF16 = mybir.dt.bfloat16
F16 = mybir.dt.float16
F32 = mybir.dt.float32
F32R = mybir.dt.float32r
AF = mybir.ActivationFunctionType
ALU = mybir.AluOpType

B, N, C, HEADS, HD, SR = 4, 2304, 512, 8, 64, 2
NK = 576
HR = 24
EPS = 1e-5
SCALE = HD ** -0.5  # folded into Wk host-side

CHUNKS5 = [(0, 120), (120, 120), (240, 120), (360, 120), (480, 96)]
NKH = [(0, 288), (288, 288)]
K_CHUNKS = [(0, 128), (128, 128), (256, 128), (384, 128), (512, 64)]
Q_CHUNKS = [(0, 512), (512, 512), (1024, 512), (1536, 512), (2048, 256)]

DEBUG = bool(int(os.environ.get("KBUILD_DEBUG", "0")))
PHASES = int(os.environ.get("KBUILD_PHASES", "4"))
P1MASK = int(os.environ.get("KBUILD_P1", "7"))  # 1=conv 2=qproj 4=ln


def _r(ap):
    return ap.bitcast(F32R)


def build_nc():
    nc = bacc.Bacc("TRN2", target_bir_lowering=False, debug=False, num_devices=8)

    xP_d = nc.dram_tensor("xP", [2, 16, 128, NK], F32R, kind="ExternalInput")
    wc_d = nc.dram_tensor("wc", [2, 3, 16, 128, 256], F32R, kind="ExternalInput")
    srb_d = nc.dram_tensor("srb", [2, C], F32R, kind="ExternalInput")
    ones_d = nc.dram_tensor("ones", [1, 512], F32R, kind="ExternalInput")
    onesc_d = nc.dram_tensor("onesc", [128, 1], F32R, kind="ExternalInput")
    wq_d = nc.dram_tensor("wq", [3, 4, 128, 256], F32R, kind="ExternalInput")
    wk_d = nc.dram_tensor("wk", [3, 4, 128, 256], F32R, kind="ExternalInput")
    wv_d = nc.dram_tensor("wv", [3, 4, 128, 256], F32R, kind="ExternalInput")
    wp_d = nc.dram_tensor("wp", [3, 2, 128, 512], F32R, kind="ExternalInput")
    bkv_d = nc.dram_tensor("bkv", [2, 2, 256], F32R, kind="ExternalInput")
    # output: [plane, c-block, q-chunk, 128 c, 512 q] (q cols permuted (p4, nk))
    outT_d = nc.dram_tensor("outT", [2, 4, 5, 128, 512], F16, kind="ExternalOutput")
    dbg = {}
    if DEBUG:
        dbg["xnT"] = nc.dram_tensor("dbg_xnT", [2, C, NK], F32, kind="ExternalOutput")
        dbg["qT"] = nc.dram_tensor("dbg_qT", [2, 2, 128, N], F32, kind="ExternalOutput")
        dbg["kT"] = nc.dram_tensor("dbg_kT", [3, 256, NK], F32, kind="ExternalOutput")
        dbg["v"] = nc.dram_tensor("dbg_v", [128, 5 * 4 * 128], F32, kind="ExternalOutput")
        dbg["conv"] = nc.dram_tensor("dbg_conv", [2, NK, C], F32, kind="ExternalOutput")

    with tile.TileContext(nc) as tc:
        _body(nc, tc, xP_d, wc_d, srb_d, ones_d, onesc_d, wq_d, wk_d,
              wv_d, wp_d, bkv_d, outT_d, dbg)

    nc.compile()
    return nc


def _ln2(nc, work, stats, stg, sz):
    """Complex LayerNorm for one chunk; stg is [128, 2, 2, 256]
    (re|im half, conv C-pass, 256 cols) so re/im are each contiguous."""
    inv_c = 1.0 / C
    re_sb = stg[:, 0].rearrange("p a b -> p (a b)")
    im_sb = stg[:, 1].rearrange("p a b -> p (a b)")
    sum_r = stats.tile([128, 1], F32, tag="sum_r")
    sum_i = stats.tile([128, 1], F32, tag="sum_i")
    nc.vector.tensor_reduce(sum_r[:sz], re_sb[:sz], mybir.AxisListType.X, ALU.add)
    nc.vector.tensor_reduce(sum_i[:sz], im_sb[:sz], mybir.AxisListType.X, ALU.add)
    junk = work.tile([128, C], F32, tag="ln_a", bufs=1, name="junk")
    sxx = stats.tile([128, 1], F32, tag="sxx")
    sii = stats.tile([128, 1], F32, tag="sii")
    sxi = stats.tile([128, 1], F32, tag="sxi")
    nc.vector.tensor_mul(junk[:sz], re_sb[:sz], re_sb[:sz])
    nc.vector.tensor_reduce(sxx[:sz], junk[:sz], mybir.AxisListType.X, ALU.add)
    nc.vector.tensor_mul(junk[:sz], im_sb[:sz], im_sb[:sz])
    nc.vector.tensor_reduce(sii[:sz], junk[:sz], mybir.AxisListType.X, ALU.add)
    nc.vector.tensor_mul(junk[:sz], re_sb[:sz], im_sb[:sz])
    nc.vector.tensor_reduce(sxi[:sz], junk[:sz], mybir.AxisListType.X, ALU.add)
    mr = stats.tile([128, 1], F32, tag="mr")
    mi = stats.tile([128, 1], F32, tag="mi")
    nc.vector.tensor_scalar_mul(mr[:sz], sum_r[:sz], inv_c)
    nc.vector.tensor_scalar_mul(mi[:sz], sum_i[:sz], inv_c)
    vre = stats.tile([128, 1], F32, tag="vre")
    vim = stats.tile([128, 1], F32, tag="vim")
    tA = stats.tile([128, 1], F32, tag="tA")
    tB = stats.tile([128, 1], F32, tag="tB")
    nc.vector.tensor_sub(tA[:sz], sxx[:sz], sii[:sz])
    nc.vector.tensor_scalar_mul(tA[:sz], tA[:sz], inv_c)
    nc.vector.tensor_mul(vre[:sz], mr[:sz], mr[:sz])
    nc.vector.tensor_mul(tB[:sz], mi[:sz], mi[:sz])
    nc.vector.tensor_sub(vre[:sz], vre[:sz], tB[:sz])
    nc.vector.tensor_sub(vre[:sz], tA[:sz], vre[:sz])
    nc.vector.tensor_scalar_add(vre[:sz], vre[:sz], EPS)
    nc.vector.tensor_mul(tB[:sz], mr[:sz], mi[:sz])
    nc.vector.tensor_scalar_mul(tB[:sz], tB[:sz], 2.0)
    nc.vector.tensor_scalar_mul(vim[:sz], sxi[:sz], 2.0 * inv_c)
    nc.vector.tensor_sub(vim[:sz], vim[:sz], tB[:sz])
    # complex rsqrt of (vre + i vim): w = conj(sqrt(v)) / |v|
    r2 = stats.tile([128, 1], F32, tag="r2")
    nc.vector.tensor_mul(r2[:sz], vre[:sz], vre[:sz])
    nc.vector.tensor_mul(tB[:sz], vim[:sz], vim[:sz])
    nc.vector.tensor_add(r2[:sz], r2[:sz], tB[:sz])
    def _sqrt_newton(out, x, sc):
        # y0 = LUT sqrt(sc*x); y1 = 0.5*(y0 + sc*x/y0)  (one Newton step)
        y0 = stats.tile([128, 1], F32, tag="nw_y0")
        nc.scalar.activation(y0[:sz], x[:sz], AF.Sqrt, scale=sc)
        yr = stats.tile([128, 1], F32, tag="nw_yr")
        nc.vector.tensor_scalar_add(y0[:sz], y0[:sz], 1e-30)
        nc.vector.reciprocal(yr[:sz], y0[:sz])
        nc.vector.tensor_mul(yr[:sz], yr[:sz], x[:sz])
        if sc != 1.0:
            nc.vector.tensor_scalar_mul(yr[:sz], yr[:sz], sc)
        nc.vector.tensor_add(out[:sz], y0[:sz], yr[:sz])
        nc.vector.tensor_scalar_mul(out[:sz], out[:sz], 0.5)

    rr = stats.tile([128, 1], F32, tag="rr")
    _sqrt_newton(rr, r2, 1.0)  # |v|
    srt = stats.tile([128, 1], F32, tag="srt")
    sia = stats.tile([128, 1], F32, tag="sia")
    nc.vector.tensor_add(tA[:sz], rr[:sz], vre[:sz])
    _sqrt_newton(srt, tA, 0.5)  # Re sqrt(v)
    nc.vector.tensor_sub(tA[:sz], rr[:sz], vre[:sz])
    _sqrt_newton(sia, tA, 0.5)  # |Im sqrt(v)|
    sgn = stats.tile([128, 1], F32, tag="sgn")
    nc.scalar.activation(sgn[:sz], vim[:sz], AF.Sign)
    nc.vector.tensor_mul(sia[:sz], sia[:sz], sgn[:sz])
    rin = stats.tile([128, 1], F32, tag="rin")
    nc.vector.reciprocal(rin[:sz], rr[:sz])
    wr = stats.tile([128, 1], F32, tag="wr")
    wn = stats.tile([128, 1], F32, tag="wn")  # = -w_im
    nc.vector.tensor_mul(wr[:sz], srt[:sz], rin[:sz])
    nc.vector.tensor_mul(wn[:sz], sia[:sz], rin[:sz])
    # xn = w * (x - m), complex
    aT = work.tile([128, C], F32, tag="ln_a", bufs=1)
    bT = work.tile([128, C], F32, tag="ln_b", bufs=1)
    xnr = work.tile([128, C], F32, tag="ln_xnr", bufs=1)
    xni = work.tile([128, C], F32, tag="ln_xni", bufs=1)
    nc.vector.tensor_scalar(aT[:sz], re_sb[:sz], mr[:sz], wr[:sz],
                            ALU.subtract, ALU.mult)
    nc.vector.tensor_scalar(bT[:sz], im_sb[:sz], mi[:sz], wn[:sz],
                            ALU.subtract, ALU.mult)
    nc.vector.tensor_add(xnr[:sz], aT[:sz], bT[:sz])
    nc.vector.tensor_scalar(aT[:sz], re_sb[:sz], mr[:sz], wn[:sz],
                            ALU.subtract, ALU.mult)
    nc.vector.tensor_scalar(bT[:sz], im_sb[:sz], mi[:sz], wr[:sz],
                            ALU.subtract, ALU.mult)
    nc.vector.tensor_sub(xni[:sz], bT[:sz], aT[:sz])
    return xnr, xni


def _body(nc, tc, xP_d, wc_d, srb_d, ones_d, onesc_d, wq_d, wk_d, wv_d,
          wp_d, bkv_d, outT_d, dbg):
    ctx = contextlib.ExitStack()
    consts = ctx.enter_context(tc.tile_pool(name="consts", bufs=1))
    big = ctx.enter_context(tc.tile_pool(name="big", bufs=1))
    stats = ctx.enter_context(tc.tile_pool(name="stats", bufs=2))
    psum = ctx.enter_context(tc.tile_pool(name="psum", bufs=7, space="PSUM"))

    # ---- constants ----
    ident = consts.tile([128, 128], F32, tag="ident")
    make_identity(nc, ident)
    ones_col = consts.tile([128, 1], F32R, tag="ones_col")
    nc.sync.dma_start(ones_col[:], onesc_d[:, :])
    ones_row = consts.tile([1, 512], F32R, tag="ones_row")
    nc.sync.dma_start(ones_row[:], ones_d[:])
    nbias = consts.tile([128, 1], F32, tag="nbias")
    nc.vector.memset(nbias, -50.0)
    srb_re = consts.tile([1, C], F32R, tag="srb_re")
    srb_im = consts.tile([1, C], F32R, tag="srb_im")
    nc.sync.dma_start(srb_re[:], srb_d[0:1, :])
    nc.sync.dma_start(srb_im[:], srb_d[1:2, :])
    bk_re = consts.tile([1, 256], F32R, tag="bk_re")
    bk_im = consts.tile([1, 256], F32R, tag="bk_im")
    bv_re = consts.tile([1, 256], F32R, tag="bv_re")
    bv_im = consts.tile([1, 256], F32R, tag="bv_im")
    nc.sync.dma_start(bk_re[:], bkv_d[0:1, 0, :])
    nc.sync.dma_start(bv_re[:], bkv_d[0:1, 1, :])
    nc.sync.dma_start(bk_im[:], bkv_d[1:2, 0, :])
    nc.sync.dma_start(bv_im[:], bkv_d[1:2, 1, :])

    # ---- whole-kernel resident SBUF ----
    qTr_sb = big.tile([128, 2, N], F32R, tag="qTr_sb")
    qTi_sb = big.tile([128, 2, N], F32R, tag="qTi_sb")
    xnTr = big.tile([128, 4, NK], F32R, tag="xnTr")
    xnTi = big.tile([128, 4, NK], F32R, tag="xnTi")
    kTr = big.tile([128, 2, NK], F32R, tag="kTr")
    kTi = big.tile([128, 2, NK], F32R, tag="kTi")
    kTin = big.tile([128, 2, NK], F32R, tag="kTin")
    vpk = big.tile([128, 5, 4, 128], F32R, tag="vpk")
    if DEBUG:
        nc.vector.memset(vpk.bitcast(F32), 0.0)

    # =====================================================================
    # Phase 1: conv (2 C-half passes) + q-projection from resident xP + LN
    # =====================================================================
    with tc.tile_pool(name="xpp", bufs=1) as xpp, \
         tc.tile_pool(name="cwork", bufs=2) as cwork:
        xP = xpp.tile([128, 2, 16, NK], F32R, tag="xP")
        wq = xpp.tile([128, 3, 4, 256], F32R, tag="wq")
        stg = []
        for ch in range(5):
            # [half(re|im), cpass, 256]: re ends up contiguous in cols 0:512
            stg.append(xpp.tile([128, 2, 2, 256], F32, tag=f"stg{ch}",
                                name=f"stg{ch}"))

        for kk in range(16):
            eng = nc.sync if kk % 2 == 0 else nc.gpsimd
            eng.dma_start(xP[:, 0, kk, :], xP_d[0, kk])
            eng2 = nc.gpsimd if kk % 2 == 0 else nc.sync
            eng2.dma_start(xP[:, 1, kk, :], xP_d[1, kk])
        for pl in range(3):
            for cj in range(4):
                eng = nc.sync if (pl * 4 + cj) % 2 == 0 else nc.gpsimd
                eng.dma_start(wq[:, pl, cj, :], wq_d[pl, cj])

        # ---- conv: for each output-C half, accumulate all 5 row chunks ----
        for cp in (range(2) if P1MASK & 1 else []):
            cps = []
            for ch, (t0, sz) in enumerate(CHUNKS5):
                cps.append(psum.tile([128, 512], F32, tag="bank",
                                     name=f"conv{cp}_{ch}"))
            for kk in range(16):
                wcr = cwork.tile([128, 256], F32R, tag="wc_r")
                wci = cwork.tile([128, 256], F32R, tag="wc_i")
                wcn = cwork.tile([128, 256], F32R, tag="wc_n")
                nc.sync.dma_start(wcr[:], wc_d[cp, 0, kk])
                nc.gpsimd.dma_start(wci[:], wc_d[cp, 1, kk])
                nc.sync.dma_start(wcn[:], wc_d[cp, 2, kk])
                st = kk == 0
                for ch, (t0, sz) in enumerate(CHUNKS5):
                    pat_r = xP[:, 0, kk, t0:t0 + sz]
                    pat_i = xP[:, 1, kk, t0:t0 + sz]
                    cpt = cps[ch]
                    # one accumulation group per bank: start only on the
                    # very first matmul, stop only on the last (im bias)
                    nc.tensor.matmul(cpt[:sz, 0:256], pat_r, wcr[:],
                                     start=st, stop=False)
                    nc.tensor.matmul(cpt[:sz, 256:512], pat_r, wci[:],
                                     start=False, stop=False)
                    nc.tensor.matmul(cpt[:sz, 0:256], pat_i, wcn[:],
                                     start=False, stop=False)
                    nc.tensor.matmul(cpt[:sz, 256:512], pat_i, wcr[:],
                                     start=False, stop=False)
            cs = slice(256 * cp, 256 * (cp + 1))
            for ch, (t0, sz) in enumerate(CHUNKS5):
                cpt = cps[ch]
                nc.tensor.matmul(cpt[:sz, 0:256], ones_row[:, :sz],
                                 srb_re[:, cs], start=False, stop=False)
                nc.tensor.matmul(cpt[:sz, 256:512], ones_row[:, :sz],
                                 srb_im[:, cs], start=False, stop=True)
            for ch, (t0, sz) in enumerate(CHUNKS5):
                # whole-bank copy: depends on every write, so it cannot race
                # the PE still accumulating into the other half
                bank_v = cps[ch].rearrange("p (a b) -> p a b", a=2)
                nc.vector.tensor_copy(stg[ch][:sz, :, cp, :], bank_v[:sz])

        # ---- q-projection from resident xP (PE; LN below runs on DVE) ----
        for p4 in (range(4) if P1MASK & 2 else []):
            for (n0, nn) in NKH:
                prs = []
                for half in range(2):
                    prs.append((psum.tile([128, 512], F32, tag="bank",
                                          name=f"qpr{half}"),
                                psum.tile([128, 512], F32, tag="bank",
                                          name=f"qpi{half}")))
                for cj in range(4):
                    kk = p4 * 4 + cj
                    xr = xP[:, 0, kk, n0:n0 + nn]
                    xi = xP[:, 1, kk, n0:n0 + nn]
                    st = cj == 0
                    sp = cj == 3
                    for half in range(2):
                        hs = slice(128 * half, 128 * (half + 1))
                        pr, pi = prs[half]
                        nc.tensor.matmul(pr[:, :nn], wq[:, 0, cj, hs], xr,
                                         start=st, stop=False)
                        nc.tensor.matmul(pi[:, :nn], wq[:, 0, cj, hs], xi,
                                         start=st, stop=False)
                        nc.tensor.matmul(pr[:, :nn], wq[:, 2, cj, hs], xi,
                                         start=False, stop=sp)
                        nc.tensor.matmul(pi[:, :nn], wq[:, 1, cj, hs], xr,
                                         start=False, stop=sp)
                for half in range(2):
                    pr, pi = prs[half]
                    q0 = p4 * NK + n0
                    nc.scalar.copy(qTr_sb[:, half, q0:q0 + nn], pr[:, :nn])
                    nc.scalar.copy(qTi_sb[:, half, q0:q0 + nn], pi[:, :nn])

        # ---- LayerNorm (DVE, overlaps q-proj PE) + transposes into xnT ----
        for ch, (t0, sz) in enumerate(CHUNKS5 if (P1MASK & 4 and P1MASK & 1) else []):
            xnr, xni = _ln2(nc, cwork, stats, stg[ch], sz)
            for cj in range(4):
                for src, dst in ((xnr, xnTr), (xni, xnTi)):
                    pt = psum.tile([128, 128], F32, tag="bank", name="tp")
                    nc.tensor.transpose(pt[:, :sz],
                                        src[:sz, 128 * cj:128 * (cj + 1)],
                                        ident[:sz, :sz])
                    nc.vector.tensor_copy(dst[:, cj, t0:t0 + sz], pt[:, :sz])

    if DEBUG:
        for cj in range(4):
            nc.sync.dma_start(dbg["xnT"][0, 128 * cj:128 * (cj + 1), :],
                              xnTr[:, cj, :].bitcast(F32))
            nc.sync.dma_start(dbg["xnT"][1, 128 * cj:128 * (cj + 1), :],
                              xnTi[:, cj, :].bitcast(F32))
        for half in range(2):
            nc.sync.dma_start(dbg["qT"][0, half, :, :],
                              qTr_sb[:, half, :].bitcast(F32))
            nc.sync.dma_start(dbg["qT"][1, half, :, :],
                              qTi_sb[:, half, :].bitcast(F32))

    # =====================================================================
    # Phases 2-4: kv projections, attention, fused output projection
    # =====================================================================
    with tc.tile_pool(name="wkv", bufs=1) as wkv, \
         tc.tile_pool(name="sm", bufs=2) as sm:
        wk = wkv.tile([128, 3, 4, 256], F32R, tag="wk")
        wv = wkv.tile([128, 3, 4, 256], F32R, tag="wv")
        wp = wkv.tile([128, 3, 2, 512], F32R, tag="wp")
        for pl in range(3):
            for cj in range(4):
                eng = nc.sync if cj % 2 == 0 else nc.gpsimd
                eng.dma_start(wk[:, pl, cj, :], wk_d[pl, cj])
                eng2 = nc.gpsimd if cj % 2 == 0 else nc.sync
                eng2.dma_start(wv[:, pl, cj, :], wv_d[pl, cj])
            for hp in range(2):
                nc.sync.dma_start(wp[:, pl, hp, :], wp_d[pl, hp])

        # ---- k^T ----
        for half in (range(2) if PHASES >= 2 else []):
            hs = slice(128 * half, 128 * (half + 1))
            for (n0, nn) in NKH:
                pr = psum.tile([128, 512], F32, tag="bank", name="kpr")
                pi = psum.tile([128, 512], F32, tag="bank", name="kpi")
                for cj in range(4):
                    st = cj == 0
                    nc.tensor.matmul(pr[:, :nn], wk[:, 0, cj, hs],
                                     xnTr[:, cj, n0:n0 + nn], start=st,
                                     stop=False)
                    nc.tensor.matmul(pi[:, :nn], wk[:, 0, cj, hs],
                                     xnTi[:, cj, n0:n0 + nn], start=st,
                                     stop=False)
                    nc.tensor.matmul(pr[:, :nn], wk[:, 2, cj, hs],
                                     xnTi[:, cj, n0:n0 + nn], start=False,
                                     stop=False)
                    nc.tensor.matmul(pi[:, :nn], wk[:, 1, cj, hs],
                                     xnTr[:, cj, n0:n0 + nn], start=False,
                                     stop=False)
                nc.tensor.matmul(pr[:, :nn], bk_re[:, hs], ones_row[:, :nn],
                                 start=False, stop=True)
                nc.tensor.matmul(pi[:, :nn], bk_im[:, hs], ones_row[:, :nn],
                                 start=False, stop=True)
                nc.vector.tensor_copy(kTr[:, half, n0:n0 + nn], pr[:, :nn])
                nc.vector.tensor_copy(kTi[:, half, n0:n0 + nn], pi[:, :nn])
                nc.vector.tensor_scalar_mul(kTin[:, half, n0:n0 + nn],
                                            pi[:, :nn], -1.0)

        # ---- v (row-major into vpk) ----
        for kcg in (((0, 1, 2), (3, 4)) if PHASES >= 2 else ()):
            pps = {}
            for kc in kcg:
                pps[kc] = (psum.tile([128, 512], F32, tag="bank",
                                     name=f"vpr{kc}"),
                           psum.tile([128, 512], F32, tag="bank",
                                     name=f"vpi{kc}"))
            for cj in range(4):
                st = cj == 0
                for kc in kcg:
                    k0, szk = K_CHUNKS[kc]
                    pr, pi = pps[kc]
                    nc.tensor.matmul(pr[:szk, :256], xnTr[:, cj, k0:k0 + szk],
                                     wv[:, 0, cj, :], start=st, stop=False)
                    nc.tensor.matmul(pi[:szk, :256], xnTr[:, cj, k0:k0 + szk],
                                     wv[:, 1, cj, :], start=st, stop=False)
                    nc.tensor.matmul(pr[:szk, :256], xnTi[:, cj, k0:k0 + szk],
                                     wv[:, 2, cj, :], start=False, stop=False)
                    nc.tensor.matmul(pi[:szk, :256], xnTi[:, cj, k0:k0 + szk],
                                     wv[:, 0, cj, :], start=False, stop=False)
            for kc in kcg:
                k0, szk = K_CHUNKS[kc]
                pr, pi = pps[kc]
                nc.tensor.matmul(pr[:szk, :256], ones_row[:, :szk], bv_re[:],
                                 start=False, stop=True)
                nc.tensor.matmul(pi[:szk, :256], ones_row[:, :szk], bv_im[:],
                                 start=False, stop=True)
                vr_v = pr[:szk, :256].rearrange("p (h d) -> p h d", h=4)
                vi_v = pi[:szk, :256].rearrange("p (h d) -> p h d", h=4)
                nc.vector.tensor_copy(vpk[:szk, kc, :, 0:64], vr_v)
                nc.vector.tensor_copy(vpk[:szk, kc, :, 64:128], vi_v)

        if DEBUG:
            for half in range(2):
                hs = slice(128 * half, 128 * (half + 1))
                nc.sync.dma_start(dbg["kT"][0, hs, :], kTr[:, half, :].bitcast(F32))
                nc.sync.dma_start(dbg["kT"][1, hs, :], kTi[:, half, :].bitcast(F32))
                nc.sync.dma_start(dbg["kT"][2, hs, :], kTin[:, half, :].bitcast(F32))
            nc.sync.dma_start(dbg["v"][:, :], vpk.rearrange("p a b c -> p (a b c)").bitcast(F32))

        # =================================================================
        # Phase 3: attention; softmax(|scores|) with f16 s-chain
        # =================================================================
        def emit_front(q0, nq, hp):
            stiles = {}
            for kc in range(5):
                k0, szk = K_CHUNKS[kc]
                s16 = sm.tile([128, 2, 512], F16, tag="s16", name=f"s{kc}",
                              bufs=6)
                for i in range(2):
                    rs = slice(64 * i, 64 * (i + 1))
                    sre = psum.tile([128, 512], F32, tag="bank")
                    sim = psum.tile([128, 512], F32, tag="bank")
                    nc.tensor.matmul(sre[:szk, :nq], kTr[rs, hp, k0:k0 + szk],
                                     qTr_sb[rs, hp, q0:q0 + nq], start=True,
                                     stop=False)
                    nc.tensor.matmul(sim[:szk, :nq], kTr[rs, hp, k0:k0 + szk],
                                     qTi_sb[rs, hp, q0:q0 + nq], start=True,
                                     stop=False)
                    nc.tensor.matmul(sre[:szk, :nq], kTin[rs, hp, k0:k0 + szk],
                                     qTi_sb[rs, hp, q0:q0 + nq], start=False,
                                     stop=True)
                    nc.tensor.matmul(sim[:szk, :nq], kTi[rs, hp, k0:k0 + szk],
                                     qTr_sb[rs, hp, q0:q0 + nq], start=False,
                                     stop=True)
                    s1 = sm.tile([128, 512], F16, tag="s1")
                    nc.scalar.activation(s1[:szk, :nq], sre[:szk, :nq],
                                         AF.Square)
                    c2 = sm.tile([128, 512], F16, tag="c2")
                    nc.vector.tensor_copy(c2[:szk, :nq], sim[:szk, :nq])
                    nc.vector.tensor_mul(s16[:szk, i, :nq], c2[:szk, :nq],
                                         c2[:szk, :nq])
                    nc.vector.tensor_add(s16[:szk, i, :nq], s16[:szk, i, :nq],
                                         s1[:szk, :nq])
                stiles[kc] = s16
            # batched sqrt (one table load), then batched exp (one load);
            # |a| must be stored f32: f16 would add |a|*2^-11 logit noise
            abs_ = {}
            for kc in range(5):
                k0_, szk = K_CHUNKS[kc]
                ab = sm.tile([128, 2, 512], F32, tag="ab", name=f"ab{kc}",
                             bufs=5)
                nc.scalar.activation(ab[:szk, :, :nq],
                                     stiles[kc][:szk, :, :nq], AF.Sqrt)
                abs_[kc] = ab
            ebufs = {}
            for kc in range(5):
                k0_, szk = K_CHUNKS[kc]
                ebuf = sm.tile([128, 2, 512], F32R, tag="ebuf", name=f"eb{kc}",
                               bufs=6)
                # constant shift keeps exp sums in f32 range; softmax is
                # shift-invariant so the result is exact
                nc.scalar.activation(ebuf[:szk, :, :nq],
                                     abs_[kc][:szk, :, :nq], AF.Exp,
                                     bias=nbias[:szk])
                ebufs[kc] = ebuf
            return ebufs

        ostore = {}

        def emit_back(qi, q0, nq, hp, ebufs):
            op0 = psum.tile([128, 512], F32, tag="bank", name="op0")
            op1 = psum.tile([128, 512], F32, tag="bank", name="op1")
            dn0 = psum.tile([128, 512], F32, tag="bank", name="dn0")
            dn1 = psum.tile([128, 512], F32, tag="bank", name="dn1")
            ops = (op0, op1)
            dns = (dn0, dn1)
            for kc in range(5):
                k0, szk = K_CHUNKS[kc]
                ebuf = ebufs[kc]
                for i in range(2):
                    hh = 2 * hp + i
                    nc.tensor.matmul(ops[i][:, :nq], vpk[:szk, kc, hh, :],
                                     ebuf[:szk, i, :nq], start=kc == 0,
                                     stop=kc == 4)
                    nc.tensor.matmul(dns[i][:1, :nq], ones_col[:szk, :],
                                     ebuf[:szk, i, :nq], start=kc == 0,
                                     stop=kc == 4)
            otr = sm.tile([128, 512], F32R, tag="otr", bufs=3)
            oti = sm.tile([128, 512], F32R, tag="oti", bufs=3)
            for i in range(2):
                rh = sm.tile([1, 512], F32R, tag="lnd", name=f"rh{i}", bufs=1)
                with nc.allow_low_precision(reason="f32r out is full f32"):
                    nc.vector.reciprocal(rh[:, :nq], dns[i][:1, :nq])
                rbp = psum.tile([128, 512], F32, tag="bank")
                nc.tensor.matmul(rbp[:, :nq], ones_row[:1, :128], rh[:, :nq],
                                 start=True, stop=True)
                rb = sm.tile([128, 512], F32, tag="rb", bufs=2)
                nc.vector.tensor_copy(rb[:, :nq], rbp[:, :nq])
                rs = slice(64 * i, 64 * (i + 1))
                nc.vector.tensor_mul(otr[rs, :nq], ops[i][0:64, :nq],
                                     rb[0:64, :nq])
                nc.vector.tensor_mul(oti[rs, :nq], ops[i][64:128, :nq],
                                     rb[64:128, :nq])
            ostore[(qi, hp)] = (otr, oti)

        def emit_proj(qi, q0, nq):
            for cc in range(4):
                cs = slice(128 * cc, 128 * (cc + 1))
                pr = psum.tile([128, 512], F32, tag="bank")
                pi = psum.tile([128, 512], F32, tag="bank")
                for hp in range(2):
                    otr, oti = ostore[(qi, hp)]
                    st = hp == 0
                    sp = hp == 1
                    nc.tensor.matmul(pr[:, :nq], wp[:, 0, hp, cs],
                                     otr[:, :nq], start=st, stop=False)
                    nc.tensor.matmul(pi[:, :nq], wp[:, 0, hp, cs],
                                     oti[:, :nq], start=st, stop=False)
                    nc.tensor.matmul(pr[:, :nq], wp[:, 2, hp, cs],
                                     oti[:, :nq], start=False, stop=sp)
                    nc.tensor.matmul(pi[:, :nq], wp[:, 1, hp, cs],
                                     otr[:, :nq], start=False, stop=sp)
                o1 = sm.tile([128, 512], F16, tag="o1")
                o2 = sm.tile([128, 512], F16, tag="o2")
                nc.vector.tensor_copy(o1[:, :nq], pr[:, :nq])
                nc.vector.tensor_copy(o2[:, :nq], pi[:, :nq])
                eng = nc.sync if cc % 2 == 0 else nc.gpsimd
                eng.dma_start(outT_d[0, cc, qi, :, :nq], o1[:, :nq])
                eng2 = nc.gpsimd if cc % 2 == 0 else nc.sync
                eng2.dma_start(outT_d[1, cc, qi, :, :nq], o2[:, :nq])

        # software pipeline: next iteration's scores+softmax are emitted
        # before the previous iteration's attn@v / normalize / projection
        prev = None
        for qi, (q0, nq) in enumerate(Q_CHUNKS if PHASES >= 3 else []):
            for hp in range(2):
                ebufs = emit_front(q0, nq, hp)
                if prev is not None:
                    pqi, pq0, pnq, php, pebufs = prev
                    if PHASES >= 4:
                        emit_back(pqi, pq0, pnq, php, pebufs)
                        if php == 1:
                            emit_proj(pqi, pq0, pnq)
                prev = (qi, q0, nq, hp, ebufs)
        if prev is not None and PHASES >= 4:
            pqi, pq0, pnq, php, pebufs = prev
            emit_back(pqi, pq0, pnq, php, pebufs)
            emit_proj(pqi, pq0, pnq)

    ctx.close()


# =========================================================================
# Host side
# =========================================================================

def _f32(x):
    return np.ascontiguousarray(x, dtype=np.float32)


def _perm():
    """q-column permutation: permuted index (p4, nk) -> original n."""
    perm = np.empty(4 * NK, dtype=np.int64)
    for p4 in range(4):
        p, q = p4 // 2, p4 % 2
        for nk in range(NK):
            hi, wi = nk // HR, nk % HR
            perm[p4 * NK + nk] = (SR * hi + p) * (SR * HR) + SR * wi + q
    return perm


_PERM = _perm()


def host_prep(x_re, x_im, Wq, Wkv, Wproj, bproj, sr_w, sr_b, gain, bias):
    x_re = np.asarray(x_re)
    x_im = np.asarray(x_im)
    Wq = np.asarray(Wq)
    Wkv = np.asarray(Wkv)
    Wproj = np.asarray(Wproj)
    sr_w = np.asarray(sr_w)
    sr_b = np.asarray(sr_b)
    gain = np.asarray(gain)
    bias = np.asarray(bias)

    Wkv_eff = gain[:, None] * Wkv
    bkv_full = bias @ Wkv
    Wc = sr_w.transpose(2, 3, 1, 0).reshape(4 * C, C)
    # wc packed [cpass, plane, kk, 128, 256]
    wc_planes = np.stack([_f32(Wc.real), _f32(Wc.imag), _f32(-Wc.imag)])
    wc_pack = np.empty((2, 3, 16, 128, 256), np.float32)
    for cp in range(2):
        wc_pack[cp] = wc_planes[:, :, 256 * cp:256 * (cp + 1)].reshape(
            3, 16, 128, 256)

    def planes4(w):  # [C, 256] -> [3, 4, 128, 256]
        return np.stack([_f32(w.real), _f32(w.imag), _f32(-w.imag)]
                        ).reshape(3, 4, 128, 256)

    in_maps = []
    for core in range(8):
        b, g = core // 2, core % 2
        cols = slice(256 * g, 256 * (g + 1))
        wk_c = Wkv_eff[:, :C][:, cols] * SCALE
        wv_c = Wkv_eff[:, C:][:, cols]
        bk_c = bkv_full[:C][cols] * SCALE
        bv_c = bkv_full[C:][cols]
        xs_c = np.stack([x_re[b].T, x_im[b].T])  # [2, C, N]
        xsp = xs_c.reshape(2, C, HR, 2, HR, 2)
        xP = np.stack([xsp[:, :, :, p, :, q].reshape(2, C, NK)
                       for p in range(2) for q in range(2)], axis=1)
        wp_c = Wproj[256 * g:256 * (g + 1), :]  # [256, C]
        wp_pack = np.stack([_f32(wp_c.real), _f32(wp_c.imag),
                            _f32(-wp_c.imag)]).reshape(3, 2, 128, C)
        m = {
            "xP": _f32(xP.reshape(2, 16, 128, NK)),
            "wc": wc_pack,
            "srb": np.stack([_f32(sr_b.real), _f32(sr_b.imag)]),
            "ones": np.ones((1, 512), np.float32),
            "onesc": np.ones((128, 1), np.float32),
            "wq": planes4(Wq[:, cols]),
            "wk": planes4(wk_c),
            "wv": planes4(wv_c),
            "wp": wp_pack,
            "bkv": np.stack([
                np.stack([_f32(bk_c.real), _f32(bv_c.real)]),
                np.stack([_f32(bk_c.imag), _f32(bv_c.imag)]),
            ]),
        }
        in_maps.append(m)
    return in_maps


_NC_CACHE = None


def _get_nc():
    global _NC_CACHE
    if _NC_CACHE is None:
        _NC_CACHE = build_nc()
    return _NC_CACHE


def kernel(x_re, x_im, Wq, Wkv, Wproj, bproj, sr_w, sr_b, gain, bias, H, W):
    from concourse.bass_utils import run_bass_kernel_spmd

    nc = _get_nc()
    in_maps = host_prep(x_re, x_im, Wq, Wkv, Wproj, bproj, sr_w, sr_b, gain, bias)
    res = run_bass_kernel_spmd(nc, in_maps, list(range(8)))
    bproj = np.asarray(bproj)
    out = np.zeros((B, N, C), dtype=np.complex64)
    for b in range(B):
        # outT: [2, 4, 5, 128, 512] -> [2, 512 c, 2560 q-padded]
        p0 = res.results[2 * b]["outT"].astype(np.float32)
        p1 = res.results[2 * b + 1]["outT"].astype(np.float32)
        acc = p0 + p1  # [2, 4cc, 5qc, 128, 512]
        accf = acc.transpose(0, 1, 3, 2, 4).reshape(2, 512, 5 * 512)[:, :, :N]
        full = np.empty((N, C), np.complex64)
        full[_PERM, :] = (accf[0] + 1j * accf[1]).T
        out[b] = full + bproj[None, :]
    return out


# revision 35
# speedup vs baseline: 1.4774x; 1.0395x over previous
"""Trainium2 Bass kernel for complex-valued spatial-reduction attention.

x: [B=4, N=2304, C=512] complex64 (re/im f32 planes), H=W=48, 8 heads,
head_dim 64, sr_ratio 2 -> Nk=576.

Sharding: 8 cores = 4 batches x 2 head-groups (4 heads each). Each core:
sr-conv over full C, complex LayerNorm, q/k/v for its heads,
softmax(|q.k^T|) attention, attn @ v, partial output projection.
Host sums the two partials per batch and adds bproj.

Everything stays on-chip: xP (patch-permuted x) is resident in SBUF and
feeds both the conv and the q-projection (q columns come out in
(patch-pos, nk) order; the host unpermutes). q / attention-out / weights
are SBUF-resident, so HBM traffic is inputs + weights + output only.

Precision: the f32r matmul path (~tf32-class rounding) dominates the
error budget; the softmax s=re^2+im^2 chain runs in f16 which measures
as noise against that. ebuf/v/proj are f32r.
"""

import os
import contextlib

import numpy as np
import ml_dtypes

import concourse.bass as bass
import concourse.mybir as mybir
import concourse.tile as tile
from concourse import bacc
from concourse.masks import make_identity

BF16 = mybir.dt.bfloat16
F16 = mybir.dt.float16
F32 = mybir.dt.float32
F32R = mybir.dt.float32r
AF = mybir.ActivationFunctionType
ALU = mybir.AluOpType

B, N, C, HEADS, HD, SR = 4, 2304, 512, 8, 64, 2
NK = 576
HR = 24
EPS = 1e-5
SCALE = HD ** -0.5  # folded into Wk host-side

CHUNKS5 = [(0, 120), (120, 120), (240, 120), (360, 120), (480, 96)]
NKH = [(0, 288), (288, 288)]
K_CHUNKS = [(0, 128), (128, 128), (256, 128), (384, 128), (512, 64)]
Q_CHUNKS = [(0, 512), (512, 512), (1024, 512), (1536, 512), (2048, 256)]

DEBUG = bool(int(os.environ.get("KBUILD_DEBUG", "0")))
PHASES = int(os.environ.get("KBUILD_PHASES", "4"))
P1MASK = int(os.environ.get("KBUILD_P1", "7"))  # 1=conv 2=qproj 4=ln


def _r(ap):
    return ap.bitcast(F32R)


def build_nc():
    nc = bacc.Bacc("TRN2", target_bir_lowering=False, debug=False, num_devices=8)

    xP_d = nc.dram_tensor("xP", [2, 16, 128, NK], F32R, kind="ExternalInput")
    wc_d = nc.dram_tensor("wc", [2, 3, 16, 128, 256], F32R, kind="ExternalInput")
    srb_d = nc.dram_tensor("srb", [2, C], F32R, kind="ExternalInput")
    ones_d = nc.dram_tensor("ones", [1, 512], F32R, kind="ExternalInput")
    onesc_d = nc.dram_tensor("onesc", [128, 1], F32R, kind="ExternalInput")
    wq_d = nc.dram_tensor("wq", [3, 4, 128, 256], F32R, kind="ExternalInput")
    wk_d = nc.dram_tensor("wk", [3, 4, 128, 256], F32R, kind="ExternalInput")
    wv_d = nc.dram_tensor("wv", [3, 4, 128, 256], F32R, kind="ExternalInput")
    wp_d = nc.dram_tensor("wp", [3, 2, 128, 512], F32R, kind="ExternalInput")
    bkv_d = nc.dram_tensor("bkv", [2, 2, 256], F32R, kind="ExternalInput")
    # output: [plane, c-block, q-chunk, 128 c, 512 q] (q cols permuted (p4, nk))
    outT_d = nc.dram_tensor("outT", [2, 4, 5, 128, 512], F16, kind="ExternalOutput")
    dbg = {}
    if DEBUG:
        dbg["xnT"] = nc.dram_tensor("dbg_xnT", [2, C, NK], F32, kind="ExternalOutput")
        dbg["qT"] = nc.dram_tensor("dbg_qT", [2, 2, 128, N], F32, kind="ExternalOutput")
        dbg["kT"] = nc.dram_tensor("dbg_kT", [3, 256, NK], F32, kind="ExternalOutput")
        dbg["v"] = nc.dram_tensor("dbg_v", [128, 5 * 4 * 128], F32, kind="ExternalOutput")
        dbg["conv"] = nc.dram_tensor("dbg_conv", [2, NK, C], F32, kind="ExternalOutput")

    with tile.TileContext(nc) as tc:
        _body(nc, tc, xP_d, wc_d, srb_d, ones_d, onesc_d, wq_d, wk_d,
              wv_d, wp_d, bkv_d, outT_d, dbg)

    nc.compile()
    return nc


def _ln2(nc, work, stats, stg, sz):
    """Complex LayerNorm for one chunk; stg is [128, 2, 2, 256]
    (re|im half, conv C-pass, 256 cols) so re/im are each contiguous."""
    inv_c = 1.0 / C
    re_sb = stg[:, 0].rearrange("p a b -> p (a b)")
    im_sb = stg[:, 1].rearrange("p a b -> p (a b)")
    sum_r = stats.tile([128, 1], F32, tag="sum_r")
    sum_i = stats.tile([128, 1], F32, tag="sum_i")
    nc.vector.tensor_reduce(sum_r[:sz], re_sb[:sz], mybir.AxisListType.X, ALU.add)
    nc.vector.tensor_reduce(sum_i[:sz], im_sb[:sz], mybir.AxisListType.X, ALU.add)
    junk = work.tile([128, C], F32, tag="ln_a", bufs=1, name="junk")
    sxx = stats.tile([128, 1], F32, tag="sxx")
    sii = stats.tile([128, 1], F32, tag="sii")
    sxi = stats.tile([128, 1], F32, tag="sxi")
    nc.vector.tensor_mul(junk[:sz], re_sb[:sz], re_sb[:sz])
    nc.vector.tensor_reduce(sxx[:sz], junk[:sz], mybir.AxisListType.X, ALU.add)
    nc.vector.tensor_mul(junk[:sz], im_sb[:sz], im_sb[:sz])
    nc.vector.tensor_reduce(sii[:sz], junk[:sz], mybir.AxisListType.X, ALU.add)
    nc.vector.tensor_mul(junk[:sz], re_sb[:sz], im_sb[:sz])
    nc.vector.tensor_reduce(sxi[:sz], junk[:sz], mybir.AxisListType.X, ALU.add)
    mr = stats.tile([128, 1], F32, tag="mr")
    mi = stats.tile([128, 1], F32, tag="mi")
    nc.vector.tensor_scalar_mul(mr[:sz], sum_r[:sz], inv_c)
    nc.vector.tensor_scalar_mul(mi[:sz], sum_i[:sz], inv_c)
    vre = stats.tile([128, 1], F32, tag="vre")
    vim = stats.tile([128, 1], F32, tag="vim")
    tA = stats.tile([128, 1], F32, tag="tA")
    tB = stats.tile([128, 1], F32, tag="tB")
    nc.vector.tensor_sub(tA[:sz], sxx[:sz], sii[:sz])
    nc.vector.tensor_scalar_mul(tA[:sz], tA[:sz], inv_c)
    nc.vector.tensor_mul(vre[:sz], mr[:sz], mr[:sz])
    nc.vector.tensor_mul(tB[:sz], mi[:sz], mi[:sz])
    nc.vector.tensor_sub(vre[:sz], vre[:sz], tB[:sz])
    nc.vector.tensor_sub(vre[:sz], tA[:sz], vre[:sz])
    nc.vector.tensor_scalar_add(vre[:sz], vre[:sz], EPS)
    nc.vector.tensor_mul(tB[:sz], mr[:sz], mi[:sz])
    nc.vector.tensor_scalar_mul(tB[:sz], tB[:sz], 2.0)
    nc.vector.tensor_scalar_mul(vim[:sz], sxi[:sz], 2.0 * inv_c)
    nc.vector.tensor_sub(vim[:sz], vim[:sz], tB[:sz])
    # complex rsqrt of (vre + i vim): w = conj(sqrt(v)) / |v|
    r2 = stats.tile([128, 1], F32, tag="r2")
    nc.vector.tensor_mul(r2[:sz], vre[:sz], vre[:sz])
    nc.vector.tensor_mul(tB[:sz], vim[:sz], vim[:sz])
    nc.vector.tensor_add(r2[:sz], r2[:sz], tB[:sz])
    def _sqrt_newton(out, x, sc):
        # y0 = LUT sqrt(sc*x); y1 = 0.5*(y0 + sc*x/y0)  (one Newton step)
        y0 = stats.tile([128, 1], F32, tag="nw_y0")
        nc.scalar.activation(y0[:sz], x[:sz], AF.Sqrt, scale=sc)
        yr = stats.tile([128, 1], F32, tag="nw_yr")
        nc.vector.tensor_scalar_add(y0[:sz], y0[:sz], 1e-30)
        nc.vector.reciprocal(yr[:sz], y0[:sz])
        nc.vector.tensor_mul(yr[:sz], yr[:sz], x[:sz])
        if sc != 1.0:
            nc.vector.tensor_scalar_mul(yr[:sz], yr[:sz], sc)
        nc.vector.tensor_add(out[:sz], y0[:sz], yr[:sz])
        nc.vector.tensor_scalar_mul(out[:sz], out[:sz], 0.5)

    rr = stats.tile([128, 1], F32, tag="rr")
    _sqrt_newton(rr, r2, 1.0)  # |v|
    srt = stats.tile([128, 1], F32, tag="srt")
    sia = stats.tile([128, 1], F32, tag="sia")
    nc.vector.tensor_add(tA[:sz], rr[:sz], vre[:sz])
    _sqrt_newton(srt, tA, 0.5)  # Re sqrt(v)
    nc.vector.tensor_sub(tA[:sz], rr[:sz], vre[:sz])
    _sqrt_newton(sia, tA, 0.5)  # |Im sqrt(v)|
    sgn = stats.tile([128, 1], F32, tag="sgn")
    nc.scalar.activation(sgn[:sz], vim[:sz], AF.Sign)
    nc.vector.tensor_mul(sia[:sz], sia[:sz], sgn[:sz])
    rin = stats.tile([128, 1], F32, tag="rin")
    nc.vector.reciprocal(rin[:sz], rr[:sz])
    wr = stats.tile([128, 1], F32, tag="wr")
    wn = stats.tile([128, 1], F32, tag="wn")  # = -w_im
    nc.vector.tensor_mul(wr[:sz], srt[:sz], rin[:sz])
    nc.vector.tensor_mul(wn[:sz], sia[:sz], rin[:sz])
    # xn = w * (x - m), complex
    aT = work.tile([128, C], F32, tag="ln_a", bufs=1)
    bT = work.tile([128, C], F32, tag="ln_b", bufs=1)
    xnr = work.tile([128, C], F32, tag="ln_xnr", bufs=1)
    xni = work.tile([128, C], F32, tag="ln_xni", bufs=1)
    nc.vector.tensor_scalar(aT[:sz], re_sb[:sz], mr[:sz], wr[:sz],
                            ALU.subtract, ALU.mult)
    nc.vector.tensor_scalar(bT[:sz], im_sb[:sz], mi[:sz], wn[:sz],
                            ALU.subtract, ALU.mult)
    nc.vector.tensor_add(xnr[:sz], aT[:sz], bT[:sz])
    nc.vector.tensor_scalar(aT[:sz], re_sb[:sz], mr[:sz], wn[:sz],
                            ALU.subtract, ALU.mult)
    nc.vector.tensor_scalar(bT[:sz], im_sb[:sz], mi[:sz], wr[:sz],
                            ALU.subtract, ALU.mult)
    nc.vector.tensor_sub(xni[:sz], bT[:sz], aT[:sz])
    return xnr, xni


def _body(nc, tc, xP_d, wc_d, srb_d, ones_d, onesc_d, wq_d, wk_d, wv_d,
          wp_d, bkv_d, outT_d, dbg):
    ctx = contextlib.ExitStack()
    consts = ctx.enter_context(tc.tile_pool(name="consts", bufs=1))
    big = ctx.enter_context(tc.tile_pool(name="big", bufs=1))
    stats = ctx.enter_context(tc.tile_pool(name="stats", bufs=2))
    psum = ctx.enter_context(tc.tile_pool(name="psum", bufs=7, space="PSUM"))

    # ---- constants ----
    ident = consts.tile([128, 128], F32, tag="ident")
    make_identity(nc, ident)
    ones_col = consts.tile([128, 1], F32R, tag="ones_col")
    nc.sync.dma_start(ones_col[:], onesc_d[:, :])
    ones_row = consts.tile([1, 512], F32R, tag="ones_row")
    nc.sync.dma_start(ones_row[:], ones_d[:])
    nbias = consts.tile([128, 1], F32, tag="nbias")
    nc.vector.memset(nbias, -50.0)
    srb_re = consts.tile([1, C], F32R, tag="srb_re")
    srb_im = consts.tile([1, C], F32R, tag="srb_im")
    nc.sync.dma_start(srb_re[:], srb_d[0:1, :])
    nc.sync.dma_start(srb_im[:], srb_d[1:2, :])
    bk_re = consts.tile([1, 256], F32R, tag="bk_re")
    bk_im = consts.tile([1, 256], F32R, tag="bk_im")
    bv_re = consts.tile([1, 256], F32R, tag="bv_re")
    bv_im = consts.tile([1, 256], F32R, tag="bv_im")
    nc.sync.dma_start(bk_re[:], bkv_d[0:1, 0, :])
    nc.sync.dma_start(bv_re[:], bkv_d[0:1, 1, :])
    nc.sync.dma_start(bk_im[:], bkv_d[1:2, 0, :])
    nc.sync.dma_start(bv_im[:], bkv_d[1:2, 1, :])

    # ---- whole-kernel resident SBUF ----
    qTr_sb = big.tile([128, 2, N], F32R, tag="qTr_sb")
    qTi_sb = big.tile([128, 2, N], F32R, tag="qTi_sb")
    xnTr = big.tile([128, 4, NK], F32R, tag="xnTr")
    xnTi = big.tile([128, 4, NK], F32R, tag="xnTi")
    kTr = big.tile([128, 2, NK], F32R, tag="kTr")
    kTi = big.tile([128, 2, NK], F32R, tag="kTi")
    kTin = big.tile([128, 2, NK], F32R, tag="kTin")
    vpk = big.tile([128, 5, 4, 128], F32R, tag="vpk")
    if DEBUG:
        nc.vector.memset(vpk.bitcast(F32), 0.0)

    # =====================================================================
    # Phase 1: conv (2 C-half passes) + q-projection from resident xP + LN
    # =====================================================================
    with tc.tile_pool(name="xpp", bufs=1) as xpp, \
         tc.tile_pool(name="cwork", bufs=2) as cwork:
        xP = xpp.tile([128, 2, 16, NK], F32R, tag="xP")
        wq = xpp.tile([128, 3, 4, 256], F32R, tag="wq")
        stg = []
        for ch in range(5):
            # [half(re|im), cpass, 256]: re ends up contiguous in cols 0:512
            stg.append(xpp.tile([128, 2, 2, 256], F32, tag=f"stg{ch}",
                                name=f"stg{ch}"))

        for kk in range(16):
            eng = nc.sync if kk % 2 == 0 else nc.gpsimd
            eng.dma_start(xP[:, 0, kk, :], xP_d[0, kk])
            eng2 = nc.gpsimd if kk % 2 == 0 else nc.sync
            eng2.dma_start(xP[:, 1, kk, :], xP_d[1, kk])
        for pl in range(3):
            for cj in range(4):
                eng = nc.sync if (pl * 4 + cj) % 2 == 0 else nc.gpsimd
                eng.dma_start(wq[:, pl, cj, :], wq_d[pl, cj])

        # ---- conv: for each output-C half, accumulate all 5 row chunks ----
        for cp in (range(2) if P1MASK & 1 else []):
            cps = []
            for ch, (t0, sz) in enumerate(CHUNKS5):
                cps.append(psum.tile([128, 512], F32, tag="bank",
                                     name=f"conv{cp}_{ch}"))
            for kk in range(16):
                wcr = cwork.tile([128, 256], F32R, tag="wc_r")
                wci = cwork.tile([128, 256], F32R, tag="wc_i")
                wcn = cwork.tile([128, 256], F32R, tag="wc_n")
                nc.sync.dma_start(wcr[:], wc_d[cp, 0, kk])
                nc.gpsimd.dma_start(wci[:], wc_d[cp, 1, kk])
                nc.sync.dma_start(wcn[:], wc_d[cp, 2, kk])
                st = kk == 0
                for ch, (t0, sz) in enumerate(CHUNKS5):
                    pat_r = xP[:, 0, kk, t0:t0 + sz]
                    pat_i = xP[:, 1, kk, t0:t0 + sz]
                    cpt = cps[ch]
                    # one accumulation group per bank: start only on the
                    # very first matmul, stop only on the last (im bias)
                    nc.tensor.matmul(cpt[:sz, 0:256], pat_r, wcr[:],
                                     start=st, stop=False)
                    nc.tensor.matmul(cpt[:sz, 256:512], pat_r, wci[:],
                                     start=False, stop=False)
                    nc.tensor.matmul(cpt[:sz, 0:256], pat_i, wcn[:],
                                     start=False, stop=False)
                    nc.tensor.matmul(cpt[:sz, 256:512], pat_i, wcr[:],
                                     start=False, stop=False)
            cs = slice(256 * cp, 256 * (cp + 1))
            for ch, (t0, sz) in enumerate(CHUNKS5):
                cpt = cps[ch]
                nc.tensor.matmul(cpt[:sz, 0:256], ones_row[:, :sz],
                                 srb_re[:, cs], start=False, stop=False)
                nc.tensor.matmul(cpt[:sz, 256:512], ones_row[:, :sz],
                                 srb_im[:, cs], start=False, stop=True)
            for ch, (t0, sz) in enumerate(CHUNKS5):
                # whole-bank copy: depends on every write, so it cannot race
                # the PE still accumulating into the other half
                bank_v = cps[ch].rearrange("p (a b) -> p a b", a=2)
                nc.vector.tensor_copy(stg[ch][:sz, :, cp, :], bank_v[:sz])

        # ---- q-projection from resident xP (PE; LN below runs on DVE) ----
        for p4 in (range(4) if P1MASK & 2 else []):
            for (n0, nn) in NKH:
                prs = []
                for half in range(2):
                    prs.append((psum.tile([128, 512], F32, tag="bank",
                                          name=f"qpr{half}"),
                                psum.tile([128, 512], F32, tag="bank",
                                          name=f"qpi{half}")))
                for cj in range(4):
                    kk = p4 * 4 + cj
                    xr = xP[:, 0, kk, n0:n0 + nn]
                    xi = xP[:, 1, kk, n0:n0 + nn]
                    st = cj == 0
                    sp = cj == 3
                    for half in range(2):
                        hs = slice(128 * half, 128 * (half + 1))
                        pr, pi = prs[half]
                        nc.tensor.matmul(pr[:, :nn], wq[:, 0, cj, hs], xr,
                                         start=st, stop=False)
                        nc.tensor.matmul(pi[:, :nn], wq[:, 0, cj, hs], xi,
                                         start=st, stop=False)
                        nc.tensor.matmul(pr[:, :nn], wq[:, 2, cj, hs], xi,
                                         start=False, stop=sp)
                        nc.tensor.matmul(pi[:, :nn], wq[:, 1, cj, hs], xr,
                                         start=False, stop=sp)
                for half in range(2):
                    pr, pi = prs[half]
                    q0 = p4 * NK + n0
                    nc.scalar.copy(qTr_sb[:, half, q0:q0 + nn], pr[:, :nn])
                    nc.scalar.copy(qTi_sb[:, half, q0:q0 + nn], pi[:, :nn])

        # ---- LayerNorm (DVE, overlaps q-proj PE) + transposes into xnT ----
        for ch, (t0, sz) in enumerate(CHUNKS5 if (P1MASK & 4 and P1MASK & 1) else []):
            xnr, xni = _ln2(nc, cwork, stats, stg[ch], sz)
            for cj in range(4):
                for src, dst in ((xnr, xnTr), (xni, xnTi)):
                    pt = psum.tile([128, 128], F32, tag="bank", name="tp")
                    nc.tensor.transpose(pt[:, :sz],
                                        src[:sz, 128 * cj:128 * (cj + 1)],
                                        ident[:sz, :sz])
                    nc.vector.tensor_copy(dst[:, cj, t0:t0 + sz], pt[:, :sz])

    if DEBUG:
        for cj in range(4):
            nc.sync.dma_start(dbg["xnT"][0, 128 * cj:128 * (cj + 1), :],
                              xnTr[:, cj, :].bitcast(F32))
            nc.sync.dma_start(dbg["xnT"][1, 128 * cj:128 * (cj + 1), :],
                              xnTi[:, cj, :].bitcast(F32))
        for half in range(2):
            nc.sync.dma_start(dbg["qT"][0, half, :, :],
                              qTr_sb[:, half, :].bitcast(F32))
            nc.sync.dma_start(dbg["qT"][1, half, :, :],
                              qTi_sb[:, half, :].bitcast(F32))

    # =====================================================================
    # Phases 2-4: kv projections, attention, fused output projection
    # =====================================================================
    with tc.tile_pool(name="wkv", bufs=1) as wkv, \
         tc.tile_pool(name="sm", bufs=2) as sm:
        wk = wkv.tile([128, 3, 4, 256], F32R, tag="wk")
        wv = wkv.tile([128, 3, 4, 256], F32R, tag="wv")
        wp = wkv.tile([128, 3, 2, 512], F32R, tag="wp")
        for pl in range(3):
            for cj in range(4):
                eng = nc.sync if cj % 2 == 0 else nc.gpsimd
                eng.dma_start(wk[:, pl, cj, :], wk_d[pl, cj])
                eng2 = nc.gpsimd if cj % 2 == 0 else nc.sync
                eng2.dma_start(wv[:, pl, cj, :], wv_d[pl, cj])
            for hp in range(2):
                nc.sync.dma_start(wp[:, pl, hp, :], wp_d[pl, hp])

        # ---- k^T ----
        for half in (range(2) if PHASES >= 2 else []):
            hs = slice(128 * half, 128 * (half + 1))
            for (n0, nn) in NKH:
                pr = psum.tile([128, 512], F32, tag="bank", name="kpr")
                pi = psum.tile([128, 512], F32, tag="bank", name="kpi")
                for cj in range(4):
                    st = cj == 0
                    nc.tensor.matmul(pr[:, :nn], wk[:, 0, cj, hs],
                                     xnTr[:, cj, n0:n0 + nn], start=st,
                                     stop=False)
                    nc.tensor.matmul(pi[:, :nn], wk[:, 0, cj, hs],
                                     xnTi[:, cj, n0:n0 + nn], start=st,
                                     stop=False)
                    nc.tensor.matmul(pr[:, :nn], wk[:, 2, cj, hs],
                                     xnTi[:, cj, n0:n0 + nn], start=False,
                                     stop=False)
                    nc.tensor.matmul(pi[:, :nn], wk[:, 1, cj, hs],
                                     xnTr[:, cj, n0:n0 + nn], start=False,
                                     stop=False)
                nc.tensor.matmul(pr[:, :nn], bk_re[:, hs], ones_row[:, :nn],
                                 start=False, stop=True)
                nc.tensor.matmul(pi[:, :nn], bk_im[:, hs], ones_row[:, :nn],
                                 start=False, stop=True)
                nc.vector.tensor_copy(kTr[:, half, n0:n0 + nn], pr[:, :nn])
                nc.vector.tensor_copy(kTi[:, half, n0:n0 + nn], pi[:, :nn])
                nc.vector.tensor_scalar_mul(kTin[:, half, n0:n0 + nn],
                                            pi[:, :nn], -1.0)

        # ---- v (row-major into vpk) ----
        for kcg in (((0, 1, 2), (3, 4)) if PHASES >= 2 else ()):
            pps = {}
            for kc in kcg:
                pps[kc] = (psum.tile([128, 512], F32, tag="bank",
                                     name=f"vpr{kc}"),
                           psum.tile([128, 512], F32, tag="bank",
                                     name=f"vpi{kc}"))
            for cj in range(4):
                st = cj == 0
                for kc in kcg:
                    k0, szk = K_CHUNKS[kc]
                    pr, pi = pps[kc]
                    nc.tensor.matmul(pr[:szk, :256], xnTr[:, cj, k0:k0 + szk],
                                     wv[:, 0, cj, :], start=st, stop=False)
                    nc.tensor.matmul(pi[:szk, :256], xnTr[:, cj, k0:k0 + szk],
                                     wv[:, 1, cj, :], start=st, stop=False)
                    nc.tensor.matmul(pr[:szk, :256], xnTi[:, cj, k0:k0 + szk],
                                     wv[:, 2, cj, :], start=False, stop=False)
                    nc.tensor.matmul(pi[:szk, :256], xnTi[:, cj, k0:k0 + szk],
                                     wv[:, 0, cj, :], start=False, stop=False)
            for kc in kcg:
                k0, szk = K_CHUNKS[kc]
                pr, pi = pps[kc]
                nc.tensor.matmul(pr[:szk, :256], ones_row[:, :szk], bv_re[:],
                                 start=False, stop=True)
                nc.tensor.matmul(pi[:szk, :256], ones_row[:, :szk], bv_im[:],
                                 start=False, stop=True)
                vr_v = pr[:szk, :256].rearrange("p (h d) -> p h d", h=4)
                vi_v = pi[:szk, :256].rearrange("p (h d) -> p h d", h=4)
                nc.vector.tensor_copy(vpk[:szk, kc, :, 0:64], vr_v)
                nc.vector.tensor_copy(vpk[:szk, kc, :, 64:128], vi_v)

        if DEBUG:
            for half in range(2):
                hs = slice(128 * half, 128 * (half + 1))
                nc.sync.dma_start(dbg["kT"][0, hs, :], kTr[:, half, :].bitcast(F32))
                nc.sync.dma_start(dbg["kT"][1, hs, :], kTi[:, half, :].bitcast(F32))
                nc.sync.dma_start(dbg["kT"][2, hs, :], kTin[:, half, :].bitcast(F32))
            nc.sync.dma_start(dbg["v"][:, :], vpk.rearrange("p a b c -> p (a b c)").bitcast(F32))

        # =================================================================
        # Phase 3: attention; softmax(|scores|) with f16 s-chain
        # =================================================================
        def emit_front(q0, nq, hp):
            stiles = {}
            for kc in range(5):
                k0, szk = K_CHUNKS[kc]
                s16 = sm.tile([128, 2, 512], F16, tag="s16", name=f"s{kc}",
                              bufs=6)
                for i in range(2):
                    rs = slice(64 * i, 64 * (i + 1))
                    sre = psum.tile([128, 512], F32, tag="bank")
                    sim = psum.tile([128, 512], F32, tag="bank")
                    nc.tensor.matmul(sre[:szk, :nq], kTr[rs, hp, k0:k0 + szk],
                                     qTr_sb[rs, hp, q0:q0 + nq], start=True,
                                     stop=False)
                    nc.tensor.matmul(sim[:szk, :nq], kTr[rs, hp, k0:k0 + szk],
                                     qTi_sb[rs, hp, q0:q0 + nq], start=True,
                                     stop=False)
                    nc.tensor.matmul(sre[:szk, :nq], kTin[rs, hp, k0:k0 + szk],
                                     qTi_sb[rs, hp, q0:q0 + nq], start=False,
                                     stop=True)
                    nc.tensor.matmul(sim[:szk, :nq], kTi[rs, hp, k0:k0 + szk],
                                     qTr_sb[rs, hp, q0:q0 + nq], start=False,
                                     stop=True)
                    s1 = sm.tile([128, 512], F16, tag="s1")
                    nc.scalar.activation(s1[:szk, :nq], sre[:szk, :nq],
                                         AF.Square)
                    if 2 * kc + i < 3:
                        # ACT/DVE balance: a few tiles square im on ACT too
                        s2t = sm.tile([128, 512], F16, tag="c2")
                        nc.scalar.activation(s2t[:szk, :nq], sim[:szk, :nq],
                                             AF.Square)
                        nc.vector.tensor_add(s16[:szk, i, :nq],
                                             s2t[:szk, :nq], s1[:szk, :nq])
                    else:
                        c2 = sm.tile([128, 512], F16, tag="c2")
                        nc.vector.tensor_copy(c2[:szk, :nq], sim[:szk, :nq])
                        nc.vector.tensor_mul(s16[:szk, i, :nq], c2[:szk, :nq],
                                             c2[:szk, :nq])
                        nc.vector.tensor_add(s16[:szk, i, :nq],
                                             s16[:szk, i, :nq], s1[:szk, :nq])
                stiles[kc] = s16
            # batched sqrt (one table load), then batched exp (one load);
            # |a| must be stored f32: f16 would add |a|*2^-11 logit noise
            abs_ = {}
            for kc in range(5):
                k0_, szk = K_CHUNKS[kc]
                ab = sm.tile([128, 2, 512], F32, tag="ab", name=f"ab{kc}",
                             bufs=5)
                nc.scalar.activation(ab[:szk, :, :nq],
                                     stiles[kc][:szk, :, :nq], AF.Sqrt)
                abs_[kc] = ab
            ebufs = {}
            for kc in range(5):
                k0_, szk = K_CHUNKS[kc]
                ebuf = sm.tile([128, 2, 512], F32R, tag="ebuf", name=f"eb{kc}",
                               bufs=6)
                # constant shift keeps exp sums in f32 range; softmax is
                # shift-invariant so the result is exact
                nc.scalar.activation(ebuf[:szk, :, :nq],
                                     abs_[kc][:szk, :, :nq], AF.Exp,
                                     bias=nbias[:szk])
                ebufs[kc] = ebuf
            return ebufs

        ostore = {}

        def emit_back(qi, q0, nq, hp, ebufs):
            op0 = psum.tile([128, 512], F32, tag="bank", name="op0")
            op1 = psum.tile([128, 512], F32, tag="bank", name="op1")
            dn0 = psum.tile([128, 512], F32, tag="bank", name="dn0")
            dn1 = psum.tile([128, 512], F32, tag="bank", name="dn1")
            ops = (op0, op1)
            dns = (dn0, dn1)
            for kc in range(5):
                k0, szk = K_CHUNKS[kc]
                ebuf = ebufs[kc]
                for i in range(2):
                    hh = 2 * hp + i
                    nc.tensor.matmul(ops[i][:, :nq], vpk[:szk, kc, hh, :],
                                     ebuf[:szk, i, :nq], start=kc == 0,
                                     stop=kc == 4)
                    nc.tensor.matmul(dns[i][:1, :nq], ones_col[:szk, :],
                                     ebuf[:szk, i, :nq], start=kc == 0,
                                     stop=kc == 4)
            otr = sm.tile([128, 512], F32R, tag="otr", bufs=3)
            oti = sm.tile([128, 512], F32R, tag="oti", bufs=3)
            for i in range(2):
                rh = sm.tile([1, 512], F32R, tag="lnd", name=f"rh{i}", bufs=1)
                with nc.allow_low_precision(reason="f32r out is full f32"):
                    nc.vector.reciprocal(rh[:, :nq], dns[i][:1, :nq])
                rbp = psum.tile([128, 512], F32, tag="bank")
                nc.tensor.matmul(rbp[:, :nq], ones_row[:1, :128], rh[:, :nq],
                                 start=True, stop=True)
                rb = sm.tile([128, 512], F32, tag="rb", bufs=2)
                nc.vector.tensor_copy(rb[:, :nq], rbp[:, :nq])
                rs = slice(64 * i, 64 * (i + 1))
                nc.vector.tensor_mul(otr[rs, :nq], ops[i][0:64, :nq],
                                     rb[0:64, :nq])
                nc.vector.tensor_mul(oti[rs, :nq], ops[i][64:128, :nq],
                                     rb[64:128, :nq])
            ostore[(qi, hp)] = (otr, oti)

        def emit_proj(qi, q0, nq):
            for cc in range(4):
                cs = slice(128 * cc, 128 * (cc + 1))
                pr = psum.tile([128, 512], F32, tag="bank")
                pi = psum.tile([128, 512], F32, tag="bank")
                for hp in range(2):
                    otr, oti = ostore[(qi, hp)]
                    st = hp == 0
                    sp = hp == 1
                    nc.tensor.matmul(pr[:, :nq], wp[:, 0, hp, cs],
                                     otr[:, :nq], start=st, stop=False)
                    nc.tensor.matmul(pi[:, :nq], wp[:, 0, hp, cs],
                                     oti[:, :nq], start=st, stop=False)
                    nc.tensor.matmul(pr[:, :nq], wp[:, 2, hp, cs],
                                     oti[:, :nq], start=False, stop=sp)
                    nc.tensor.matmul(pi[:, :nq], wp[:, 1, hp, cs],
                                     otr[:, :nq], start=False, stop=sp)
                o1 = sm.tile([128, 512], F16, tag="o1")
                o2 = sm.tile([128, 512], F16, tag="o2")
                nc.vector.tensor_copy(o1[:, :nq], pr[:, :nq])
                nc.vector.tensor_copy(o2[:, :nq], pi[:, :nq])
                eng = nc.sync if cc % 2 == 0 else nc.gpsimd
                eng.dma_start(outT_d[0, cc, qi, :, :nq], o1[:, :nq])
                eng2 = nc.gpsimd if cc % 2 == 0 else nc.sync
                eng2.dma_start(outT_d[1, cc, qi, :, :nq], o2[:, :nq])

        # software pipeline: next iteration's scores+softmax are emitted
        # before the previous iteration's attn@v / normalize / projection
        prev = None
        for qi, (q0, nq) in enumerate(Q_CHUNKS if PHASES >= 3 else []):
            for hp in range(2):
                ebufs = emit_front(q0, nq, hp)
                if prev is not None:
                    pqi, pq0, pnq, php, pebufs = prev
                    if PHASES >= 4:
                        emit_back(pqi, pq0, pnq, php, pebufs)
                        if php == 1:
                            emit_proj(pqi, pq0, pnq)
                prev = (qi, q0, nq, hp, ebufs)
        if prev is not None and PHASES >= 4:
            pqi, pq0, pnq, php, pebufs = prev
            emit_back(pqi, pq0, pnq, php, pebufs)
            emit_proj(pqi, pq0, pnq)

    ctx.close()


# =========================================================================
# Host side
# =========================================================================

def _f32(x):
    return np.ascontiguousarray(x, dtype=np.float32)


def _perm():
    """q-column permutation: permuted index (p4, nk) -> original n."""
    perm = np.empty(4 * NK, dtype=np.int64)
    for p4 in range(4):
        p, q = p4 // 2, p4 % 2
        for nk in range(NK):
            hi, wi = nk // HR, nk % HR
            perm[p4 * NK + nk] = (SR * hi + p) * (SR * HR) + SR * wi + q
    return perm


_PERM = _perm()


def host_prep(x_re, x_im, Wq, Wkv, Wproj, bproj, sr_w, sr_b, gain, bias):
    x_re = np.asarray(x_re)
    x_im = np.asarray(x_im)
    Wq = np.asarray(Wq)
    Wkv = np.asarray(Wkv)
    Wproj = np.asarray(Wproj)
    sr_w = np.asarray(sr_w)
    sr_b = np.asarray(sr_b)
    gain = np.asarray(gain)
    bias = np.asarray(bias)

    Wkv_eff = gain[:, None] * Wkv
    bkv_full = bias @ Wkv
    Wc = sr_w.transpose(2, 3, 1, 0).reshape(4 * C, C)
    # wc packed [cpass, plane, kk, 128, 256]
    wc_planes = np.stack([_f32(Wc.real), _f32(Wc.imag), _f32(-Wc.imag)])
    wc_pack = np.empty((2, 3, 16, 128, 256), np.float32)
    for cp in range(2):
        wc_pack[cp] = wc_planes[:, :, 256 * cp:256 * (cp + 1)].reshape(
            3, 16, 128, 256)

    def planes4(w):  # [C, 256] -> [3, 4, 128, 256]
        return np.stack([_f32(w.real), _f32(w.imag), _f32(-w.imag)]
                        ).reshape(3, 4, 128, 256)

    in_maps = []
    for core in range(8):
        b, g = core // 2, core % 2
        cols = slice(256 * g, 256 * (g + 1))
        wk_c = Wkv_eff[:, :C][:, cols] * SCALE
        wv_c = Wkv_eff[:, C:][:, cols]
        bk_c = bkv_full[:C][cols] * SCALE
        bv_c = bkv_full[C:][cols]
        xs_c = np.stack([x_re[b].T, x_im[b].T])  # [2, C, N]
        xsp = xs_c.reshape(2, C, HR, 2, HR, 2)
        xP = np.stack([xsp[:, :, :, p, :, q].reshape(2, C, NK)
                       for p in range(2) for q in range(2)], axis=1)
        wp_c = Wproj[256 * g:256 * (g + 1), :]  # [256, C]
        wp_pack = np.stack([_f32(wp_c.real), _f32(wp_c.imag),
                            _f32(-wp_c.imag)]).reshape(3, 2, 128, C)
        m = {
            "xP": _f32(xP.reshape(2, 16, 128, NK)),
            "wc": wc_pack,
            "srb": np.stack([_f32(sr_b.real), _f32(sr_b.imag)]),
            "ones": np.ones((1, 512), np.float32),
            "onesc": np.ones((128, 1), np.float32),
            "wq": planes4(Wq[:, cols]),
            "wk": planes4(wk_c),
            "wv": planes4(wv_c),
            "wp": wp_pack,
            "bkv": np.stack([
                np.stack([_f32(bk_c.real), _f32(bv_c.real)]),
                np.stack([_f32(bk_c.imag), _f32(bv_c.imag)]),
            ]),
        }
        in_maps.append(m)
    return in_maps


_NC_CACHE = None


def _get_nc():
    global _NC_CACHE
    if _NC_CACHE is None:
        _NC_CACHE = build_nc()
    return _NC_CACHE


def kernel(x_re, x_im, Wq, Wkv, Wproj, bproj, sr_w, sr_b, gain, bias, H, W):
    from concourse.bass_utils import run_bass_kernel_spmd

    nc = _get_nc()
    in_maps = host_prep(x_re, x_im, Wq, Wkv, Wproj, bproj, sr_w, sr_b, gain, bias)
    res = run_bass_kernel_spmd(nc, in_maps, list(range(8)))
    bproj = np.asarray(bproj)
    out = np.zeros((B, N, C), dtype=np.complex64)
    for b in range(B):
        # outT: [2, 4, 5, 128, 512] -> [2, 512 c, 2560 q-padded]
        p0 = res.results[2 * b]["outT"].astype(np.float32)
        p1 = res.results[2 * b + 1]["outT"].astype(np.float32)
        acc = p0 + p1  # [2, 4cc, 5qc, 128, 512]
        accf = acc.transpose(0, 1, 3, 2, 4).reshape(2, 512, 5 * 512)[:, :, :N]
        full = np.empty((N, C), np.complex64)
        full[_PERM, :] = (accf[0] + 1j * accf[1]).T
        out[b] = full + bproj[None, :]
    return out


# revision 36
# speedup vs baseline: 1.5368x; 1.0402x over previous
"""Trainium2 Bass kernel for complex-valued spatial-reduction attention.

x: [B=4, N=2304, C=512] complex64 (re/im f32 planes), H=W=48, 8 heads,
head_dim 64, sr_ratio 2 -> Nk=576.

Sharding: 8 cores = 4 batches x 2 head-groups (4 heads each). Each core:
sr-conv over full C, complex LayerNorm, q/k/v for its heads,
softmax(|q.k^T|) attention, attn @ v, partial output projection.
Host sums the two partials per batch and adds bproj.

Everything stays on-chip: xP (patch-permuted x) is resident in SBUF and
feeds both the conv and the q-projection (q columns come out in
(patch-pos, nk) order; the host unpermutes). q / attention-out / weights
are SBUF-resident, so HBM traffic is inputs + weights + output only.

Precision: the f32r matmul path (~tf32-class rounding) dominates the
error budget; the softmax s=re^2+im^2 chain runs in f16 which measures
as noise against that. ebuf/v/proj are f32r.
"""

import os
import contextlib

import numpy as np
import ml_dtypes

import concourse.bass as bass
import concourse.mybir as mybir
import concourse.tile as tile
from concourse import bacc
from concourse.masks import make_identity

BF16 = mybir.dt.bfloat16
F16 = mybir.dt.float16
F32 = mybir.dt.float32
F32R = mybir.dt.float32r
AF = mybir.ActivationFunctionType
ALU = mybir.AluOpType

B, N, C, HEADS, HD, SR = 4, 2304, 512, 8, 64, 2
NK = 576
HR = 24
EPS = 1e-5
SCALE = HD ** -0.5  # folded into Wk host-side

CHUNKS5 = [(0, 120), (120, 120), (240, 120), (360, 120), (480, 96)]
NKH = [(0, 288), (288, 288)]
K_CHUNKS = [(0, 128), (128, 128), (256, 128), (384, 128), (512, 64)]
Q_CHUNKS = [(0, 512), (512, 512), (1024, 512), (1536, 512), (2048, 256)]

DEBUG = bool(int(os.environ.get("KBUILD_DEBUG", "0")))
PHASES = int(os.environ.get("KBUILD_PHASES", "4"))
P1MASK = int(os.environ.get("KBUILD_P1", "7"))  # 1=conv 2=qproj 4=ln


def _r(ap):
    return ap.bitcast(F32R)


def build_nc():
    nc = bacc.Bacc("TRN2", target_bir_lowering=False, debug=False, num_devices=8)

    xP_d = nc.dram_tensor("xP", [2, 16, 128, NK], F32R, kind="ExternalInput")
    wc_d = nc.dram_tensor("wc", [2, 3, 16, 128, 256], F32R, kind="ExternalInput")
    srb_d = nc.dram_tensor("srb", [2, C], F32R, kind="ExternalInput")
    ones_d = nc.dram_tensor("ones", [1, 512], F32R, kind="ExternalInput")
    onesc_d = nc.dram_tensor("onesc", [128, 1], F32R, kind="ExternalInput")
    wq_d = nc.dram_tensor("wq", [3, 4, 128, 256], F32R, kind="ExternalInput")
    wk_d = nc.dram_tensor("wk", [3, 4, 128, 256], F32R, kind="ExternalInput")
    wv_d = nc.dram_tensor("wv", [3, 4, 128, 256], F32R, kind="ExternalInput")
    wp_d = nc.dram_tensor("wp", [3, 2, 128, 512], F32R, kind="ExternalInput")
    bkv_d = nc.dram_tensor("bkv", [2, 2, 256], F32R, kind="ExternalInput")
    # output: [plane, c-block, q-chunk, 128 c, 512 q] (q cols permuted (p4, nk))
    outT_d = nc.dram_tensor("outT", [2, 4, 5, 128, 512], F16, kind="ExternalOutput")
    dbg = {}
    if DEBUG:
        dbg["xnT"] = nc.dram_tensor("dbg_xnT", [2, C, NK], F32, kind="ExternalOutput")
        dbg["qT"] = nc.dram_tensor("dbg_qT", [2, 2, 128, N], F32, kind="ExternalOutput")
        dbg["kT"] = nc.dram_tensor("dbg_kT", [3, 256, NK], F32, kind="ExternalOutput")
        dbg["v"] = nc.dram_tensor("dbg_v", [128, 5 * 4 * 128], F32, kind="ExternalOutput")
        dbg["conv"] = nc.dram_tensor("dbg_conv", [2, NK, C], F32, kind="ExternalOutput")

    with tile.TileContext(nc) as tc:
        _body(nc, tc, xP_d, wc_d, srb_d, ones_d, onesc_d, wq_d, wk_d,
              wv_d, wp_d, bkv_d, outT_d, dbg)

    nc.compile()
    return nc


def _ln2(nc, work, stats, stg, sz):
    """Complex LayerNorm for one chunk; stg is [128, 2, 2, 256]
    (re|im half, conv C-pass, 256 cols) so re/im are each contiguous."""
    inv_c = 1.0 / C
    re_sb = stg[:, 0].rearrange("p a b -> p (a b)")
    im_sb = stg[:, 1].rearrange("p a b -> p (a b)")
    sum_r = stats.tile([128, 1], F32, tag="sum_r")
    sum_i = stats.tile([128, 1], F32, tag="sum_i")
    nc.vector.tensor_reduce(sum_r[:sz], re_sb[:sz], mybir.AxisListType.X, ALU.add)
    nc.vector.tensor_reduce(sum_i[:sz], im_sb[:sz], mybir.AxisListType.X, ALU.add)
    junk = work.tile([128, C], F32, tag="ln_a", bufs=1, name="junk")
    sxx = stats.tile([128, 1], F32, tag="sxx")
    sii = stats.tile([128, 1], F32, tag="sii")
    sxi = stats.tile([128, 1], F32, tag="sxi")
    nc.vector.tensor_mul(junk[:sz], re_sb[:sz], re_sb[:sz])
    nc.vector.tensor_reduce(sxx[:sz], junk[:sz], mybir.AxisListType.X, ALU.add)
    nc.vector.tensor_mul(junk[:sz], im_sb[:sz], im_sb[:sz])
    nc.vector.tensor_reduce(sii[:sz], junk[:sz], mybir.AxisListType.X, ALU.add)
    nc.vector.tensor_mul(junk[:sz], re_sb[:sz], im_sb[:sz])
    nc.vector.tensor_reduce(sxi[:sz], junk[:sz], mybir.AxisListType.X, ALU.add)
    mr = stats.tile([128, 1], F32, tag="mr")
    mi = stats.tile([128, 1], F32, tag="mi")
    nc.vector.tensor_scalar_mul(mr[:sz], sum_r[:sz], inv_c)
    nc.vector.tensor_scalar_mul(mi[:sz], sum_i[:sz], inv_c)
    vre = stats.tile([128, 1], F32, tag="vre")
    vim = stats.tile([128, 1], F32, tag="vim")
    tA = stats.tile([128, 1], F32, tag="tA")
    tB = stats.tile([128, 1], F32, tag="tB")
    nc.vector.tensor_sub(tA[:sz], sxx[:sz], sii[:sz])
    nc.vector.tensor_scalar_mul(tA[:sz], tA[:sz], inv_c)
    nc.vector.tensor_mul(vre[:sz], mr[:sz], mr[:sz])
    nc.vector.tensor_mul(tB[:sz], mi[:sz], mi[:sz])
    nc.vector.tensor_sub(vre[:sz], vre[:sz], tB[:sz])
    nc.vector.tensor_sub(vre[:sz], tA[:sz], vre[:sz])
    nc.vector.tensor_scalar_add(vre[:sz], vre[:sz], EPS)
    nc.vector.tensor_mul(tB[:sz], mr[:sz], mi[:sz])
    nc.vector.tensor_scalar_mul(tB[:sz], tB[:sz], 2.0)
    nc.vector.tensor_scalar_mul(vim[:sz], sxi[:sz], 2.0 * inv_c)
    nc.vector.tensor_sub(vim[:sz], vim[:sz], tB[:sz])
    # complex rsqrt of (vre + i vim): w = conj(sqrt(v)) / |v|
    r2 = stats.tile([128, 1], F32, tag="r2")
    nc.vector.tensor_mul(r2[:sz], vre[:sz], vre[:sz])
    nc.vector.tensor_mul(tB[:sz], vim[:sz], vim[:sz])
    nc.vector.tensor_add(r2[:sz], r2[:sz], tB[:sz])
    def _sqrt_newton(out, x, sc):
        # y0 = LUT sqrt(sc*x); y1 = 0.5*(y0 + sc*x/y0)  (one Newton step)
        y0 = stats.tile([128, 1], F32, tag="nw_y0")
        nc.scalar.activation(y0[:sz], x[:sz], AF.Sqrt, scale=sc)
        yr = stats.tile([128, 1], F32, tag="nw_yr")
        nc.vector.tensor_scalar_add(y0[:sz], y0[:sz], 1e-30)
        nc.vector.reciprocal(yr[:sz], y0[:sz])
        nc.vector.tensor_mul(yr[:sz], yr[:sz], x[:sz])
        if sc != 1.0:
            nc.vector.tensor_scalar_mul(yr[:sz], yr[:sz], sc)
        nc.vector.tensor_add(out[:sz], y0[:sz], yr[:sz])
        nc.vector.tensor_scalar_mul(out[:sz], out[:sz], 0.5)

    rr = stats.tile([128, 1], F32, tag="rr")
    _sqrt_newton(rr, r2, 1.0)  # |v|
    srt = stats.tile([128, 1], F32, tag="srt")
    sia = stats.tile([128, 1], F32, tag="sia")
    nc.vector.tensor_add(tA[:sz], rr[:sz], vre[:sz])
    _sqrt_newton(srt, tA, 0.5)  # Re sqrt(v)
    nc.vector.tensor_sub(tA[:sz], rr[:sz], vre[:sz])
    _sqrt_newton(sia, tA, 0.5)  # |Im sqrt(v)|
    sgn = stats.tile([128, 1], F32, tag="sgn")
    nc.scalar.activation(sgn[:sz], vim[:sz], AF.Sign)
    nc.vector.tensor_mul(sia[:sz], sia[:sz], sgn[:sz])
    rin = stats.tile([128, 1], F32, tag="rin")
    nc.vector.reciprocal(rin[:sz], rr[:sz])
    wr = stats.tile([128, 1], F32, tag="wr")
    wn = stats.tile([128, 1], F32, tag="wn")  # = -w_im
    nc.vector.tensor_mul(wr[:sz], srt[:sz], rin[:sz])
    nc.vector.tensor_mul(wn[:sz], sia[:sz], rin[:sz])
    # xn = w * (x - m), complex
    aT = work.tile([128, C], F32, tag="ln_a", bufs=1)
    bT = work.tile([128, C], F32, tag="ln_b", bufs=1)
    xnr = work.tile([128, C], F32, tag="ln_xnr", bufs=1)
    xni = work.tile([128, C], F32, tag="ln_xni", bufs=1)
    nc.vector.tensor_scalar(aT[:sz], re_sb[:sz], mr[:sz], wr[:sz],
                            ALU.subtract, ALU.mult)
    nc.vector.tensor_scalar(bT[:sz], im_sb[:sz], mi[:sz], wn[:sz],
                            ALU.subtract, ALU.mult)
    nc.vector.tensor_add(xnr[:sz], aT[:sz], bT[:sz])
    nc.vector.tensor_scalar(aT[:sz], re_sb[:sz], mr[:sz], wn[:sz],
                            ALU.subtract, ALU.mult)
    nc.vector.tensor_scalar(bT[:sz], im_sb[:sz], mi[:sz], wr[:sz],
                            ALU.subtract, ALU.mult)
    nc.vector.tensor_sub(xni[:sz], bT[:sz], aT[:sz])
    return xnr, xni


def _body(nc, tc, xP_d, wc_d, srb_d, ones_d, onesc_d, wq_d, wk_d, wv_d,
          wp_d, bkv_d, outT_d, dbg):
    ctx = contextlib.ExitStack()
    consts = ctx.enter_context(tc.tile_pool(name="consts", bufs=1))
    big = ctx.enter_context(tc.tile_pool(name="big", bufs=1))
    stats = ctx.enter_context(tc.tile_pool(name="stats", bufs=2))
    psum = ctx.enter_context(tc.tile_pool(name="psum", bufs=7, space="PSUM"))

    # ---- constants ----
    ident = consts.tile([128, 128], F32, tag="ident")
    make_identity(nc, ident)
    ones_col = consts.tile([128, 1], F32R, tag="ones_col")
    nc.sync.dma_start(ones_col[:], onesc_d[:, :])
    ones_row = consts.tile([1, 512], F32R, tag="ones_row")
    nc.sync.dma_start(ones_row[:], ones_d[:])
    nbias = consts.tile([128, 1], F32, tag="nbias")
    nc.vector.memset(nbias, -50.0)
    srb_re = consts.tile([1, C], F32R, tag="srb_re")
    srb_im = consts.tile([1, C], F32R, tag="srb_im")
    nc.sync.dma_start(srb_re[:], srb_d[0:1, :])
    nc.sync.dma_start(srb_im[:], srb_d[1:2, :])
    bk_re = consts.tile([1, 256], F32R, tag="bk_re")
    bk_im = consts.tile([1, 256], F32R, tag="bk_im")
    bv_re = consts.tile([1, 256], F32R, tag="bv_re")
    bv_im = consts.tile([1, 256], F32R, tag="bv_im")
    nc.sync.dma_start(bk_re[:], bkv_d[0:1, 0, :])
    nc.sync.dma_start(bv_re[:], bkv_d[0:1, 1, :])
    nc.sync.dma_start(bk_im[:], bkv_d[1:2, 0, :])
    nc.sync.dma_start(bv_im[:], bkv_d[1:2, 1, :])

    # ---- whole-kernel resident SBUF ----
    qTr_sb = big.tile([128, 2, N], F32R, tag="qTr_sb")
    qTi_sb = big.tile([128, 2, N], F32R, tag="qTi_sb")
    xnTr = big.tile([128, 4, NK], F32R, tag="xnTr")
    xnTi = big.tile([128, 4, NK], F32R, tag="xnTi")
    kTr = big.tile([128, 2, NK], F32R, tag="kTr")
    kTi = big.tile([128, 2, NK], F32R, tag="kTi")
    kTin = big.tile([128, 2, NK], F32R, tag="kTin")
    vpk = big.tile([128, 5, 4, 128], F32R, tag="vpk")
    if DEBUG:
        nc.vector.memset(vpk.bitcast(F32), 0.0)

    # =====================================================================
    # Phase 1: conv (2 C-half passes) + q-projection from resident xP + LN
    # =====================================================================
    with tc.tile_pool(name="xpp", bufs=1) as xpp, \
         tc.tile_pool(name="cwork", bufs=2) as cwork:
        xP = xpp.tile([128, 2, 16, NK], F32R, tag="xP")
        wq = xpp.tile([128, 3, 4, 256], F32R, tag="wq")
        stg = []
        for ch in range(5):
            # [half(re|im), cpass, 256]: re ends up contiguous in cols 0:512
            stg.append(xpp.tile([128, 2, 2, 256], F32, tag=f"stg{ch}",
                                name=f"stg{ch}"))

        # conv streams its own inputs: each kk's weight tiles + xP tiles
        # are issued together so the PE starts within a couple of tiles
        # instead of waiting behind the whole xP/wq prefetch.
        for cp in (range(2) if P1MASK & 1 else []):
            cps = []
            for ch, (t0, sz) in enumerate(CHUNKS5):
                cps.append(psum.tile([128, 512], F32, tag="bank",
                                     name=f"conv{cp}_{ch}"))
            for kk in range(16):
                wcr = cwork.tile([128, 256], F32R, tag="wc_r")
                wci = cwork.tile([128, 256], F32R, tag="wc_i")
                wcn = cwork.tile([128, 256], F32R, tag="wc_n")
                nc.sync.dma_start(wcr[:], wc_d[cp, 0, kk])
                nc.gpsimd.dma_start(wci[:], wc_d[cp, 1, kk])
                if cp == 0:
                    nc.sync.dma_start(xP[:, 0, kk, :], xP_d[0, kk])
                    nc.gpsimd.dma_start(xP[:, 1, kk, :], xP_d[1, kk])
                nc.sync.dma_start(wcn[:], wc_d[cp, 2, kk])
                st = kk == 0
                for ch, (t0, sz) in enumerate(CHUNKS5):
                    pat_r = xP[:, 0, kk, t0:t0 + sz]
                    pat_i = xP[:, 1, kk, t0:t0 + sz]
                    cpt = cps[ch]
                    # one accumulation group per bank: start only on the
                    # very first matmul, stop only on the last (im bias)
                    nc.tensor.matmul(cpt[:sz, 0:256], pat_r, wcr[:],
                                     start=st, stop=False)
                    nc.tensor.matmul(cpt[:sz, 256:512], pat_r, wci[:],
                                     start=False, stop=False)
                    nc.tensor.matmul(cpt[:sz, 0:256], pat_i, wcn[:],
                                     start=False, stop=False)
                    nc.tensor.matmul(cpt[:sz, 256:512], pat_i, wcr[:],
                                     start=False, stop=False)
            cs = slice(256 * cp, 256 * (cp + 1))
            for ch, (t0, sz) in enumerate(CHUNKS5):
                cpt = cps[ch]
                nc.tensor.matmul(cpt[:sz, 0:256], ones_row[:, :sz],
                                 srb_re[:, cs], start=False, stop=False)
                nc.tensor.matmul(cpt[:sz, 256:512], ones_row[:, :sz],
                                 srb_im[:, cs], start=False, stop=True)
            for ch, (t0, sz) in enumerate(CHUNKS5):
                # whole-bank copy: depends on every write, so it cannot race
                # the PE still accumulating into the other half
                bank_v = cps[ch].rearrange("p (a b) -> p a b", a=2)
                nc.vector.tensor_copy(stg[ch][:sz, :, cp, :], bank_v[:sz])

        if not (P1MASK & 1):
            for kk in range(16):
                nc.sync.dma_start(xP[:, 0, kk, :], xP_d[0, kk])
                nc.gpsimd.dma_start(xP[:, 1, kk, :], xP_d[1, kk])
        for pl in range(3):
            for cj in range(4):
                eng = nc.sync if (pl * 4 + cj) % 2 == 0 else nc.gpsimd
                eng.dma_start(wq[:, pl, cj, :], wq_d[pl, cj])

        # ---- q-projection from resident xP (PE; LN below runs on DVE) ----
        for p4 in (range(4) if P1MASK & 2 else []):
            for (n0, nn) in NKH:
                prs = []
                for half in range(2):
                    prs.append((psum.tile([128, 512], F32, tag="bank",
                                          name=f"qpr{half}"),
                                psum.tile([128, 512], F32, tag="bank",
                                          name=f"qpi{half}")))
                for cj in range(4):
                    kk = p4 * 4 + cj
                    xr = xP[:, 0, kk, n0:n0 + nn]
                    xi = xP[:, 1, kk, n0:n0 + nn]
                    st = cj == 0
                    sp = cj == 3
                    for half in range(2):
                        hs = slice(128 * half, 128 * (half + 1))
                        pr, pi = prs[half]
                        nc.tensor.matmul(pr[:, :nn], wq[:, 0, cj, hs], xr,
                                         start=st, stop=False)
                        nc.tensor.matmul(pi[:, :nn], wq[:, 0, cj, hs], xi,
                                         start=st, stop=False)
                        nc.tensor.matmul(pr[:, :nn], wq[:, 2, cj, hs], xi,
                                         start=False, stop=sp)
                        nc.tensor.matmul(pi[:, :nn], wq[:, 1, cj, hs], xr,
                                         start=False, stop=sp)
                for half in range(2):
                    pr, pi = prs[half]
                    q0 = p4 * NK + n0
                    nc.scalar.copy(qTr_sb[:, half, q0:q0 + nn], pr[:, :nn])
                    nc.scalar.copy(qTi_sb[:, half, q0:q0 + nn], pi[:, :nn])

        # ---- LayerNorm (DVE, overlaps q-proj PE) + transposes into xnT ----
        for ch, (t0, sz) in enumerate(CHUNKS5 if (P1MASK & 4 and P1MASK & 1) else []):
            xnr, xni = _ln2(nc, cwork, stats, stg[ch], sz)
            for cj in range(4):
                for src, dst in ((xnr, xnTr), (xni, xnTi)):
                    pt = psum.tile([128, 128], F32, tag="bank", name="tp")
                    nc.tensor.transpose(pt[:, :sz],
                                        src[:sz, 128 * cj:128 * (cj + 1)],
                                        ident[:sz, :sz])
                    nc.vector.tensor_copy(dst[:, cj, t0:t0 + sz], pt[:, :sz])

    if DEBUG:
        for cj in range(4):
            nc.sync.dma_start(dbg["xnT"][0, 128 * cj:128 * (cj + 1), :],
                              xnTr[:, cj, :].bitcast(F32))
            nc.sync.dma_start(dbg["xnT"][1, 128 * cj:128 * (cj + 1), :],
                              xnTi[:, cj, :].bitcast(F32))
        for half in range(2):
            nc.sync.dma_start(dbg["qT"][0, half, :, :],
                              qTr_sb[:, half, :].bitcast(F32))
            nc.sync.dma_start(dbg["qT"][1, half, :, :],
                              qTi_sb[:, half, :].bitcast(F32))

    # =====================================================================
    # Phases 2-4: kv projections, attention, fused output projection
    # =====================================================================
    with tc.tile_pool(name="wkv", bufs=1) as wkv, \
         tc.tile_pool(name="sm", bufs=2) as sm:
        wk = wkv.tile([128, 3, 4, 256], F32R, tag="wk")
        wv = wkv.tile([128, 3, 4, 256], F32R, tag="wv")
        wp = wkv.tile([128, 3, 2, 512], F32R, tag="wp")
        for pl in range(3):
            for cj in range(4):
                eng = nc.sync if cj % 2 == 0 else nc.gpsimd
                eng.dma_start(wk[:, pl, cj, :], wk_d[pl, cj])
                eng2 = nc.gpsimd if cj % 2 == 0 else nc.sync
                eng2.dma_start(wv[:, pl, cj, :], wv_d[pl, cj])
            for hp in range(2):
                nc.sync.dma_start(wp[:, pl, hp, :], wp_d[pl, hp])

        # ---- k^T ----
        for half in (range(2) if PHASES >= 2 else []):
            hs = slice(128 * half, 128 * (half + 1))
            for (n0, nn) in NKH:
                pr = psum.tile([128, 512], F32, tag="bank", name="kpr")
                pi = psum.tile([128, 512], F32, tag="bank", name="kpi")
                for cj in range(4):
                    st = cj == 0
                    nc.tensor.matmul(pr[:, :nn], wk[:, 0, cj, hs],
                                     xnTr[:, cj, n0:n0 + nn], start=st,
                                     stop=False)
                    nc.tensor.matmul(pi[:, :nn], wk[:, 0, cj, hs],
                                     xnTi[:, cj, n0:n0 + nn], start=st,
                                     stop=False)
                    nc.tensor.matmul(pr[:, :nn], wk[:, 2, cj, hs],
                                     xnTi[:, cj, n0:n0 + nn], start=False,
                                     stop=False)
                    nc.tensor.matmul(pi[:, :nn], wk[:, 1, cj, hs],
                                     xnTr[:, cj, n0:n0 + nn], start=False,
                                     stop=False)
                nc.tensor.matmul(pr[:, :nn], bk_re[:, hs], ones_row[:, :nn],
                                 start=False, stop=True)
                nc.tensor.matmul(pi[:, :nn], bk_im[:, hs], ones_row[:, :nn],
                                 start=False, stop=True)
                nc.vector.tensor_copy(kTr[:, half, n0:n0 + nn], pr[:, :nn])
                nc.vector.tensor_copy(kTi[:, half, n0:n0 + nn], pi[:, :nn])
                nc.vector.tensor_scalar_mul(kTin[:, half, n0:n0 + nn],
                                            pi[:, :nn], -1.0)

        # ---- v (row-major into vpk) ----
        for kcg in (((0, 1, 2), (3, 4)) if PHASES >= 2 else ()):
            pps = {}
            for kc in kcg:
                pps[kc] = (psum.tile([128, 512], F32, tag="bank",
                                     name=f"vpr{kc}"),
                           psum.tile([128, 512], F32, tag="bank",
                                     name=f"vpi{kc}"))
            for cj in range(4):
                st = cj == 0
                for kc in kcg:
                    k0, szk = K_CHUNKS[kc]
                    pr, pi = pps[kc]
                    nc.tensor.matmul(pr[:szk, :256], xnTr[:, cj, k0:k0 + szk],
                                     wv[:, 0, cj, :], start=st, stop=False)
                    nc.tensor.matmul(pi[:szk, :256], xnTr[:, cj, k0:k0 + szk],
                                     wv[:, 1, cj, :], start=st, stop=False)
                    nc.tensor.matmul(pr[:szk, :256], xnTi[:, cj, k0:k0 + szk],
                                     wv[:, 2, cj, :], start=False, stop=False)
                    nc.tensor.matmul(pi[:szk, :256], xnTi[:, cj, k0:k0 + szk],
                                     wv[:, 0, cj, :], start=False, stop=False)
            for kc in kcg:
                k0, szk = K_CHUNKS[kc]
                pr, pi = pps[kc]
                nc.tensor.matmul(pr[:szk, :256], ones_row[:, :szk], bv_re[:],
                                 start=False, stop=True)
                nc.tensor.matmul(pi[:szk, :256], ones_row[:, :szk], bv_im[:],
                                 start=False, stop=True)
                vr_v = pr[:szk, :256].rearrange("p (h d) -> p h d", h=4)
                vi_v = pi[:szk, :256].rearrange("p (h d) -> p h d", h=4)
                nc.vector.tensor_copy(vpk[:szk, kc, :, 0:64], vr_v)
                nc.vector.tensor_copy(vpk[:szk, kc, :, 64:128], vi_v)

        if DEBUG:
            for half in range(2):
                hs = slice(128 * half, 128 * (half + 1))
                nc.sync.dma_start(dbg["kT"][0, hs, :], kTr[:, half, :].bitcast(F32))
                nc.sync.dma_start(dbg["kT"][1, hs, :], kTi[:, half, :].bitcast(F32))
                nc.sync.dma_start(dbg["kT"][2, hs, :], kTin[:, half, :].bitcast(F32))
            nc.sync.dma_start(dbg["v"][:, :], vpk.rearrange("p a b c -> p (a b c)").bitcast(F32))

        # =================================================================
        # Phase 3: attention; softmax(|scores|) with f16 s-chain
        # =================================================================
        def emit_front(q0, nq, hp):
            stiles = {}
            for kc in range(5):
                k0, szk = K_CHUNKS[kc]
                s16 = sm.tile([128, 2, 512], F16, tag="s16", name=f"s{kc}",
                              bufs=6)
                for i in range(2):
                    rs = slice(64 * i, 64 * (i + 1))
                    sre = psum.tile([128, 512], F32, tag="bank")
                    sim = psum.tile([128, 512], F32, tag="bank")
                    nc.tensor.matmul(sre[:szk, :nq], kTr[rs, hp, k0:k0 + szk],
                                     qTr_sb[rs, hp, q0:q0 + nq], start=True,
                                     stop=False)
                    nc.tensor.matmul(sim[:szk, :nq], kTr[rs, hp, k0:k0 + szk],
                                     qTi_sb[rs, hp, q0:q0 + nq], start=True,
                                     stop=False)
                    nc.tensor.matmul(sre[:szk, :nq], kTin[rs, hp, k0:k0 + szk],
                                     qTi_sb[rs, hp, q0:q0 + nq], start=False,
                                     stop=True)
                    nc.tensor.matmul(sim[:szk, :nq], kTi[rs, hp, k0:k0 + szk],
                                     qTr_sb[rs, hp, q0:q0 + nq], start=False,
                                     stop=True)
                    s1 = sm.tile([128, 512], F16, tag="s1")
                    nc.scalar.activation(s1[:szk, :nq], sre[:szk, :nq],
                                         AF.Square)
                    if 2 * kc + i < 3:
                        # ACT/DVE balance: a few tiles square im on ACT too
                        s2t = sm.tile([128, 512], F16, tag="c2")
                        nc.scalar.activation(s2t[:szk, :nq], sim[:szk, :nq],
                                             AF.Square)
                        nc.vector.tensor_add(s16[:szk, i, :nq],
                                             s2t[:szk, :nq], s1[:szk, :nq])
                    else:
                        c2 = sm.tile([128, 512], F16, tag="c2")
                        nc.vector.tensor_copy(c2[:szk, :nq], sim[:szk, :nq])
                        nc.vector.tensor_mul(s16[:szk, i, :nq], c2[:szk, :nq],
                                             c2[:szk, :nq])
                        nc.vector.tensor_add(s16[:szk, i, :nq],
                                             s16[:szk, i, :nq], s1[:szk, :nq])
                stiles[kc] = s16
            # batched sqrt (one table load), then batched exp (one load);
            # |a| must be stored f32: f16 would add |a|*2^-11 logit noise
            abs_ = {}
            for kc in range(5):
                k0_, szk = K_CHUNKS[kc]
                ab = sm.tile([128, 2, 512], F32, tag="ab", name=f"ab{kc}",
                             bufs=5)
                nc.scalar.activation(ab[:szk, :, :nq],
                                     stiles[kc][:szk, :, :nq], AF.Sqrt)
                abs_[kc] = ab
            ebufs = {}
            for kc in range(5):
                k0_, szk = K_CHUNKS[kc]
                ebuf = sm.tile([128, 2, 512], F32R, tag="ebuf", name=f"eb{kc}",
                               bufs=6)
                # constant shift keeps exp sums in f32 range; softmax is
                # shift-invariant so the result is exact
                nc.scalar.activation(ebuf[:szk, :, :nq],
                                     abs_[kc][:szk, :, :nq], AF.Exp,
                                     bias=nbias[:szk])
                ebufs[kc] = ebuf
            return ebufs

        ostore = {}

        def emit_back(qi, q0, nq, hp, ebufs):
            op0 = psum.tile([128, 512], F32, tag="bank", name="op0")
            op1 = psum.tile([128, 512], F32, tag="bank", name="op1")
            dn0 = psum.tile([128, 512], F32, tag="bank", name="dn0")
            dn1 = psum.tile([128, 512], F32, tag="bank", name="dn1")
            ops = (op0, op1)
            dns = (dn0, dn1)
            for kc in range(5):
                k0, szk = K_CHUNKS[kc]
                ebuf = ebufs[kc]
                for i in range(2):
                    hh = 2 * hp + i
                    nc.tensor.matmul(ops[i][:, :nq], vpk[:szk, kc, hh, :],
                                     ebuf[:szk, i, :nq], start=kc == 0,
                                     stop=kc == 4)
                    nc.tensor.matmul(dns[i][:1, :nq], ones_col[:szk, :],
                                     ebuf[:szk, i, :nq], start=kc == 0,
                                     stop=kc == 4)
            otr = sm.tile([128, 512], F32R, tag="otr", bufs=3)
            oti = sm.tile([128, 512], F32R, tag="oti", bufs=3)
            for i in range(2):
                rh = sm.tile([1, 512], F32R, tag="lnd", name=f"rh{i}", bufs=1)
                with nc.allow_low_precision(reason="f32r out is full f32"):
                    nc.vector.reciprocal(rh[:, :nq], dns[i][:1, :nq])
                rbp = psum.tile([128, 512], F32, tag="bank")
                nc.tensor.matmul(rbp[:, :nq], ones_row[:1, :128], rh[:, :nq],
                                 start=True, stop=True)
                rb = sm.tile([128, 512], F32, tag="rb", bufs=2)
                nc.vector.tensor_copy(rb[:, :nq], rbp[:, :nq])
                rs = slice(64 * i, 64 * (i + 1))
                nc.vector.tensor_mul(otr[rs, :nq], ops[i][0:64, :nq],
                                     rb[0:64, :nq])
                nc.vector.tensor_mul(oti[rs, :nq], ops[i][64:128, :nq],
                                     rb[64:128, :nq])
            ostore[(qi, hp)] = (otr, oti)

        def emit_proj(qi, q0, nq):
            for cc in range(4):
                cs = slice(128 * cc, 128 * (cc + 1))
                pr = psum.tile([128, 512], F32, tag="bank")
                pi = psum.tile([128, 512], F32, tag="bank")
                for hp in range(2):
                    otr, oti = ostore[(qi, hp)]
                    st = hp == 0
                    sp = hp == 1
                    nc.tensor.matmul(pr[:, :nq], wp[:, 0, hp, cs],
                                     otr[:, :nq], start=st, stop=False)
                    nc.tensor.matmul(pi[:, :nq], wp[:, 0, hp, cs],
                                     oti[:, :nq], start=st, stop=False)
                    nc.tensor.matmul(pr[:, :nq], wp[:, 2, hp, cs],
                                     oti[:, :nq], start=False, stop=sp)
                    nc.tensor.matmul(pi[:, :nq], wp[:, 1, hp, cs],
                                     otr[:, :nq], start=False, stop=sp)
                o1 = sm.tile([128, 512], F16, tag="o1")
                o2 = sm.tile([128, 512], F16, tag="o2")
                nc.vector.tensor_copy(o1[:, :nq], pr[:, :nq])
                nc.vector.tensor_copy(o2[:, :nq], pi[:, :nq])
                eng = nc.sync if cc % 2 == 0 else nc.gpsimd
                eng.dma_start(outT_d[0, cc, qi, :, :nq], o1[:, :nq])
                eng2 = nc.gpsimd if cc % 2 == 0 else nc.sync
                eng2.dma_start(outT_d[1, cc, qi, :, :nq], o2[:, :nq])

        # software pipeline: next iteration's scores+softmax are emitted
        # before the previous iteration's attn@v / normalize / projection
        prev = None
        for qi, (q0, nq) in enumerate(Q_CHUNKS if PHASES >= 3 else []):
            for hp in range(2):
                ebufs = emit_front(q0, nq, hp)
                if prev is not None:
                    pqi, pq0, pnq, php, pebufs = prev
                    if PHASES >= 4:
                        emit_back(pqi, pq0, pnq, php, pebufs)
                        if php == 1:
                            emit_proj(pqi, pq0, pnq)
                prev = (qi, q0, nq, hp, ebufs)
        if prev is not None and PHASES >= 4:
            pqi, pq0, pnq, php, pebufs = prev
            emit_back(pqi, pq0, pnq, php, pebufs)
            emit_proj(pqi, pq0, pnq)

    ctx.close()


# =========================================================================
# Host side
# =========================================================================

def _f32(x):
    return np.ascontiguousarray(x, dtype=np.float32)


def _perm():
    """q-column permutation: permuted index (p4, nk) -> original n."""
    perm = np.empty(4 * NK, dtype=np.int64)
    for p4 in range(4):
        p, q = p4 // 2, p4 % 2
        for nk in range(NK):
            hi, wi = nk // HR, nk % HR
            perm[p4 * NK + nk] = (SR * hi + p) * (SR * HR) + SR * wi + q
    return perm


_PERM = _perm()


def host_prep(x_re, x_im, Wq, Wkv, Wproj, bproj, sr_w, sr_b, gain, bias):
    x_re = np.asarray(x_re)
    x_im = np.asarray(x_im)
    Wq = np.asarray(Wq)
    Wkv = np.asarray(Wkv)
    Wproj = np.asarray(Wproj)
    sr_w = np.asarray(sr_w)
    sr_b = np.asarray(sr_b)
    gain = np.asarray(gain)
    bias = np.asarray(bias)

    Wkv_eff = gain[:, None] * Wkv
    bkv_full = bias @ Wkv
    Wc = sr_w.transpose(2, 3, 1, 0).reshape(4 * C, C)
    # wc packed [cpass, plane, kk, 128, 256]
    wc_planes = np.stack([_f32(Wc.real), _f32(Wc.imag), _f32(-Wc.imag)])
    wc_pack = np.empty((2, 3, 16, 128, 256), np.float32)
    for cp in range(2):
        wc_pack[cp] = wc_planes[:, :, 256 * cp:256 * (cp + 1)].reshape(
            3, 16, 128, 256)

    def planes4(w):  # [C, 256] -> [3, 4, 128, 256]
        return np.stack([_f32(w.real), _f32(w.imag), _f32(-w.imag)]
                        ).reshape(3, 4, 128, 256)

    in_maps = []
    for core in range(8):
        b, g = core // 2, core % 2
        cols = slice(256 * g, 256 * (g + 1))
        wk_c = Wkv_eff[:, :C][:, cols] * SCALE
        wv_c = Wkv_eff[:, C:][:, cols]
        bk_c = bkv_full[:C][cols] * SCALE
        bv_c = bkv_full[C:][cols]
        xs_c = np.stack([x_re[b].T, x_im[b].T])  # [2, C, N]
        xsp = xs_c.reshape(2, C, HR, 2, HR, 2)
        xP = np.stack([xsp[:, :, :, p, :, q].reshape(2, C, NK)
                       for p in range(2) for q in range(2)], axis=1)
        wp_c = Wproj[256 * g:256 * (g + 1), :]  # [256, C]
        wp_pack = np.stack([_f32(wp_c.real), _f32(wp_c.imag),
                            _f32(-wp_c.imag)]).reshape(3, 2, 128, C)
        m = {
            "xP": _f32(xP.reshape(2, 16, 128, NK)),
            "wc": wc_pack,
            "srb": np.stack([_f32(sr_b.real), _f32(sr_b.imag)]),
            "ones": np.ones((1, 512), np.float32),
            "onesc": np.ones((128, 1), np.float32),
            "wq": planes4(Wq[:, cols]),
            "wk": planes4(wk_c),
            "wv": planes4(wv_c),
            "wp": wp_pack,
            "bkv": np.stack([
                np.stack([_f32(bk_c.real), _f32(bv_c.real)]),
                np.stack([_f32(bk_c.imag), _f32(bv_c.imag)]),
            ]),
        }
        in_maps.append(m)
    return in_maps


_NC_CACHE = None


def _get_nc():
    global _NC_CACHE
    if _NC_CACHE is None:
        _NC_CACHE = build_nc()
    return _NC_CACHE


def kernel(x_re, x_im, Wq, Wkv, Wproj, bproj, sr_w, sr_b, gain, bias, H, W):
    from concourse.bass_utils import run_bass_kernel_spmd

    nc = _get_nc()
    in_maps = host_prep(x_re, x_im, Wq, Wkv, Wproj, bproj, sr_w, sr_b, gain, bias)
    res = run_bass_kernel_spmd(nc, in_maps, list(range(8)))
    bproj = np.asarray(bproj)
    out = np.zeros((B, N, C), dtype=np.complex64)
    for b in range(B):
        # outT: [2, 4, 5, 128, 512] -> [2, 512 c, 2560 q-padded]
        p0 = res.results[2 * b]["outT"].astype(np.float32)
        p1 = res.results[2 * b + 1]["outT"].astype(np.float32)
        acc = p0 + p1  # [2, 4cc, 5qc, 128, 512]
        accf = acc.transpose(0, 1, 3, 2, 4).reshape(2, 512, 5 * 512)[:, :, :N]
        full = np.empty((N, C), np.complex64)
        full[_PERM, :] = (accf[0] + 1j * accf[1]).T
        out[b] = full + bproj[None, :]
    return out


# revision 37
# speedup vs baseline: 1.5543x; 1.0114x over previous
"""Trainium2 Bass kernel for complex-valued spatial-reduction attention.

x: [B=4, N=2304, C=512] complex64 (re/im f32 planes), H=W=48, 8 heads,
head_dim 64, sr_ratio 2 -> Nk=576.

Sharding: 8 cores = 4 batches x 2 head-groups (4 heads each). Each core:
sr-conv over full C, complex LayerNorm, q/k/v for its heads,
softmax(|q.k^T|) attention, attn @ v, partial output projection.
Host sums the two partials per batch and adds bproj.

Everything stays on-chip: xP (patch-permuted x) is resident in SBUF and
feeds both the conv and the q-projection (q columns come out in
(patch-pos, nk) order; the host unpermutes). q / attention-out / weights
are SBUF-resident, so HBM traffic is inputs + weights + output only.

Precision: the f32r matmul path (~tf32-class rounding) dominates the
error budget; the softmax s=re^2+im^2 chain runs in f16 which measures
as noise against that. ebuf/v/proj are f32r.
"""

import os
import contextlib

import numpy as np
import ml_dtypes

import concourse.bass as bass
import concourse.mybir as mybir
import concourse.tile as tile
from concourse import bacc
from concourse.masks import make_identity

BF16 = mybir.dt.bfloat16
F16 = mybir.dt.float16
F32 = mybir.dt.float32
F32R = mybir.dt.float32r
AF = mybir.ActivationFunctionType
ALU = mybir.AluOpType

B, N, C, HEADS, HD, SR = 4, 2304, 512, 8, 64, 2
NK = 576
HR = 24
EPS = 1e-5
SCALE = HD ** -0.5  # folded into Wk host-side

CHUNKS5 = [(0, 120), (120, 120), (240, 120), (360, 120), (480, 96)]
NKH = [(0, 288), (288, 288)]
K_CHUNKS = [(0, 128), (128, 128), (256, 128), (384, 128), (512, 64)]
Q_CHUNKS = [(0, 512), (512, 512), (1024, 512), (1536, 512), (2048, 256)]

DEBUG = bool(int(os.environ.get("KBUILD_DEBUG", "0")))
PHASES = int(os.environ.get("KBUILD_PHASES", "4"))
P1MASK = int(os.environ.get("KBUILD_P1", "7"))  # 1=conv 2=qproj 4=ln


def _r(ap):
    return ap.bitcast(F32R)


def build_nc():
    nc = bacc.Bacc("TRN2", target_bir_lowering=False, debug=False, num_devices=8)

    xP_d = nc.dram_tensor("xP", [2, 16, 128, NK], F32R, kind="ExternalInput")
    wc_d = nc.dram_tensor("wc", [2, 3, 16, 128, 256], F32R, kind="ExternalInput")
    srb_d = nc.dram_tensor("srb", [2, C], F32R, kind="ExternalInput")
    ones_d = nc.dram_tensor("ones", [1, 512], F32R, kind="ExternalInput")
    onesc_d = nc.dram_tensor("onesc", [128, 1], F32R, kind="ExternalInput")
    wq_d = nc.dram_tensor("wq", [3, 4, 128, 256], F32R, kind="ExternalInput")
    wk_d = nc.dram_tensor("wk", [3, 4, 128, 256], F32R, kind="ExternalInput")
    wv_d = nc.dram_tensor("wv", [3, 4, 128, 256], F32R, kind="ExternalInput")
    wp_d = nc.dram_tensor("wp", [3, 2, 128, 512], F32R, kind="ExternalInput")
    bkv_d = nc.dram_tensor("bkv", [2, 2, 256], F32R, kind="ExternalInput")
    # output: [plane, c-block, q-chunk, 128 c, 512 q] (q cols permuted (p4, nk))
    outT_d = nc.dram_tensor("outT", [2, 4, 5, 128, 512], F16, kind="ExternalOutput")
    dbg = {}
    if DEBUG:
        dbg["xnT"] = nc.dram_tensor("dbg_xnT", [2, C, NK], F32, kind="ExternalOutput")
        dbg["qT"] = nc.dram_tensor("dbg_qT", [2, 2, 128, N], F32, kind="ExternalOutput")
        dbg["kT"] = nc.dram_tensor("dbg_kT", [3, 256, NK], F32, kind="ExternalOutput")
        dbg["v"] = nc.dram_tensor("dbg_v", [128, 5 * 4 * 128], F32, kind="ExternalOutput")
        dbg["conv"] = nc.dram_tensor("dbg_conv", [2, NK, C], F32, kind="ExternalOutput")

    with tile.TileContext(nc) as tc:
        _body(nc, tc, xP_d, wc_d, srb_d, ones_d, onesc_d, wq_d, wk_d,
              wv_d, wp_d, bkv_d, outT_d, dbg)

    nc.compile()
    return nc


def _ln2(nc, work, stats, stg, sz):
    """Complex LayerNorm for one chunk; stg is [128, 2, 2, 256]
    (re|im half, conv C-pass, 256 cols) so re/im are each contiguous."""
    inv_c = 1.0 / C
    re_sb = stg[:, 0].rearrange("p a b -> p (a b)")
    im_sb = stg[:, 1].rearrange("p a b -> p (a b)")
    sum_r = stats.tile([128, 1], F32, tag="sum_r")
    sum_i = stats.tile([128, 1], F32, tag="sum_i")
    sxx = stats.tile([128, 1], F32, tag="sxx")
    sii = stats.tile([128, 1], F32, tag="sii")
    sxi = stats.tile([128, 1], F32, tag="sxi")
    # sums and sums-of-squares on the (otherwise idle) scalar engine via
    # accum_out; Identity/Square live in every table set, so no LUT loads.
    junkA = work.tile([128, C], F32, tag="ln_b", bufs=1, name="junkA")
    nc.scalar.activation(junkA[:sz], re_sb[:sz], AF.Identity,
                         accum_out=sum_r[:sz])
    nc.scalar.activation(junkA[:sz], im_sb[:sz], AF.Identity,
                         accum_out=sum_i[:sz])
    nc.scalar.activation(junkA[:sz], re_sb[:sz], AF.Square,
                         accum_out=sxx[:sz])
    nc.scalar.activation(junkA[:sz], im_sb[:sz], AF.Square,
                         accum_out=sii[:sz])
    junk = work.tile([128, C], F32, tag="ln_a", bufs=1, name="junk")
    nc.vector.tensor_mul(junk[:sz], re_sb[:sz], im_sb[:sz])
    nc.vector.tensor_reduce(sxi[:sz], junk[:sz], mybir.AxisListType.X, ALU.add)
    mr = stats.tile([128, 1], F32, tag="mr")
    mi = stats.tile([128, 1], F32, tag="mi")
    nc.vector.tensor_scalar_mul(mr[:sz], sum_r[:sz], inv_c)
    nc.vector.tensor_scalar_mul(mi[:sz], sum_i[:sz], inv_c)
    vre = stats.tile([128, 1], F32, tag="vre")
    vim = stats.tile([128, 1], F32, tag="vim")
    tA = stats.tile([128, 1], F32, tag="tA")
    tB = stats.tile([128, 1], F32, tag="tB")
    nc.vector.tensor_sub(tA[:sz], sxx[:sz], sii[:sz])
    nc.vector.tensor_scalar_mul(tA[:sz], tA[:sz], inv_c)
    nc.vector.tensor_mul(vre[:sz], mr[:sz], mr[:sz])
    nc.vector.tensor_mul(tB[:sz], mi[:sz], mi[:sz])
    nc.vector.tensor_sub(vre[:sz], vre[:sz], tB[:sz])
    nc.vector.tensor_sub(vre[:sz], tA[:sz], vre[:sz])
    nc.vector.tensor_scalar_add(vre[:sz], vre[:sz], EPS)
    nc.vector.tensor_mul(tB[:sz], mr[:sz], mi[:sz])
    nc.vector.tensor_scalar_mul(tB[:sz], tB[:sz], 2.0)
    nc.vector.tensor_scalar_mul(vim[:sz], sxi[:sz], 2.0 * inv_c)
    nc.vector.tensor_sub(vim[:sz], vim[:sz], tB[:sz])
    # complex rsqrt of (vre + i vim): w = conj(sqrt(v)) / |v|
    r2 = stats.tile([128, 1], F32, tag="r2")
    nc.vector.tensor_mul(r2[:sz], vre[:sz], vre[:sz])
    nc.vector.tensor_mul(tB[:sz], vim[:sz], vim[:sz])
    nc.vector.tensor_add(r2[:sz], r2[:sz], tB[:sz])
    def _sqrt_newton(out, x, sc):
        # y0 = LUT sqrt(sc*x); y1 = 0.5*(y0 + sc*x/y0)  (one Newton step)
        y0 = stats.tile([128, 1], F32, tag="nw_y0")
        nc.scalar.activation(y0[:sz], x[:sz], AF.Sqrt, scale=sc)
        yr = stats.tile([128, 1], F32, tag="nw_yr")
        nc.vector.tensor_scalar_add(y0[:sz], y0[:sz], 1e-30)
        nc.vector.reciprocal(yr[:sz], y0[:sz])
        nc.vector.tensor_mul(yr[:sz], yr[:sz], x[:sz])
        if sc != 1.0:
            nc.vector.tensor_scalar_mul(yr[:sz], yr[:sz], sc)
        nc.vector.tensor_add(out[:sz], y0[:sz], yr[:sz])
        nc.vector.tensor_scalar_mul(out[:sz], out[:sz], 0.5)

    rr = stats.tile([128, 1], F32, tag="rr")
    _sqrt_newton(rr, r2, 1.0)  # |v|
    srt = stats.tile([128, 1], F32, tag="srt")
    sia = stats.tile([128, 1], F32, tag="sia")
    nc.vector.tensor_add(tA[:sz], rr[:sz], vre[:sz])
    _sqrt_newton(srt, tA, 0.5)  # Re sqrt(v)
    nc.vector.tensor_sub(tA[:sz], rr[:sz], vre[:sz])
    _sqrt_newton(sia, tA, 0.5)  # |Im sqrt(v)|
    sgn = stats.tile([128, 1], F32, tag="sgn")
    nc.scalar.activation(sgn[:sz], vim[:sz], AF.Sign)
    nc.vector.tensor_mul(sia[:sz], sia[:sz], sgn[:sz])
    rin = stats.tile([128, 1], F32, tag="rin")
    nc.vector.reciprocal(rin[:sz], rr[:sz])
    wr = stats.tile([128, 1], F32, tag="wr")
    wn = stats.tile([128, 1], F32, tag="wn")  # = -w_im
    nc.vector.tensor_mul(wr[:sz], srt[:sz], rin[:sz])
    nc.vector.tensor_mul(wn[:sz], sia[:sz], rin[:sz])
    # xn = w * (x - m), complex
    aT = work.tile([128, C], F32, tag="ln_a", bufs=1)
    bT = work.tile([128, C], F32, tag="ln_b", bufs=1)
    xnr = work.tile([128, C], F32, tag="ln_xnr", bufs=1)
    xni = work.tile([128, C], F32, tag="ln_xni", bufs=1)
    nc.vector.tensor_scalar(aT[:sz], re_sb[:sz], mr[:sz], wr[:sz],
                            ALU.subtract, ALU.mult)
    nc.vector.tensor_scalar(bT[:sz], im_sb[:sz], mi[:sz], wn[:sz],
                            ALU.subtract, ALU.mult)
    nc.vector.tensor_add(xnr[:sz], aT[:sz], bT[:sz])
    nc.vector.tensor_scalar(aT[:sz], re_sb[:sz], mr[:sz], wn[:sz],
                            ALU.subtract, ALU.mult)
    nc.vector.tensor_scalar(bT[:sz], im_sb[:sz], mi[:sz], wr[:sz],
                            ALU.subtract, ALU.mult)
    nc.vector.tensor_sub(xni[:sz], bT[:sz], aT[:sz])
    return xnr, xni


def _body(nc, tc, xP_d, wc_d, srb_d, ones_d, onesc_d, wq_d, wk_d, wv_d,
          wp_d, bkv_d, outT_d, dbg):
    ctx = contextlib.ExitStack()
    consts = ctx.enter_context(tc.tile_pool(name="consts", bufs=1))
    big = ctx.enter_context(tc.tile_pool(name="big", bufs=1))
    stats = ctx.enter_context(tc.tile_pool(name="stats", bufs=2))
    psum = ctx.enter_context(tc.tile_pool(name="psum", bufs=7, space="PSUM"))

    # ---- constants ----
    ident = consts.tile([128, 128], F32, tag="ident")
    make_identity(nc, ident)
    ones_col = consts.tile([128, 1], F32R, tag="ones_col")
    nc.sync.dma_start(ones_col[:], onesc_d[:, :])
    ones_row = consts.tile([1, 512], F32R, tag="ones_row")
    nc.sync.dma_start(ones_row[:], ones_d[:])
    nbias = consts.tile([128, 1], F32, tag="nbias")
    nc.vector.memset(nbias, -50.0)
    srb_re = consts.tile([1, C], F32R, tag="srb_re")
    srb_im = consts.tile([1, C], F32R, tag="srb_im")
    nc.sync.dma_start(srb_re[:], srb_d[0:1, :])
    nc.sync.dma_start(srb_im[:], srb_d[1:2, :])
    bk_re = consts.tile([1, 256], F32R, tag="bk_re")
    bk_im = consts.tile([1, 256], F32R, tag="bk_im")
    bv_re = consts.tile([1, 256], F32R, tag="bv_re")
    bv_im = consts.tile([1, 256], F32R, tag="bv_im")
    nc.sync.dma_start(bk_re[:], bkv_d[0:1, 0, :])
    nc.sync.dma_start(bv_re[:], bkv_d[0:1, 1, :])
    nc.sync.dma_start(bk_im[:], bkv_d[1:2, 0, :])
    nc.sync.dma_start(bv_im[:], bkv_d[1:2, 1, :])

    # ---- whole-kernel resident SBUF ----
    qTr_sb = big.tile([128, 2, N], F32R, tag="qTr_sb")
    qTi_sb = big.tile([128, 2, N], F32R, tag="qTi_sb")
    xnTr = big.tile([128, 4, NK], F32R, tag="xnTr")
    xnTi = big.tile([128, 4, NK], F32R, tag="xnTi")
    kTr = big.tile([128, 2, NK], F32R, tag="kTr")
    kTi = big.tile([128, 2, NK], F32R, tag="kTi")
    kTin = big.tile([128, 2, NK], F32R, tag="kTin")
    vpk = big.tile([128, 5, 4, 128], F32R, tag="vpk")
    if DEBUG:
        nc.vector.memset(vpk.bitcast(F32), 0.0)

    # =====================================================================
    # Phase 1: conv (2 C-half passes) + q-projection from resident xP + LN
    # =====================================================================
    with tc.tile_pool(name="xpp", bufs=1) as xpp, \
         tc.tile_pool(name="cwork", bufs=2) as cwork:
        xP = xpp.tile([128, 2, 16, NK], F32R, tag="xP")
        wq = xpp.tile([128, 3, 4, 256], F32R, tag="wq")
        stg = []
        for ch in range(5):
            # [half(re|im), cpass, 256]: re ends up contiguous in cols 0:512
            stg.append(xpp.tile([128, 2, 2, 256], F32, tag=f"stg{ch}",
                                name=f"stg{ch}"))

        # conv streams its own inputs: each kk's weight tiles + xP tiles
        # are issued together so the PE starts within a couple of tiles
        # instead of waiting behind the whole xP/wq prefetch.
        for cp in (range(2) if P1MASK & 1 else []):
            cps = []
            for ch, (t0, sz) in enumerate(CHUNKS5):
                cps.append(psum.tile([128, 512], F32, tag="bank",
                                     name=f"conv{cp}_{ch}"))
            for kk in range(16):
                wcr = cwork.tile([128, 256], F32R, tag="wc_r")
                wci = cwork.tile([128, 256], F32R, tag="wc_i")
                wcn = cwork.tile([128, 256], F32R, tag="wc_n")
                nc.sync.dma_start(wcr[:], wc_d[cp, 0, kk])
                nc.gpsimd.dma_start(wci[:], wc_d[cp, 1, kk])
                if cp == 0:
                    nc.sync.dma_start(xP[:, 0, kk, :], xP_d[0, kk])
                    nc.gpsimd.dma_start(xP[:, 1, kk, :], xP_d[1, kk])
                nc.sync.dma_start(wcn[:], wc_d[cp, 2, kk])
                st = kk == 0
                for ch, (t0, sz) in enumerate(CHUNKS5):
                    pat_r = xP[:, 0, kk, t0:t0 + sz]
                    pat_i = xP[:, 1, kk, t0:t0 + sz]
                    cpt = cps[ch]
                    # one accumulation group per bank: start only on the
                    # very first matmul, stop only on the last (im bias)
                    nc.tensor.matmul(cpt[:sz, 0:256], pat_r, wcr[:],
                                     start=st, stop=False)
                    nc.tensor.matmul(cpt[:sz, 256:512], pat_r, wci[:],
                                     start=False, stop=False)
                    nc.tensor.matmul(cpt[:sz, 0:256], pat_i, wcn[:],
                                     start=False, stop=False)
                    nc.tensor.matmul(cpt[:sz, 256:512], pat_i, wcr[:],
                                     start=False, stop=False)
            cs = slice(256 * cp, 256 * (cp + 1))
            for ch, (t0, sz) in enumerate(CHUNKS5):
                cpt = cps[ch]
                nc.tensor.matmul(cpt[:sz, 0:256], ones_row[:, :sz],
                                 srb_re[:, cs], start=False, stop=False)
                nc.tensor.matmul(cpt[:sz, 256:512], ones_row[:, :sz],
                                 srb_im[:, cs], start=False, stop=True)
            for ch, (t0, sz) in enumerate(CHUNKS5):
                # whole-bank copy: depends on every write, so it cannot race
                # the PE still accumulating into the other half
                bank_v = cps[ch].rearrange("p (a b) -> p a b", a=2)
                nc.vector.tensor_copy(stg[ch][:sz, :, cp, :], bank_v[:sz])

        if not (P1MASK & 1):
            for kk in range(16):
                nc.sync.dma_start(xP[:, 0, kk, :], xP_d[0, kk])
                nc.gpsimd.dma_start(xP[:, 1, kk, :], xP_d[1, kk])
        for pl in range(3):
            for cj in range(4):
                eng = nc.sync if (pl * 4 + cj) % 2 == 0 else nc.gpsimd
                eng.dma_start(wq[:, pl, cj, :], wq_d[pl, cj])

        # ---- q-projection from resident xP (PE; LN below runs on DVE) ----
        for p4 in (range(4) if P1MASK & 2 else []):
            for (n0, nn) in NKH:
                prs = []
                for half in range(2):
                    prs.append((psum.tile([128, 512], F32, tag="bank",
                                          name=f"qpr{half}"),
                                psum.tile([128, 512], F32, tag="bank",
                                          name=f"qpi{half}")))
                for cj in range(4):
                    kk = p4 * 4 + cj
                    xr = xP[:, 0, kk, n0:n0 + nn]
                    xi = xP[:, 1, kk, n0:n0 + nn]
                    st = cj == 0
                    sp = cj == 3
                    for half in range(2):
                        hs = slice(128 * half, 128 * (half + 1))
                        pr, pi = prs[half]
                        nc.tensor.matmul(pr[:, :nn], wq[:, 0, cj, hs], xr,
                                         start=st, stop=False)
                        nc.tensor.matmul(pi[:, :nn], wq[:, 0, cj, hs], xi,
                                         start=st, stop=False)
                        nc.tensor.matmul(pr[:, :nn], wq[:, 2, cj, hs], xi,
                                         start=False, stop=sp)
                        nc.tensor.matmul(pi[:, :nn], wq[:, 1, cj, hs], xr,
                                         start=False, stop=sp)
                for half in range(2):
                    pr, pi = prs[half]
                    q0 = p4 * NK + n0
                    nc.scalar.copy(qTr_sb[:, half, q0:q0 + nn], pr[:, :nn])
                    nc.scalar.copy(qTi_sb[:, half, q0:q0 + nn], pi[:, :nn])

        # ---- LayerNorm (DVE, overlaps q-proj PE) + transposes into xnT ----
        for ch, (t0, sz) in enumerate(CHUNKS5 if (P1MASK & 4 and P1MASK & 1) else []):
            xnr, xni = _ln2(nc, cwork, stats, stg[ch], sz)
            for cj in range(4):
                for src, dst in ((xnr, xnTr), (xni, xnTi)):
                    pt = psum.tile([128, 128], F32, tag="bank", name="tp")
                    nc.tensor.transpose(pt[:, :sz],
                                        src[:sz, 128 * cj:128 * (cj + 1)],
                                        ident[:sz, :sz])
                    nc.vector.tensor_copy(dst[:, cj, t0:t0 + sz], pt[:, :sz])

    if DEBUG:
        for cj in range(4):
            nc.sync.dma_start(dbg["xnT"][0, 128 * cj:128 * (cj + 1), :],
                              xnTr[:, cj, :].bitcast(F32))
            nc.sync.dma_start(dbg["xnT"][1, 128 * cj:128 * (cj + 1), :],
                              xnTi[:, cj, :].bitcast(F32))
        for half in range(2):
            nc.sync.dma_start(dbg["qT"][0, half, :, :],
                              qTr_sb[:, half, :].bitcast(F32))
            nc.sync.dma_start(dbg["qT"][1, half, :, :],
                              qTi_sb[:, half, :].bitcast(F32))

    # =====================================================================
    # Phases 2-4: kv projections, attention, fused output projection
    # =====================================================================
    with tc.tile_pool(name="wkv", bufs=1) as wkv, \
         tc.tile_pool(name="sm", bufs=2) as sm:
        wk = wkv.tile([128, 3, 4, 256], F32R, tag="wk")
        wv = wkv.tile([128, 3, 4, 256], F32R, tag="wv")
        wp = wkv.tile([128, 3, 2, 512], F32R, tag="wp")
        for pl in range(3):
            for cj in range(4):
                eng = nc.sync if cj % 2 == 0 else nc.gpsimd
                eng.dma_start(wk[:, pl, cj, :], wk_d[pl, cj])
                eng2 = nc.gpsimd if cj % 2 == 0 else nc.sync
                eng2.dma_start(wv[:, pl, cj, :], wv_d[pl, cj])
            for hp in range(2):
                nc.sync.dma_start(wp[:, pl, hp, :], wp_d[pl, hp])

        # ---- k^T ----
        for half in (range(2) if PHASES >= 2 else []):
            hs = slice(128 * half, 128 * (half + 1))
            for (n0, nn) in NKH:
                pr = psum.tile([128, 512], F32, tag="bank", name="kpr")
                pi = psum.tile([128, 512], F32, tag="bank", name="kpi")
                for cj in range(4):
                    st = cj == 0
                    nc.tensor.matmul(pr[:, :nn], wk[:, 0, cj, hs],
                                     xnTr[:, cj, n0:n0 + nn], start=st,
                                     stop=False)
                    nc.tensor.matmul(pi[:, :nn], wk[:, 0, cj, hs],
                                     xnTi[:, cj, n0:n0 + nn], start=st,
                                     stop=False)
                    nc.tensor.matmul(pr[:, :nn], wk[:, 2, cj, hs],
                                     xnTi[:, cj, n0:n0 + nn], start=False,
                                     stop=False)
                    nc.tensor.matmul(pi[:, :nn], wk[:, 1, cj, hs],
                                     xnTr[:, cj, n0:n0 + nn], start=False,
                                     stop=False)
                nc.tensor.matmul(pr[:, :nn], bk_re[:, hs], ones_row[:, :nn],
                                 start=False, stop=True)
                nc.tensor.matmul(pi[:, :nn], bk_im[:, hs], ones_row[:, :nn],
                                 start=False, stop=True)
                nc.vector.tensor_copy(kTr[:, half, n0:n0 + nn], pr[:, :nn])
                nc.vector.tensor_copy(kTi[:, half, n0:n0 + nn], pi[:, :nn])
                nc.vector.tensor_scalar_mul(kTin[:, half, n0:n0 + nn],
                                            pi[:, :nn], -1.0)

        # ---- v (row-major into vpk) ----
        for kcg in (((0, 1, 2), (3, 4)) if PHASES >= 2 else ()):
            pps = {}
            for kc in kcg:
                pps[kc] = (psum.tile([128, 512], F32, tag="bank",
                                     name=f"vpr{kc}"),
                           psum.tile([128, 512], F32, tag="bank",
                                     name=f"vpi{kc}"))
            for cj in range(4):
                st = cj == 0
                for kc in kcg:
                    k0, szk = K_CHUNKS[kc]
                    pr, pi = pps[kc]
                    nc.tensor.matmul(pr[:szk, :256], xnTr[:, cj, k0:k0 + szk],
                                     wv[:, 0, cj, :], start=st, stop=False)
                    nc.tensor.matmul(pi[:szk, :256], xnTr[:, cj, k0:k0 + szk],
                                     wv[:, 1, cj, :], start=st, stop=False)
                    nc.tensor.matmul(pr[:szk, :256], xnTi[:, cj, k0:k0 + szk],
                                     wv[:, 2, cj, :], start=False, stop=False)
                    nc.tensor.matmul(pi[:szk, :256], xnTi[:, cj, k0:k0 + szk],
                                     wv[:, 0, cj, :], start=False, stop=False)
            for kc in kcg:
                k0, szk = K_CHUNKS[kc]
                pr, pi = pps[kc]
                nc.tensor.matmul(pr[:szk, :256], ones_row[:, :szk], bv_re[:],
                                 start=False, stop=True)
                nc.tensor.matmul(pi[:szk, :256], ones_row[:, :szk], bv_im[:],
                                 start=False, stop=True)
                vr_v = pr[:szk, :256].rearrange("p (h d) -> p h d", h=4)
                vi_v = pi[:szk, :256].rearrange("p (h d) -> p h d", h=4)
                nc.vector.tensor_copy(vpk[:szk, kc, :, 0:64], vr_v)
                nc.vector.tensor_copy(vpk[:szk, kc, :, 64:128], vi_v)

        if DEBUG:
            for half in range(2):
                hs = slice(128 * half, 128 * (half + 1))
                nc.sync.dma_start(dbg["kT"][0, hs, :], kTr[:, half, :].bitcast(F32))
                nc.sync.dma_start(dbg["kT"][1, hs, :], kTi[:, half, :].bitcast(F32))
                nc.sync.dma_start(dbg["kT"][2, hs, :], kTin[:, half, :].bitcast(F32))
            nc.sync.dma_start(dbg["v"][:, :], vpk.rearrange("p a b c -> p (a b c)").bitcast(F32))

        # =================================================================
        # Phase 3: attention; softmax(|scores|) with f16 s-chain
        # =================================================================
        def emit_front(q0, nq, hp):
            stiles = {}
            for kc in range(5):
                k0, szk = K_CHUNKS[kc]
                s16 = sm.tile([128, 2, 512], F16, tag="s16", name=f"s{kc}",
                              bufs=6)
                for i in range(2):
                    rs = slice(64 * i, 64 * (i + 1))
                    sre = psum.tile([128, 512], F32, tag="bank")
                    sim = psum.tile([128, 512], F32, tag="bank")
                    nc.tensor.matmul(sre[:szk, :nq], kTr[rs, hp, k0:k0 + szk],
                                     qTr_sb[rs, hp, q0:q0 + nq], start=True,
                                     stop=False)
                    nc.tensor.matmul(sim[:szk, :nq], kTr[rs, hp, k0:k0 + szk],
                                     qTi_sb[rs, hp, q0:q0 + nq], start=True,
                                     stop=False)
                    nc.tensor.matmul(sre[:szk, :nq], kTin[rs, hp, k0:k0 + szk],
                                     qTi_sb[rs, hp, q0:q0 + nq], start=False,
                                     stop=True)
                    nc.tensor.matmul(sim[:szk, :nq], kTi[rs, hp, k0:k0 + szk],
                                     qTr_sb[rs, hp, q0:q0 + nq], start=False,
                                     stop=True)
                    s1 = sm.tile([128, 512], F16, tag="s1")
                    nc.scalar.activation(s1[:szk, :nq], sre[:szk, :nq],
                                         AF.Square)
                    if 2 * kc + i < 3:
                        # ACT/DVE balance: a few tiles square im on ACT too
                        s2t = sm.tile([128, 512], F16, tag="c2")
                        nc.scalar.activation(s2t[:szk, :nq], sim[:szk, :nq],
                                             AF.Square)
                        nc.vector.tensor_add(s16[:szk, i, :nq],
                                             s2t[:szk, :nq], s1[:szk, :nq])
                    else:
                        c2 = sm.tile([128, 512], F16, tag="c2")
                        nc.vector.tensor_copy(c2[:szk, :nq], sim[:szk, :nq])
                        nc.vector.tensor_mul(s16[:szk, i, :nq], c2[:szk, :nq],
                                             c2[:szk, :nq])
                        nc.vector.tensor_add(s16[:szk, i, :nq],
                                             s16[:szk, i, :nq], s1[:szk, :nq])
                stiles[kc] = s16
            # batched sqrt (one table load), then batched exp (one load);
            # |a| must be stored f32: f16 would add |a|*2^-11 logit noise
            abs_ = {}
            for kc in range(5):
                k0_, szk = K_CHUNKS[kc]
                ab = sm.tile([128, 2, 512], F32, tag="ab", name=f"ab{kc}",
                             bufs=5)
                nc.scalar.activation(ab[:szk, :, :nq],
                                     stiles[kc][:szk, :, :nq], AF.Sqrt)
                abs_[kc] = ab
            ebufs = {}
            for kc in range(5):
                k0_, szk = K_CHUNKS[kc]
                ebuf = sm.tile([128, 2, 512], F32R, tag="ebuf", name=f"eb{kc}",
                               bufs=6)
                # constant shift keeps exp sums in f32 range; softmax is
                # shift-invariant so the result is exact
                nc.scalar.activation(ebuf[:szk, :, :nq],
                                     abs_[kc][:szk, :, :nq], AF.Exp,
                                     bias=nbias[:szk])
                ebufs[kc] = ebuf
            return ebufs

        ostore = {}

        def emit_back(qi, q0, nq, hp, ebufs):
            op0 = psum.tile([128, 512], F32, tag="bank", name="op0")
            op1 = psum.tile([128, 512], F32, tag="bank", name="op1")
            dn0 = psum.tile([128, 512], F32, tag="bank", name="dn0")
            dn1 = psum.tile([128, 512], F32, tag="bank", name="dn1")
            ops = (op0, op1)
            dns = (dn0, dn1)
            for kc in range(5):
                k0, szk = K_CHUNKS[kc]
                ebuf = ebufs[kc]
                for i in range(2):
                    hh = 2 * hp + i
                    nc.tensor.matmul(ops[i][:, :nq], vpk[:szk, kc, hh, :],
                                     ebuf[:szk, i, :nq], start=kc == 0,
                                     stop=kc == 4)
                    nc.tensor.matmul(dns[i][:1, :nq], ones_col[:szk, :],
                                     ebuf[:szk, i, :nq], start=kc == 0,
                                     stop=kc == 4)
            otr = sm.tile([128, 512], F32R, tag="otr", bufs=3)
            oti = sm.tile([128, 512], F32R, tag="oti", bufs=3)
            for i in range(2):
                rh = sm.tile([1, 512], F32R, tag="lnd", name=f"rh{i}", bufs=1)
                with nc.allow_low_precision(reason="f32r out is full f32"):
                    nc.vector.reciprocal(rh[:, :nq], dns[i][:1, :nq])
                rbp = psum.tile([128, 512], F32, tag="bank")
                nc.tensor.matmul(rbp[:, :nq], ones_row[:1, :128], rh[:, :nq],
                                 start=True, stop=True)
                rb = sm.tile([128, 512], F32, tag="rb", bufs=2)
                nc.vector.tensor_copy(rb[:, :nq], rbp[:, :nq])
                rs = slice(64 * i, 64 * (i + 1))
                nc.vector.tensor_mul(otr[rs, :nq], ops[i][0:64, :nq],
                                     rb[0:64, :nq])
                nc.vector.tensor_mul(oti[rs, :nq], ops[i][64:128, :nq],
                                     rb[64:128, :nq])
            ostore[(qi, hp)] = (otr, oti)

        def emit_proj(qi, q0, nq):
            for cc in range(4):
                cs = slice(128 * cc, 128 * (cc + 1))
                pr = psum.tile([128, 512], F32, tag="bank")
                pi = psum.tile([128, 512], F32, tag="bank")
                for hp in range(2):
                    otr, oti = ostore[(qi, hp)]
                    st = hp == 0
                    sp = hp == 1
                    nc.tensor.matmul(pr[:, :nq], wp[:, 0, hp, cs],
                                     otr[:, :nq], start=st, stop=False)
                    nc.tensor.matmul(pi[:, :nq], wp[:, 0, hp, cs],
                                     oti[:, :nq], start=st, stop=False)
                    nc.tensor.matmul(pr[:, :nq], wp[:, 2, hp, cs],
                                     oti[:, :nq], start=False, stop=sp)
                    nc.tensor.matmul(pi[:, :nq], wp[:, 1, hp, cs],
                                     otr[:, :nq], start=False, stop=sp)
                o1 = sm.tile([128, 512], F16, tag="o1")
                o2 = sm.tile([128, 512], F16, tag="o2")
                nc.vector.tensor_copy(o1[:, :nq], pr[:, :nq])
                nc.vector.tensor_copy(o2[:, :nq], pi[:, :nq])
                eng = nc.sync if cc % 2 == 0 else nc.gpsimd
                eng.dma_start(outT_d[0, cc, qi, :, :nq], o1[:, :nq])
                eng2 = nc.gpsimd if cc % 2 == 0 else nc.sync
                eng2.dma_start(outT_d[1, cc, qi, :, :nq], o2[:, :nq])

        # software pipeline: next iteration's scores+softmax are emitted
        # before the previous iteration's attn@v / normalize / projection
        prev = None
        for qi, (q0, nq) in enumerate(Q_CHUNKS if PHASES >= 3 else []):
            for hp in range(2):
                ebufs = emit_front(q0, nq, hp)
                if prev is not None:
                    pqi, pq0, pnq, php, pebufs = prev
                    if PHASES >= 4:
                        emit_back(pqi, pq0, pnq, php, pebufs)
                        if php == 1:
                            emit_proj(pqi, pq0, pnq)
                prev = (qi, q0, nq, hp, ebufs)
        if prev is not None and PHASES >= 4:
            pqi, pq0, pnq, php, pebufs = prev
            emit_back(pqi, pq0, pnq, php, pebufs)
            emit_proj(pqi, pq0, pnq)

    ctx.close()


# =========================================================================
# Host side
# =========================================================================

def _f32(x):
    return np.ascontiguousarray(x, dtype=np.float32)


def _perm():
    """q-column permutation: permuted index (p4, nk) -> original n."""
    perm = np.empty(4 * NK, dtype=np.int64)
    for p4 in range(4):
        p, q = p4 // 2, p4 % 2
        for nk in range(NK):
            hi, wi = nk // HR, nk % HR
            perm[p4 * NK + nk] = (SR * hi + p) * (SR * HR) + SR * wi + q
    return perm


_PERM = _perm()


def host_prep(x_re, x_im, Wq, Wkv, Wproj, bproj, sr_w, sr_b, gain, bias):
    x_re = np.asarray(x_re)
    x_im = np.asarray(x_im)
    Wq = np.asarray(Wq)
    Wkv = np.asarray(Wkv)
    Wproj = np.asarray(Wproj)
    sr_w = np.asarray(sr_w)
    sr_b = np.asarray(sr_b)
    gain = np.asarray(gain)
    bias = np.asarray(bias)

    Wkv_eff = gain[:, None] * Wkv
    bkv_full = bias @ Wkv
    Wc = sr_w.transpose(2, 3, 1, 0).reshape(4 * C, C)
    # wc packed [cpass, plane, kk, 128, 256]
    wc_planes = np.stack([_f32(Wc.real), _f32(Wc.imag), _f32(-Wc.imag)])
    wc_pack = np.empty((2, 3, 16, 128, 256), np.float32)
    for cp in range(2):
        wc_pack[cp] = wc_planes[:, :, 256 * cp:256 * (cp + 1)].reshape(
            3, 16, 128, 256)

    def planes4(w):  # [C, 256] -> [3, 4, 128, 256]
        return np.stack([_f32(w.real), _f32(w.imag), _f32(-w.imag)]
                        ).reshape(3, 4, 128, 256)

    in_maps = []
    for core in range(8):
        b, g = core // 2, core % 2
        cols = slice(256 * g, 256 * (g + 1))
        wk_c = Wkv_eff[:, :C][:, cols] * SCALE
        wv_c = Wkv_eff[:, C:][:, cols]
        bk_c = bkv_full[:C][cols] * SCALE
        bv_c = bkv_full[C:][cols]
        xs_c = np.stack([x_re[b].T, x_im[b].T])  # [2, C, N]
        xsp = xs_c.reshape(2, C, HR, 2, HR, 2)
        xP = np.stack([xsp[:, :, :, p, :, q].reshape(2, C, NK)
                       for p in range(2) for q in range(2)], axis=1)
        wp_c = Wproj[256 * g:256 * (g + 1), :]  # [256, C]
        wp_pack = np.stack([_f32(wp_c.real), _f32(wp_c.imag),
                            _f32(-wp_c.imag)]).reshape(3, 2, 128, C)
        m = {
            "xP": _f32(xP.reshape(2, 16, 128, NK)),
            "wc": wc_pack,
            "srb": np.stack([_f32(sr_b.real), _f32(sr_b.imag)]),
            "ones": np.ones((1, 512), np.float32),
            "onesc": np.ones((128, 1), np.float32),
            "wq": planes4(Wq[:, cols]),
            "wk": planes4(wk_c),
            "wv": planes4(wv_c),
            "wp": wp_pack,
            "bkv": np.stack([
                np.stack([_f32(bk_c.real), _f32(bv_c.real)]),
                np.stack([_f32(bk_c.imag), _f32(bv_c.imag)]),
            ]),
        }
        in_maps.append(m)
    return in_maps


_NC_CACHE = None


def _get_nc():
    global _NC_CACHE
    if _NC_CACHE is None:
        _NC_CACHE = build_nc()
    return _NC_CACHE


def kernel(x_re, x_im, Wq, Wkv, Wproj, bproj, sr_w, sr_b, gain, bias, H, W):
    from concourse.bass_utils import run_bass_kernel_spmd

    nc = _get_nc()
    in_maps = host_prep(x_re, x_im, Wq, Wkv, Wproj, bproj, sr_w, sr_b, gain, bias)
    res = run_bass_kernel_spmd(nc, in_maps, list(range(8)))
    bproj = np.asarray(bproj)
    out = np.zeros((B, N, C), dtype=np.complex64)
    for b in range(B):
        # outT: [2, 4, 5, 128, 512] -> [2, 512 c, 2560 q-padded]
        p0 = res.results[2 * b]["outT"].astype(np.float32)
        p1 = res.results[2 * b + 1]["outT"].astype(np.float32)
        acc = p0 + p1  # [2, 4cc, 5qc, 128, 512]
        accf = acc.transpose(0, 1, 3, 2, 4).reshape(2, 512, 5 * 512)[:, :, :N]
        full = np.empty((N, C), np.complex64)
        full[_PERM, :] = (accf[0] + 1j * accf[1]).T
        out[b] = full + bproj[None, :]
    return out
